# revision 23
# baseline (speedup 1.0000x reference)
"""Distributed Trainium2 kernel for the linear-attention transformer block.

Math (per batch element b):
  Q = elu(x @ Wq + bq), K = elu(x @ Wk + bk), V = x @ Wv + bv   (per-head d=64)
  KV_h = K_h^T V_h  [64,64];  Ksum_h = sum_n K_h[n]  [64]
  attn_h = (Q_h @ KV_h) / (Q_h . Ksum_h)
  out = LayerNorm(x + attn @ Wo + bo) * gamma + beta

Sharding: 16384 tokens over 8 cores (2048 each; core c owns batch c//2,
half c%2). Each core computes Q/K/V only for its tokens, partial KV/Ksum,
then a 266KB AllReduce over core pairs {2b, 2b+1} completes the KV stats;
attention + output projection + LayerNorm finish locally.

Precision: the 1/(Q.Ksum) denominators pass near zero (min |den| on the
nominal instance is ~0.35) and amplify projection noise into sign flips,
so the Q projection uses a 3-term bf16 split (x_hi@W_hi + x_hi@W_lo +
x_lo@W_hi; per-element err ~5e-6). The K/V projections run as single
float32r matmuls (full PE rate at N=512; measured HW err ~1.5e-4), which
keeps the Ksum-side denominator error ~0.07 rms - flip-safe with >5 sigma
margin. Ksum and the denominator run in f32. ~1.7e-3 global rel err.
"""

import sys

sys.path.insert(0, "/opt/trn_rl_repo")

import numpy as np
import ml_dtypes

import concourse.bass as bass
import concourse.bass_isa as bass_isa
import concourse.mybir as mybir
import concourse.tile as tile
from concourse import bacc
from concourse.bass_utils import run_bass_kernel_spmd

AF = mybir.ActivationFunctionType
OP = mybir.AluOpType
F32 = mybir.dt.float32
F32R = mybir.dt.float32r
BF16 = mybir.dt.bfloat16

B, N, D = 4, 4096, 1024
H, HD = 16, 64
TOK = 2048            # tokens per core
NCORES = 8
LN_EPS = 1e-3
P = 128
KC = D // P           # 8 contraction chunks
TC = TOK // P         # 16 token chunks of 128
TQ = TOK // 512       # 4 token chunks of 512
WARM_MM = 10          # PE clock-gate warmup matmuls

LAST_RESULT = None    # BassKernelResults of the most recent run (for test.py)
DEBUG_TAPS = False    # set True (e.g. from debug.py) to add intermediate outputs


def _build(apply_bias, apply_gamma, apply_beta):
    nc = bacc.Bacc("TRN2", target_bir_lowering=False, debug=False, num_devices=NCORES)

    def din(name, shape, dtype=BF16):
        return nc.dram_tensor(name, shape, dtype, kind="ExternalInput")

    xtf = din("xtf", [D, TOK], F32R)
    xthi = din("xthi", [D, TOK])
    xtlo = din("xtlo", [D, TOK])
    wqh = din("wqh", [D, D])
    wql = din("wql", [D, D])
    wk = din("wk", [D, D], F32R)
    wv = din("wv", [D, D], F32R)
    woh = din("woh", [D, D])
    xres = din("xres", [TOK, D], F32)
    e_sel = din("e_sel", [2, P])
    if apply_bias:
        bq_d = din("bq", [D], F32)
        bk_d = din("bk", [D], F32)
        bv_d = din("bv", [D], F32)
        bo_d = din("bo", [D], F32)
    if apply_gamma:
        gamma_d = din("gamma", [D], F32)
    if apply_beta:
        beta_d = din("beta", [D], F32)
    out_d = nc.dram_tensor("out", [TOK, D], F32, kind="ExternalOutput")
    if DEBUG_TAPS:
        dbg_k0 = nc.dram_tensor("dbg_k0", [P, 512], F32, kind="ExternalOutput")
        dbg_kv = nc.dram_tensor("dbg_kv", [P, 512], F32, kind="ExternalOutput")
        dbg_ksum = nc.dram_tensor("dbg_ksum", [P, KC], F32, kind="ExternalOutput")
        dbg_ar = nc.dram_tensor("dbg_ar", [P, 520], F32, kind="ExternalOutput")
        dbg_qt0 = nc.dram_tensor("dbg_qt0", [P, TOK], F32, kind="ExternalOutput")
        dbg_at = nc.dram_tensor("dbg_at", [P, KC, TOK], F32, kind="ExternalOutput")

    r8 = lambda t: t.ap().rearrange("(ko p) n -> p ko n", p=P)

    def bcast_row(dram_vec, sb_tile):
        # DMA-broadcast a [D] vector to [P, D] (stride-0 partition dim).
        src = bass.AP(
            tensor=dram_vec.ap().tensor,
            offset=dram_vec.ap().offset,
            ap=[[0, P]] + list(dram_vec.ap().ap),
        )
        nc.sync.dma_start(out=sb_tile, in_=src)

    with tile.TileContext(nc) as tc:
        with (
            tc.tile_pool(name="smalls", bufs=1) as smalls,
            tc.tile_pool(name="dram", bufs=1, space="DRAM") as dram,
        ):
            e_sb = smalls.tile([2, P], BF16)
            nc.sync.dma_start(e_sb[:], e_sel.ap())
            ones_sb = smalls.tile([P, 1], F32)
            nc.vector.memset(ones_sb[:], 1.0)
            eps_sb = smalls.tile([P, 1], F32)
            nc.vector.memset(eps_sb[:], LN_EPS)
            if apply_bias:
                bq_sb = smalls.tile([P, KC], F32)   # per-partition layout for Q^T
                nc.sync.dma_start(bq_sb[:], bq_d.ap().rearrange("(ko p) -> p ko", p=P))
                bk_b = smalls.tile([P, D], F32)
                bv_b = smalls.tile([P, D], F32)
                bo_b = smalls.tile([P, D], F32)
                bcast_row(bk_d, bk_b[:])
                bcast_row(bv_d, bv_b[:])
                bcast_row(bo_d, bo_b[:])
            if apply_gamma:
                gamma_b = smalls.tile([P, D], F32)
                bcast_row(gamma_d, gamma_b[:])
            if apply_beta:
                beta_b = smalls.tile([P, D], F32)
                bcast_row(beta_d, beta_b[:])

            if DEBUG_TAPS:
                kv_loc = smalls.tile([P, 512], F32)
                ksum_loc = smalls.tile([P, KC], F32)

            # PE warmup: matmuls on zeroed tiles release the HAM clock
            # gate (1.2->2.4 GHz) while the first input DMAs are in flight.
            with (
                tc.tile_pool(name="warmsb", bufs=1) as warmsb,
                tc.tile_pool(name="warmps", bufs=2, space="PSUM") as warmps,
            ):
                warm_a = warmsb.tile([P, P], BF16)
                warm_b = warmsb.tile([P, 512], BF16)
                nc.gpsimd.memset(warm_a[:], 0.0)
                nc.gpsimd.memset(warm_b[:], 0.0)
                for w in range(WARM_MM):
                    wp = warmps.tile([P, 512], F32, tag="warm", name=f"warm_{w}")
                    nc.tensor.matmul(wp[:], warm_a[:], warm_b[:], start=True, stop=True)

            # Prefetch the first two Q-weight slices; their DMAs have no deps
            # and fill otherwise-idle DMA time during phase 1.
            wqp_cm = tc.tile_pool(name="wqp", bufs=3)
            wqp = wqp_cm.__enter__()
            wq_tiles = {}

            def load_wq(hp):
                msl = slice(hp * P, (hp + 1) * P)
                wq_t = wqp.tile([P, KC, 2, P], BF16, tag="wq", name=f"wq_{hp}")
                nc.sync.dma_start(wq_t[:, :, 0, :], r8(wqh)[:, :, msl])
                nc.sync.dma_start(wq_t[:, :, 1, :], r8(wql)[:, :, msl])
                wq_tiles[hp] = wq_t

            # ================= Phase 1: K, V, partial KV + Ksum =================
            # K = x @ Wk and V = x @ Wv as single f32r matmuls (full PE rate
            # at N=512). x^T is resident in f32; the four psum accumulators
            # (K/V x dh halves) share each stationary x^T chunk.
            with (
                tc.tile_pool(name="ph1x", bufs=1) as ph1x,
                tc.tile_pool(name="wkv", bufs=1) as wkv,
                tc.tile_pool(name="kvps_pool", bufs=1, space="PSUM") as kvps_pool,
                tc.tile_pool(name="ph1ps", bufs=5, space="PSUM") as ph1ps,
                tc.tile_pool(name="ph1sb", bufs=4) as ph1sb,
            ):
                xtf_sb = ph1x.tile([P, KC, TOK], F32R)
                wk_sb = wkv.tile([P, KC, D], F32R)
                wv_sb = wkv.tile([P, KC, D], F32R)
                # DMA order: first t=0..1 x chunks + Wk (needed by t=0 K
                # matmuls), then Wv, then remaining x token-sliced t-major.
                for k in range(KC):
                    nc.sync.dma_start(wk_sb[:, k, :], r8(wk)[:, k, :])
                    nc.sync.dma_start(xtf_sb[:, k, 0:2 * P], r8(xtf)[:, k, 0:2 * P])
                for k in range(KC):
                    nc.sync.dma_start(wv_sb[:, k, :], r8(wv)[:, k, :])
                for t in range(2, TC):
                    ts = slice(t * P, (t + 1) * P)
                    for k in range(KC):
                        nc.sync.dma_start(xtf_sb[:, k, ts], r8(xtf)[:, k, ts])
                load_wq(0)
                load_wq(1)

                # SBUF accumulator (DVE-add per token chunk; interleaved
                # multi-chunk PSUM accumulation groups proved unreliable).
                # Layout per dh half: [dh*260, dh*260+256) = KV, [+256, +260) = Ksum.
                acc = smalls.tile([P, 520], F32)
                nc.vector.memset(acc[:], 0.0)

                for t in range(TC):
                    ts = slice(t * P, (t + 1) * P)
                    kps = {}
                    vps = {}
                    for dh in range(2):
                        kps[dh] = ph1ps.tile([P, 512], F32, tag="proj",
                                             name=f"kps_{t}_{dh}")
                        vps[dh] = ph1ps.tile([P, 512], F32, tag="proj",
                                             name=f"vps_{t}_{dh}")
                    for k in range(KC):
                        st, sp = (k == 0), (k == KC - 1)
                        for dh in range(2):
                            dsl = slice(dh * 512, (dh + 1) * 512)
                            nc.tensor.matmul(kps[dh][:], xtf_sb[:, k, ts],
                                             wk_sb[:, k, dsl], start=st, stop=sp)
                            nc.tensor.matmul(vps[dh][:], xtf_sb[:, k, ts],
                                             wv_sb[:, k, dsl], start=st, stop=sp)
                    kb_chunks = []
                    kvs_tiles = {}
                    for dh in range(2):
                        dsl = slice(dh * 512, (dh + 1) * 512)
                        if apply_bias:
                            kraw = ph1sb.tile([P, 512], F32, tag="kraw", name=f"kraw_{t}_{dh}")
                            nc.vector.tensor_tensor(kraw[:], kps[dh][:], bk_b[:, dsl], OP.add)
                            ksrc = kraw
                        else:
                            ksrc = kps[dh]
                        kmin = ph1sb.tile([P, 512], F32, tag="kmin", name=f"kmin_{t}_{dh}")
                        nc.vector.tensor_scalar(kmin[:], ksrc[:], 0.0, None, OP.min)
                        kexp = ph1sb.tile([P, 512], F32, tag="kexp", name=f"kexp_{t}_{dh}")
                        nc.scalar.activation(kexp[:], kmin[:], AF.Exp)
                        kmax = ph1sb.tile([P, 512], F32, tag="kmax", name=f"kmax_{t}_{dh}")
                        nc.vector.tensor_scalar(kmax[:], ksrc[:], 0.0, -1.0, OP.max, OP.add)
                        kf = ph1sb.tile([P, 512], F32, tag="kf", name=f"kf_{t}_{dh}")
                        nc.vector.tensor_tensor(kf[:], kmax[:], kexp[:], OP.add)
                        kb = ph1sb.tile([P, 512], BF16, tag="kb", name=f"kb_{t}_{dh}")
                        nc.vector.tensor_copy(kb[:], kf[:])
                        if DEBUG_TAPS and t == 0 and dh == 0:
                            nc.sync.dma_start(dbg_k0.ap(), kf[:])
                        kb_chunks.append(kb)
                        # Ksum column blocks (f32 matmul against ones) go into
                        # cols [256, 260) of the shared kvs_t psum tile.
                        kvs_t = kvps_pool.tile([P, 260], F32, tag="kvs_t",
                                               name=f"kvs_t_{t}_{dh}", bufs=1)
                        kvs_tiles[dh] = kvs_t
                        for j in range(4):
                            nc.tensor.matmul(
                                kvs_t[:, 256 + j:257 + j], kf[:, j * P:(j + 1) * P],
                                ones_sb[:], start=True, stop=True, skip_group_check=True)
                    for dh in range(2):
                        dsl = slice(dh * 512, (dh + 1) * 512)
                        vb = ph1sb.tile([P, 512], BF16, tag="vb", name=f"vb_{t}_{dh}")
                        if apply_bias:
                            nc.vector.tensor_tensor(vb[:], vps[dh][:], bv_b[:, dsl], OP.add)
                        else:
                            nc.any.tensor_copy(vb[:], vps[dh][:])
                        kb = kb_chunks[dh]
                        kvs_t = kvs_tiles[dh]
                        for hh in range(8):
                            h = dh * 8 + hh
                            pr = (h % 2) * 64
                            fc = (h // 2) * 64 - dh * 256
                            nc.tensor.matmul(
                                kvs_t[pr:pr + 64, fc:fc + 64],
                                kb[:, hh * 64:(hh + 1) * 64],
                                vb[:, hh * 64:(hh + 1) * 64],
                                start=True, stop=True,
                                tile_position=(0, pr), skip_group_check=True)
                        nc.vector.tensor_tensor(
                            acc[:, dh * 260:(dh + 1) * 260],
                            acc[:, dh * 260:(dh + 1) * 260], kvs_t[:], OP.add)

                if DEBUG_TAPS:
                    nc.vector.tensor_copy(kv_loc[:, :256], acc[:, :256])
                    nc.vector.tensor_copy(kv_loc[:, 256:], acc[:, 260:516])
                    nc.vector.tensor_copy(ksum_loc[:, :4], acc[:, 256:260])
                    nc.vector.tensor_copy(ksum_loc[:, 4:], acc[:, 516:520])
                    nc.sync.dma_start(dbg_kv.ap(), kv_loc[:])
                    nc.sync.dma_start(dbg_ksum.ap(), ksum_loc[:])

            # ========== Phases 2-4: AllReduce; Q^T; attention (pipelined) ==========
            with (
                tc.tile_pool(name="qx", bufs=1) as qx,
                tc.tile_pool(name="late", bufs=1) as late,
            ):
                # bf16 hi/lo x^T for the Q 3-term split; DMA'd into the SBUF
                # space phase 1 just freed, overlapping the AllReduce.
                xthi_sb = qx.tile([P, KC, TOK], BF16)
                xtlo_sb = qx.tile([P, KC, TOK], BF16)
                for k in range(KC):
                    nc.sync.dma_start(xthi_sb[:, k, :], r8(xthi)[:, k, :])
                    nc.sync.dma_start(xtlo_sb[:, k, :], r8(xtlo)[:, k, :])

                at_sb = late.tile([P, KC, TOK], BF16)
                woh_sb = late.tile([P, KC, D], BF16)

                with (
                    tc.tile_pool(name="qtp", bufs=4) as qtp,
                    tc.tile_pool(name="ph3ps", bufs=4, space="PSUM") as ph3ps,
                    tc.tile_pool(name="ph3sb", bufs=3) as ph3sb,
                    tc.tile_pool(name="ph4ps_a", bufs=2, space="PSUM") as ph4ps_a,
                    tc.tile_pool(name="ph4sb", bufs=2) as ph4sb,
                ):
                    qt_tiles = {}

                    # -- AllReduce of the packed KV/Ksum accumulator --
                    cc_in = dram.tile([P, 520], F32)
                    cc_out = dram.tile([P, 520], F32)
                    nc.sync.dma_start(cc_in[:], acc[:])
                    nc.gpsimd.collective_compute(
                        "AllReduce", OP.add,
                        replica_groups=[[0, 1], [2, 3], [4, 5], [6, 7]],
                        ins=[cc_in[:].opt()], outs=[cc_out[:].opt()])
                    ar_sb = smalls.tile([P, 520], F32)
                    nc.sync.dma_start(ar_sb[:], cc_out[:])
                    if DEBUG_TAPS:
                        nc.sync.dma_start(dbg_ar.ap(), ar_sb[:])
                    kv_bf = smalls.tile([P, 512], BF16)
                    nc.gpsimd.tensor_copy(kv_bf[:, :256], ar_sb[:, :256])
                    nc.gpsimd.tensor_copy(kv_bf[:, 256:], ar_sb[:, 260:516])
                    # kdp col hp: Ksum_{2hp}/2 on partitions 0-63 and
                    # Ksum_{2hp+1}/2 on 64-127 -- aligned with qt's partition
                    # layout. den halves come from two full-128 partition
                    # reduces (the gpsimd ucode only supports base partition
                    # 0): A = sum(qt*kdp) = (den_e+den_o)/2 over all 128,
                    # B = sum(qt*kdps) = (den_e-den_o)/2 with kdps = +-kdp,
                    # so den_e = A+B (rows 0-63) and den_o = A-B (rows 64+).
                    kdp = smalls.tile([P, KC], F32)
                    for h in range(H):
                        pr = (h % 2) * 64
                        c = h // 2
                        sc = 256 + c if c < 4 else 516 + (c - 4)
                        nc.gpsimd.tensor_copy(
                            kdp[pr:pr + 64, h // 2:h // 2 + 1],
                            ar_sb[pr:pr + 64, sc:sc + 1])
                    nc.vector.tensor_scalar(kdp[:], kdp[:], 0.5, None, OP.mult)
                    sgn = smalls.tile([P, 1], F32)
                    nc.vector.memset(sgn[0:64, :], 1.0)
                    nc.vector.memset(sgn[64:128, :], -1.0)
                    kdps = smalls.tile([P, KC], F32)
                    nc.vector.tensor_scalar(kdps[:], kdp[:], sgn[:, 0:1], None, OP.mult)
                    # Block-diagonal KV stationary per head pair: one 128-wide
                    # matmul computes both heads' attention numerators.
                    kvq = smalls.tile([P, KC, P], BF16)
                    nc.gpsimd.memset(kvq[:], 0.0)
                    for hp in range(KC):
                        nc.gpsimd.tensor_copy(kvq[0:64, hp, 0:64],
                                              kv_bf[0:64, hp * 64:(hp + 1) * 64])
                        nc.gpsimd.tensor_copy(kvq[64:128, hp, 64:128],
                                              kv_bf[64:128, hp * 64:(hp + 1) * 64])

                    for k in range(KC):
                        nc.sync.dma_start(woh_sb[:, k, :], r8(woh)[:, k, :])

                    def q_proj(hp):
                        wq_t = wq_tiles.pop(hp)
                        qt = qtp.tile([P, TOK], F32, tag="qt", name=f"qt_{hp}")
                        qt_tiles[hp] = qt
                        # k-outer over all four tq psum tiles: each arriving
                        # x^T chunk immediately feeds 12 matmuls, so the
                        # hp=0 wave overlaps the xthi/xtlo DMA chunk-by-chunk.
                        qps_t = [ph3ps.tile([P, 512], F32, tag="qps",
                                            name=f"qps_{hp}_{tq}") for tq in range(TQ)]
                        for k in range(KC):
                            st, sp = (k == 0), (k == KC - 1)
                            for tq in range(TQ):
                                tsl = slice(tq * 512, (tq + 1) * 512)
                                nc.tensor.matmul(qps_t[tq][:], wq_t[:, k, 0, :],
                                                 xthi_sb[:, k, tsl], start=st, stop=False)
                                nc.tensor.matmul(qps_t[tq][:], wq_t[:, k, 1, :],
                                                 xthi_sb[:, k, tsl], start=False, stop=False)
                                nc.tensor.matmul(qps_t[tq][:], wq_t[:, k, 0, :],
                                                 xtlo_sb[:, k, tsl], start=False, stop=sp)
                        for tq in range(TQ):
                            tsl = slice(tq * 512, (tq + 1) * 512)
                            qps = qps_t[tq]
                            if apply_bias:
                                qraw = ph3sb.tile([P, 512], F32, tag="qraw",
                                                  name=f"qraw_{hp}_{tq}")
                                nc.vector.tensor_scalar(qraw[:], qps[:], bq_sb[:, hp:hp + 1],
                                                        None, OP.add)
                                qsrc = qraw
                            else:
                                qsrc = qps
                            qmin = ph3sb.tile([P, 512], F32, tag="qmin", name=f"qmin_{hp}_{tq}")
                            nc.vector.tensor_scalar(qmin[:], qsrc[:], 0.0, None, OP.min)
                            qexp = ph3sb.tile([P, 512], F32, tag="qexp", name=f"qexp_{hp}_{tq}")
                            nc.scalar.activation(qexp[:], qmin[:], AF.Exp)
                            qmax = ph3sb.tile([P, 512], F32, tag="qmax", name=f"qmax_{hp}_{tq}")
                            nc.vector.tensor_scalar(qmax[:], qsrc[:], 0.0, -1.0, OP.max, OP.add)
                            nc.vector.tensor_tensor(qt[:, tsl], qmax[:], qexp[:], OP.add)

                    def attention(hp):
                        qt = qt_tiles.pop(hp)
                        if DEBUG_TAPS and hp == 0:
                            nc.sync.dma_start(dbg_qt0.ap(), qt[:])
                        for tq in range(TQ):
                            tsl = slice(tq * 512, (tq + 1) * 512)
                            # den on gpsimd+DVE (keeps PE free); see kdp note.
                            prod = ph4sb.tile([P, 512], F32, tag="prod",
                                              name=f"prod_{hp}_{tq}")
                            nc.vector.tensor_scalar(prod[:], qt[:, tsl],
                                                    kdp[:, hp:hp + 1], None, OP.mult)
                            sprd = ph4sb.tile([P, 512], F32, tag="sprd",
                                              name=f"sprd_{hp}_{tq}")
                            nc.vector.tensor_scalar(sprd[:], qt[:, tsl],
                                                    kdps[:, hp:hp + 1], None, OP.mult)
                            denA = ph4sb.tile([P, 512], F32, tag="denA",
                                              name=f"denA_{hp}_{tq}")
                            denB = ph4sb.tile([P, 512], F32, tag="denB",
                                              name=f"denB_{hp}_{tq}")
                            nc.gpsimd.partition_all_reduce(
                                denA[:], prod[:], channels=128,
                                reduce_op=bass_isa.ReduceOp.add)
                            nc.gpsimd.partition_all_reduce(
                                denB[:], sprd[:], channels=128,
                                reduce_op=bass_isa.ReduceOp.add)
                            nc.vector.tensor_tensor(denA[0:64, :], denA[0:64, :],
                                                    denB[0:64, :], OP.add)
                            nc.vector.tensor_tensor(denA[64:128, :], denA[64:128, :],
                                                    denB[64:128, :], OP.subtract)
                            nc.vector.reciprocal(denA[:], denA[:])
                            qbf = ph4sb.tile([P, 512], BF16, tag="qbf", name=f"qbf_{hp}_{tq}")
                            nc.vector.tensor_copy(qbf[:], qt[:, tsl])
                            aps = ph4ps_a.tile([P, 512], F32, tag="aps", name=f"aps_{hp}_{tq}")
                            nc.tensor.matmul(aps[:], kvq[:, hp, :], qbf[:],
                                             start=True, stop=True)
                            nc.vector.tensor_tensor(at_sb[:, hp, tsl], aps[:], denA[:], OP.mult)

                    # depth-2 software pipeline: attention(hp) runs two Q chunks
                    # behind, so the AllReduce hides under ~3 Q projections.
                    q_proj(0)
                    for hp in range(1, KC):
                        if hp + 1 < KC:
                            load_wq(hp + 1)
                        q_proj(hp)
                        if hp >= 2:
                            attention(hp - 2)
                    attention(KC - 2)
                    attention(KC - 1)

                if DEBUG_TAPS:
                    with tc.tile_pool(name="dbgat", bufs=2) as dbgat:
                        for c in range(KC):
                            atf = dbgat.tile([P, TOK], F32, tag="atf", name=f"atf_{c}")
                            nc.vector.tensor_copy(atf[:], at_sb[:, c, :])
                            nc.sync.dma_start(dbg_at.ap()[:, c, :], atf[:])

                # ===== Phase 5: output projection + residual + LayerNorm =====
                with (
                    tc.tile_pool(name="ph5ps", bufs=3, space="PSUM") as ph5ps,
                    tc.tile_pool(name="ph5sb", bufs=3) as ph5sb,
                ):
                    for t in range(TC):
                        ts = slice(t * P, (t + 1) * P)
                        y = ph5sb.tile([P, D], F32, tag="y", name=f"y_{t}")
                        xr = ph5sb.tile([P, D], F32, tag="xr", name=f"xr_{t}")
                        nc.sync.dma_start(xr[:], xres.ap()[ts, :])
                        ops = ph5ps.tile([P, D], F32, tag="ops", name=f"ops_{t}")
                        for dh in range(2):
                            dsl = slice(dh * 512, (dh + 1) * 512)
                            for c in range(KC):
                                nc.tensor.matmul(ops[:, dsl], at_sb[:, c, ts], woh_sb[:, c, dsl],
                                                 start=(c == 0), stop=(c == KC - 1))
                        nc.vector.tensor_tensor(y[:], ops[:], xr[:], OP.add)
                        if apply_bias:
                            nc.vector.tensor_tensor(y[:], y[:], bo_b[:], OP.add)
                        stats = ph5sb.tile([P, 2, 6], F32, tag="stats", name=f"stats_{t}")
                        nc.vector.bn_stats(out=stats[:, 0, :], in_=y[:, :512])
                        nc.vector.bn_stats(out=stats[:, 1, :], in_=y[:, 512:])
                        mv = ph5sb.tile([P, 2], F32, tag="mv", name=f"mv_{t}")
                        nc.vector.bn_aggr(out=mv[:], in_=stats[:])
                        nc.scalar.activation(out=mv[:, 1:2], in_=mv[:, 1:2], func=AF.Sqrt,
                                             bias=eps_sb[:], scale=1.0)
                        nc.vector.reciprocal(mv[:, 1:2], mv[:, 1:2])
                        yo = ph5sb.tile([P, D], F32, tag="yo", name=f"yo_{t}")
                        nc.gpsimd.tensor_scalar(yo[:], y[:], mv[:, 0:1], mv[:, 1:2],
                                                OP.subtract, OP.mult)
                        if apply_gamma:
                            nc.vector.tensor_tensor(yo[:], yo[:], gamma_b[:], OP.mult)
                        if apply_beta:
                            nc.vector.tensor_tensor(yo[:], yo[:], beta_b[:], OP.add)
                        nc.sync.dma_start(out_d.ap()[ts, :], yo[:])

            wqp_cm.__exit__(None, None, None)

    nc.compile()
    return nc


def kernel(x, Wq, bq, Wk, bk, Wv, bv, Wo, bo, gamma, beta):
    global LAST_RESULT
    x = np.asarray(x, dtype=np.float32)
    f32 = np.float32
    bf16 = ml_dtypes.bfloat16

    apply_bias = any(np.any(np.asarray(b)) for b in (bq, bk, bv, bo))
    apply_gamma = not np.all(np.asarray(gamma) == 1.0)
    apply_beta = bool(np.any(np.asarray(beta)))

    nc = _build(apply_bias, apply_gamma, apply_beta)

    def split(W):
        W = np.asarray(W, dtype=f32)
        hi = W.astype(bf16)
        lo = (W - hi.astype(f32)).astype(bf16)
        return hi, lo

    wq_h, wq_l = split(Wq)
    wk_f = np.ascontiguousarray(np.asarray(Wk, dtype=f32))
    wv_f = np.ascontiguousarray(np.asarray(Wv, dtype=f32))
    wo_h, _ = split(Wo)
    e_sel = np.zeros((2, P), dtype=bf16)
    e_sel[0, :64] = 1
    e_sel[1, 64:] = 1

    in_maps = []
    for c in range(NCORES):
        b, half = c // 2, c % 2
        xs = x[b, half * TOK:(half + 1) * TOK]          # [2048, 1024]
        xhi = xs.astype(bf16)
        xlo = (xs - xhi.astype(f32)).astype(bf16)
        m = {
            "xtf": np.ascontiguousarray(xs.T),
            "xthi": np.ascontiguousarray(xhi.T),
            "xtlo": np.ascontiguousarray(xlo.T),
            "wqh": wq_h, "wql": wq_l,
            "wk": wk_f, "wv": wv_f, "woh": wo_h,
            "xres": np.ascontiguousarray(xs),
            "e_sel": e_sel,
        }
        if apply_bias:
            m.update(bq=np.asarray(bq, f32), bk=np.asarray(bk, f32),
                     bv=np.asarray(bv, f32), bo=np.asarray(bo, f32))
        if apply_gamma:
            m["gamma"] = np.asarray(gamma, f32)
        if apply_beta:
            m["beta"] = np.asarray(beta, f32)
        in_maps.append(m)

    import os
    try:
        LAST_RESULT = run_bass_kernel_spmd(nc, in_maps, core_ids=list(range(NCORES)))
    except ModuleNotFoundError:
        # no antenv.axon_hooks in this container -> NTFF tracing unavailable
        os.environ["BASS_NEVER_TRACE"] = "1"
        LAST_RESULT = run_bass_kernel_spmd(nc, in_maps, core_ids=list(range(NCORES)))
    out = np.empty((B, N, D), dtype=np.float32)
    for c in range(NCORES):
        b, half = c // 2, c % 2
        out[b, half * TOK:(half + 1) * TOK] = LAST_RESULT.results[c]["out"]
    return out


# revision 42
# speedup vs baseline: 1.0107x; 1.0107x over previous
"""Distributed Trainium2 kernel for the linear-attention transformer block.

Math (per batch element b):
  Q = elu(x @ Wq + bq), K = elu(x @ Wk + bk), V = x @ Wv + bv   (per-head d=64)
  KV_h = K_h^T V_h  [64,64];  Ksum_h = sum_n K_h[n]  [64]
  attn_h = (Q_h @ KV_h) / (Q_h . Ksum_h)
  out = LayerNorm(x + attn @ Wo + bo) * gamma + beta

Sharding: 16384 tokens over 8 cores (2048 each; core c owns batch c//2,
half c%2). Each core computes Q/K/V only for its tokens, partial KV/Ksum,
then a 266KB AllReduce over core pairs {2b, 2b+1} completes the KV stats;
attention + output projection + LayerNorm finish locally.

Precision: the 1/(Q.Ksum) denominators pass near zero (min |den| on the
nominal instance is ~0.35) and amplify projection noise into sign flips,
so the Q projection uses a 3-term bf16 split (x_hi@W_hi + x_hi@W_lo +
x_lo@W_hi; per-element err ~5e-6). The K/V projections run as single
float32r matmuls (full PE rate at N=512; measured HW err ~1.5e-4), which
keeps the Ksum-side denominator error ~0.07 rms - flip-safe with >5 sigma
margin. Ksum and the denominator run in f32. ~1.7e-3 global rel err.
"""

import sys

sys.path.insert(0, "/opt/trn_rl_repo")

import numpy as np
import ml_dtypes

import concourse.bass as bass
import concourse.bass_isa as bass_isa
import concourse.mybir as mybir
import concourse.tile as tile
from concourse import bacc
from concourse.bass_utils import run_bass_kernel_spmd

AF = mybir.ActivationFunctionType
OP = mybir.AluOpType
F32 = mybir.dt.float32
F32R = mybir.dt.float32r
BF16 = mybir.dt.bfloat16

B, N, D = 4, 4096, 1024
H, HD = 16, 64
TOK = 2048            # tokens per core
NCORES = 8
LN_EPS = 1e-3
P = 128
KC = D // P           # 8 contraction chunks
TC = TOK // P         # 16 token chunks of 128
TQ = TOK // 512       # 4 token chunks of 512
WARM_MM = 10          # PE clock-gate warmup matmuls

LAST_RESULT = None    # BassKernelResults of the most recent run (for test.py)
DEBUG_TAPS = False    # set True (e.g. from debug.py) to add intermediate outputs


def _build(apply_bias, apply_gamma, apply_beta):
    nc = bacc.Bacc("TRN2", target_bir_lowering=False, debug=False, num_devices=NCORES)

    def din(name, shape, dtype=BF16):
        return nc.dram_tensor(name, shape, dtype, kind="ExternalInput")

    xtf = din("xtf", [D, TOK], F32R)
    xthi = din("xthi", [D, TOK])
    xtlo = din("xtlo", [D, TOK])
    wqh = din("wqh", [D, D])
    wql = din("wql", [D, D])
    wk = din("wk", [D, D], F32R)
    wv = din("wv", [D, D], F32R)
    woh = din("woh", [D, D])
    xres = din("xres", [TOK, D], F32)
    e_sel = din("e_sel", [2, P])
    if apply_bias:
        bq_d = din("bq", [D], F32)
        bk_d = din("bk", [D], F32)
        bv_d = din("bv", [D], F32)
        bo_d = din("bo", [D], F32)
    if apply_gamma:
        gamma_d = din("gamma", [D], F32)
    if apply_beta:
        beta_d = din("beta", [D], F32)
    out_d = nc.dram_tensor("out", [TOK, D], F32, kind="ExternalOutput")
    if DEBUG_TAPS:
        dbg_k0 = nc.dram_tensor("dbg_k0", [P, 512], F32, kind="ExternalOutput")
        dbg_kv = nc.dram_tensor("dbg_kv", [P, 512], F32, kind="ExternalOutput")
        dbg_ksum = nc.dram_tensor("dbg_ksum", [P, KC], F32, kind="ExternalOutput")
        dbg_ar = nc.dram_tensor("dbg_ar", [P, 520], F32, kind="ExternalOutput")
        dbg_qt0 = nc.dram_tensor("dbg_qt0", [P, TOK], F32, kind="ExternalOutput")
        dbg_at = nc.dram_tensor("dbg_at", [P, KC, TOK], F32, kind="ExternalOutput")

    r8 = lambda t: t.ap().rearrange("(ko p) n -> p ko n", p=P)

    def bcast_row(dram_vec, sb_tile):
        # DMA-broadcast a [D] vector to [P, D] (stride-0 partition dim).
        src = bass.AP(
            tensor=dram_vec.ap().tensor,
            offset=dram_vec.ap().offset,
            ap=[[0, P]] + list(dram_vec.ap().ap),
        )
        nc.sync.dma_start(out=sb_tile, in_=src)

    with tile.TileContext(nc) as tc:
        with (
            tc.tile_pool(name="smalls", bufs=1) as smalls,
            tc.tile_pool(name="dram", bufs=1, space="DRAM") as dram,
        ):
            e_sb = smalls.tile([2, P], BF16)
            nc.sync.dma_start(e_sb[:], e_sel.ap())
            ones_sb = smalls.tile([P, 1], F32)
            nc.vector.memset(ones_sb[:], 1.0)
            eps_sb = smalls.tile([P, 1], F32)
            nc.vector.memset(eps_sb[:], LN_EPS)
            if apply_bias:
                bq_sb = smalls.tile([P, KC], F32)   # per-partition layout for Q^T
                nc.sync.dma_start(bq_sb[:], bq_d.ap().rearrange("(ko p) -> p ko", p=P))
                bk_b = smalls.tile([P, D], F32)
                bv_b = smalls.tile([P, D], F32)
                bo_b = smalls.tile([P, D], F32)
                bcast_row(bk_d, bk_b[:])
                bcast_row(bv_d, bv_b[:])
                bcast_row(bo_d, bo_b[:])
            if apply_gamma:
                gamma_b = smalls.tile([P, D], F32)
                bcast_row(gamma_d, gamma_b[:])
            if apply_beta:
                beta_b = smalls.tile([P, D], F32)
                bcast_row(beta_d, beta_b[:])

            if DEBUG_TAPS:
                kv_loc = smalls.tile([P, 512], F32)
                ksum_loc = smalls.tile([P, KC], F32)

            # PE warmup: matmuls on zeroed tiles release the HAM clock
            # gate (1.2->2.4 GHz) while the first input DMAs are in flight.
            with (
                tc.tile_pool(name="warmsb", bufs=1) as warmsb,
                tc.tile_pool(name="warmps", bufs=2, space="PSUM") as warmps,
            ):
                warm_a = warmsb.tile([P, P], BF16)
                warm_b = warmsb.tile([P, 512], BF16)
                nc.gpsimd.memset(warm_a[:], 0.0)
                nc.gpsimd.memset(warm_b[:], 0.0)
                for w in range(WARM_MM):
                    wp = warmps.tile([P, 512], F32, tag="warm", name=f"warm_{w}")
                    nc.tensor.matmul(wp[:], warm_a[:], warm_b[:], start=True, stop=True)

            # Prefetch the first two Q-weight slices; their DMAs have no deps
            # and fill otherwise-idle DMA time during phase 1.
            wqp_cm = tc.tile_pool(name="wqp", bufs=3)
            wqp = wqp_cm.__enter__()
            wq_tiles = {}

            def load_wq(hp):
                msl = slice(hp * P, (hp + 1) * P)
                wq_t = wqp.tile([P, KC, 2, P], BF16, tag="wq", name=f"wq_{hp}")
                nc.sync.dma_start(wq_t[:, :, 0, :], r8(wqh)[:, :, msl])
                nc.sync.dma_start(wq_t[:, :, 1, :], r8(wql)[:, :, msl])
                wq_tiles[hp] = wq_t

            # ================= Phase 1: K, V, partial KV + Ksum =================
            # K = x @ Wk and V = x @ Wv as single f32r matmuls (full PE rate
            # at N=512). x^T is resident in f32; the four psum accumulators
            # (K/V x dh halves) share each stationary x^T chunk.
            with (
                tc.tile_pool(name="ph1x", bufs=1) as ph1x,
                tc.tile_pool(name="wkv", bufs=1) as wkv,
                tc.tile_pool(name="kvps_pool", bufs=1, space="PSUM") as kvps_pool,
                tc.tile_pool(name="ph1ps", bufs=5, space="PSUM") as ph1ps,
                tc.tile_pool(name="ph1sb", bufs=4) as ph1sb,
            ):
                xtf_sb = ph1x.tile([P, KC, TOK], F32R)
                wk_sb = wkv.tile([P, KC, D], F32R)
                wv_sb = wkv.tile([P, KC, D], F32R)
                # DMA order: first t=0..1 x chunks + Wk (needed by t=0 K
                # matmuls), then Wv, then remaining x token-sliced t-major.
                for k in range(KC):
                    nc.sync.dma_start(wk_sb[:, k, :], r8(wk)[:, k, :])
                    nc.sync.dma_start(xtf_sb[:, k, 0:2 * P], r8(xtf)[:, k, 0:2 * P])
                for k in range(KC):
                    nc.sync.dma_start(wv_sb[:, k, :], r8(wv)[:, k, :])
                for t in range(2, TC):
                    ts = slice(t * P, (t + 1) * P)
                    for k in range(KC):
                        nc.sync.dma_start(xtf_sb[:, k, ts], r8(xtf)[:, k, ts])
                load_wq(0)
                load_wq(1)

                # SBUF accumulator (DVE-add per token chunk; interleaved
                # multi-chunk PSUM accumulation groups proved unreliable).
                # Layout per dh half: [dh*260, dh*260+256) = KV, [+256, +260) = Ksum.
                acc = smalls.tile([P, 520], F32)
                nc.vector.memset(acc[:], 0.0)

                for t in range(TC):
                    ts = slice(t * P, (t + 1) * P)
                    kps = {}
                    vps = {}
                    for dh in range(2):
                        kps[dh] = ph1ps.tile([P, 512], F32, tag="proj",
                                             name=f"kps_{t}_{dh}")
                        vps[dh] = ph1ps.tile([P, 512], F32, tag="proj",
                                             name=f"vps_{t}_{dh}")
                    for k in range(KC):
                        st, sp = (k == 0), (k == KC - 1)
                        for dh in range(2):
                            dsl = slice(dh * 512, (dh + 1) * 512)
                            nc.tensor.matmul(kps[dh][:], xtf_sb[:, k, ts],
                                             wk_sb[:, k, dsl], start=st, stop=sp)
                            nc.tensor.matmul(vps[dh][:], xtf_sb[:, k, ts],
                                             wv_sb[:, k, dsl], start=st, stop=sp)
                    kb_chunks = []
                    kvs_tiles = {}
                    for dh in range(2):
                        dsl = slice(dh * 512, (dh + 1) * 512)
                        if apply_bias:
                            kraw = ph1sb.tile([P, 512], F32, tag="kraw", name=f"kraw_{t}_{dh}")
                            nc.vector.tensor_tensor(kraw[:], kps[dh][:], bk_b[:, dsl], OP.add)
                            ksrc = kraw
                        else:
                            ksrc = kps[dh]
                        kmin = ph1sb.tile([P, 512], F32, tag="kmin", name=f"kmin_{t}_{dh}")
                        nc.vector.tensor_scalar(kmin[:], ksrc[:], 0.0, None, OP.min)
                        kexp = ph1sb.tile([P, 512], F32, tag="kexp", name=f"kexp_{t}_{dh}")
                        nc.scalar.activation(kexp[:], kmin[:], AF.Exp)
                        kmax = ph1sb.tile([P, 512], F32, tag="kmax", name=f"kmax_{t}_{dh}")
                        nc.vector.tensor_scalar(kmax[:], ksrc[:], 0.0, -1.0, OP.max, OP.add)
                        kf = ph1sb.tile([P, 512], F32, tag="kf", name=f"kf_{t}_{dh}")
                        nc.vector.tensor_tensor(kf[:], kmax[:], kexp[:], OP.add)
                        kb = ph1sb.tile([P, 512], BF16, tag="kb", name=f"kb_{t}_{dh}")
                        nc.vector.tensor_copy(kb[:], kf[:])
                        if DEBUG_TAPS and t == 0 and dh == 0:
                            nc.sync.dma_start(dbg_k0.ap(), kf[:])
                        kb_chunks.append(kb)
                        # Ksum column blocks (f32 matmul against ones) go into
                        # cols [256, 260) of the shared kvs_t psum tile.
                        kvs_t = kvps_pool.tile([P, 260], F32, tag="kvs_t",
                                               name=f"kvs_t_{t}_{dh}", bufs=1)
                        kvs_tiles[dh] = kvs_t
                        for j in range(4):
                            nc.tensor.matmul(
                                kvs_t[:, 256 + j:257 + j], kf[:, j * P:(j + 1) * P],
                                ones_sb[:], start=True, stop=True, skip_group_check=True)
                    for dh in range(2):
                        dsl = slice(dh * 512, (dh + 1) * 512)
                        vb = ph1sb.tile([P, 512], BF16, tag="vb", name=f"vb_{t}_{dh}")
                        if apply_bias:
                            nc.vector.tensor_tensor(vb[:], vps[dh][:], bv_b[:, dsl], OP.add)
                        else:
                            nc.any.tensor_copy(vb[:], vps[dh][:])
                        kb = kb_chunks[dh]
                        kvs_t = kvs_tiles[dh]
                        for hh in range(8):
                            h = dh * 8 + hh
                            pr = (h % 2) * 64
                            fc = (h // 2) * 64 - dh * 256
                            nc.tensor.matmul(
                                kvs_t[pr:pr + 64, fc:fc + 64],
                                kb[:, hh * 64:(hh + 1) * 64],
                                vb[:, hh * 64:(hh + 1) * 64],
                                start=True, stop=True,
                                tile_position=(0, pr), skip_group_check=True)
                        nc.vector.tensor_tensor(
                            acc[:, dh * 260:(dh + 1) * 260],
                            acc[:, dh * 260:(dh + 1) * 260], kvs_t[:], OP.add)

                if DEBUG_TAPS:
                    nc.vector.tensor_copy(kv_loc[:, :256], acc[:, :256])
                    nc.vector.tensor_copy(kv_loc[:, 256:], acc[:, 260:516])
                    nc.vector.tensor_copy(ksum_loc[:, :4], acc[:, 256:260])
                    nc.vector.tensor_copy(ksum_loc[:, 4:], acc[:, 516:520])
                    nc.sync.dma_start(dbg_kv.ap(), kv_loc[:])
                    nc.sync.dma_start(dbg_ksum.ap(), ksum_loc[:])

            # ========== Phases 2-4: AllReduce; Q^T; attention (pipelined) ==========
            with (
                tc.tile_pool(name="qx", bufs=1) as qx,
                tc.tile_pool(name="late", bufs=1) as late,
            ):
                # bf16 hi/lo x^T for the Q 3-term split; DMA'd into the SBUF
                # space phase 1 just freed, overlapping the AllReduce.
                xthi_sb = qx.tile([P, KC, TOK], BF16)
                xtlo_sb = qx.tile([P, KC, TOK], BF16)
                for k in range(KC):
                    nc.sync.dma_start(xthi_sb[:, k, :], r8(xthi)[:, k, :])
                    nc.sync.dma_start(xtlo_sb[:, k, :], r8(xtlo)[:, k, :])

                at_sb = late.tile([P, KC, TOK], BF16)
                woh_sb = late.tile([P, KC, D], BF16)

                with (
                    tc.tile_pool(name="qtp", bufs=4) as qtp,
                    tc.tile_pool(name="ph3ps", bufs=4, space="PSUM") as ph3ps,
                    tc.tile_pool(name="ph3sb", bufs=3) as ph3sb,
                    tc.tile_pool(name="ph4ps_a", bufs=2, space="PSUM") as ph4ps_a,
                    tc.tile_pool(name="ph4sb", bufs=2) as ph4sb,
                ):
                    qt_tiles = {}

                    # -- AllReduce of the packed KV/Ksum accumulator --
                    cc_in = dram.tile([P, 520], F32)
                    cc_out = dram.tile([P, 520], F32)
                    nc.sync.dma_start(cc_in[:], acc[:])
                    nc.gpsimd.collective_compute(
                        "AllReduce", OP.add,
                        replica_groups=[[0, 1], [2, 3], [4, 5], [6, 7]],
                        ins=[cc_in[:].opt()], outs=[cc_out[:].opt()])
                    ar_sb = smalls.tile([P, 520], F32)
                    nc.sync.dma_start(ar_sb[:], cc_out[:])
                    if DEBUG_TAPS:
                        nc.sync.dma_start(dbg_ar.ap(), ar_sb[:])
                    kv_bf = smalls.tile([P, 512], BF16)
                    nc.any.tensor_copy(kv_bf[:, :256], ar_sb[:, :256])
                    nc.any.tensor_copy(kv_bf[:, 256:], ar_sb[:, 260:516])
                    # kdp col hp: Ksum_{2hp}/2 on partitions 0-63 and
                    # Ksum_{2hp+1}/2 on 64-127 -- aligned with qt's partition
                    # layout. den halves come from two full-128 partition
                    # reduces (the gpsimd ucode only supports base partition
                    # 0): A = sum(qt*kdp) = (den_e+den_o)/2 over all 128,
                    # B = sum(qt*kdps) = (den_e-den_o)/2 with kdps = +-kdp,
                    # so den_e = A+B (rows 0-63) and den_o = A-B (rows 64+).
                    kdp = smalls.tile([P, KC], F32)
                    for h in range(H):
                        pr = (h % 2) * 64
                        c = h // 2
                        sc = 256 + c if c < 4 else 516 + (c - 4)
                        nc.vector.tensor_scalar(
                            kdp[pr:pr + 64, h // 2:h // 2 + 1],
                            ar_sb[pr:pr + 64, sc:sc + 1], 0.5, None, OP.mult)
                    sgn = smalls.tile([P, 1], F32)
                    nc.vector.memset(sgn[0:64, :], 1.0)
                    nc.vector.memset(sgn[64:128, :], -1.0)
                    kdps = smalls.tile([P, KC], F32)
                    nc.vector.tensor_scalar(kdps[:], kdp[:], sgn[:, 0:1], None, OP.mult)
                    # Block-diagonal KV stationary per head pair: one 128-wide
                    # matmul computes both heads' attention numerators.
                    kvq = smalls.tile([P, KC, P], BF16)
                    nc.any.memset(kvq[:], 0.0)
                    for hp in range(KC):
                        nc.any.tensor_copy(kvq[0:64, hp, 0:64],
                                           kv_bf[0:64, hp * 64:(hp + 1) * 64])
                        nc.any.tensor_copy(kvq[64:128, hp, 64:128],
                                           kv_bf[64:128, hp * 64:(hp + 1) * 64])

                    for k in range(KC):
                        nc.sync.dma_start(woh_sb[:, k, :], r8(woh)[:, k, :])

                    def q_proj(hp):
                        wq_t = wq_tiles.pop(hp)
                        qt = qtp.tile([P, TOK], F32, tag="qt", name=f"qt_{hp}")
                        qt_tiles[hp] = qt
                        # k-outer over all four tq psum tiles: each arriving
                        # x^T chunk immediately feeds 12 matmuls, so the
                        # hp=0 wave overlaps the xthi/xtlo DMA chunk-by-chunk.
                        qps_t = [ph3ps.tile([P, 512], F32, tag="qps",
                                            name=f"qps_{hp}_{tq}") for tq in range(TQ)]
                        for k in range(KC):
                            st, sp = (k == 0), (k == KC - 1)
                            for tq in range(TQ):
                                tsl = slice(tq * 512, (tq + 1) * 512)
                                nc.tensor.matmul(qps_t[tq][:], wq_t[:, k, 0, :],
                                                 xthi_sb[:, k, tsl], start=st, stop=False)
                                nc.tensor.matmul(qps_t[tq][:], wq_t[:, k, 1, :],
                                                 xthi_sb[:, k, tsl], start=False, stop=False)
                                nc.tensor.matmul(qps_t[tq][:], wq_t[:, k, 0, :],
                                                 xtlo_sb[:, k, tsl], start=False, stop=sp)
                        for tq in range(TQ):
                            tsl = slice(tq * 512, (tq + 1) * 512)
                            qps = qps_t[tq]
                            if apply_bias:
                                qraw = ph3sb.tile([P, 512], F32, tag="qraw",
                                                  name=f"qraw_{hp}_{tq}")
                                nc.vector.tensor_scalar(qraw[:], qps[:], bq_sb[:, hp:hp + 1],
                                                        None, OP.add)
                                qsrc = qraw
                            else:
                                qsrc = qps
                            qmin = ph3sb.tile([P, 512], F32, tag="qmin", name=f"qmin_{hp}_{tq}")
                            nc.vector.tensor_scalar(qmin[:], qsrc[:], 0.0, None, OP.min)
                            qexp = ph3sb.tile([P, 512], F32, tag="qexp", name=f"qexp_{hp}_{tq}")
                            nc.scalar.activation(qexp[:], qmin[:], AF.Exp)
                            qmax = ph3sb.tile([P, 512], F32, tag="qmax", name=f"qmax_{hp}_{tq}")
                            nc.vector.tensor_scalar(qmax[:], qsrc[:], 0.0, -1.0, OP.max, OP.add)
                            nc.vector.tensor_tensor(qt[:, tsl], qmax[:], qexp[:], OP.add)

                    def attention(hp):
                        qt = qt_tiles.pop(hp)
                        if DEBUG_TAPS and hp == 0:
                            nc.sync.dma_start(dbg_qt0.ap(), qt[:])
                        for tq in range(TQ):
                            tsl = slice(tq * 512, (tq + 1) * 512)
                            # den on gpsimd+DVE (keeps PE free); see kdp note.
                            prod = ph4sb.tile([P, 512], F32, tag="prod",
                                              name=f"prod_{hp}_{tq}")
                            nc.vector.tensor_scalar(prod[:], qt[:, tsl],
                                                    kdp[:, hp:hp + 1], None, OP.mult)
                            sprd = ph4sb.tile([P, 512], F32, tag="sprd",
                                              name=f"sprd_{hp}_{tq}")
                            nc.vector.tensor_scalar(sprd[:], qt[:, tsl],
                                                    kdps[:, hp:hp + 1], None, OP.mult)
                            denA = ph4sb.tile([P, 512], F32, tag="denA",
                                              name=f"denA_{hp}_{tq}")
                            denB = ph4sb.tile([P, 512], F32, tag="denB",
                                              name=f"denB_{hp}_{tq}")
                            nc.gpsimd.partition_all_reduce(
                                denA[:], prod[:], channels=128,
                                reduce_op=bass_isa.ReduceOp.add)
                            nc.gpsimd.partition_all_reduce(
                                denB[:], sprd[:], channels=128,
                                reduce_op=bass_isa.ReduceOp.add)
                            nc.vector.tensor_tensor(denA[0:64, :], denA[0:64, :],
                                                    denB[0:64, :], OP.add)
                            nc.vector.tensor_tensor(denA[64:128, :], denA[64:128, :],
                                                    denB[64:128, :], OP.subtract)
                            nc.vector.reciprocal(denA[:], denA[:])
                            qbf = ph4sb.tile([P, 512], BF16, tag="qbf", name=f"qbf_{hp}_{tq}")
                            nc.vector.tensor_copy(qbf[:], qt[:, tsl])
                            aps = ph4ps_a.tile([P, 512], F32, tag="aps", name=f"aps_{hp}_{tq}")
                            nc.tensor.matmul(aps[:], kvq[:, hp, :], qbf[:],
                                             start=True, stop=True)
                            nc.vector.tensor_tensor(at_sb[:, hp, tsl], aps[:], denA[:], OP.mult)

                    # depth-2 software pipeline: attention(hp) runs two Q chunks
                    # behind, so the AllReduce hides under ~3 Q projections.
                    q_proj(0)
                    for hp in range(1, KC):
                        if hp + 1 < KC:
                            load_wq(hp + 1)
                        q_proj(hp)
                        if hp >= 2:
                            attention(hp - 2)
                    attention(KC - 2)
                    attention(KC - 1)

                if DEBUG_TAPS:
                    with tc.tile_pool(name="dbgat", bufs=2) as dbgat:
                        for c in range(KC):
                            atf = dbgat.tile([P, TOK], F32, tag="atf", name=f"atf_{c}")
                            nc.vector.tensor_copy(atf[:], at_sb[:, c, :])
                            nc.sync.dma_start(dbg_at.ap()[:, c, :], atf[:])

                # ===== Phase 5: output projection + residual + LayerNorm =====
                with (
                    tc.tile_pool(name="ph5ps", bufs=3, space="PSUM") as ph5ps,
                    tc.tile_pool(name="ph5sb", bufs=3) as ph5sb,
                ):
                    for t in range(TC):
                        ts = slice(t * P, (t + 1) * P)
                        y = ph5sb.tile([P, D], F32, tag="y", name=f"y_{t}")
                        xr = ph5sb.tile([P, D], F32, tag="xr", name=f"xr_{t}")
                        nc.sync.dma_start(xr[:], xres.ap()[ts, :])
                        ops = ph5ps.tile([P, D], F32, tag="ops", name=f"ops_{t}")
                        for dh in range(2):
                            dsl = slice(dh * 512, (dh + 1) * 512)
                            for c in range(KC):
                                nc.tensor.matmul(ops[:, dsl], at_sb[:, c, ts], woh_sb[:, c, dsl],
                                                 start=(c == 0), stop=(c == KC - 1))
                        nc.vector.tensor_tensor(y[:], ops[:], xr[:], OP.add)
                        if apply_bias:
                            nc.vector.tensor_tensor(y[:], y[:], bo_b[:], OP.add)
                        stats = ph5sb.tile([P, 2, 6], F32, tag="stats", name=f"stats_{t}")
                        nc.vector.bn_stats(out=stats[:, 0, :], in_=y[:, :512])
                        nc.vector.bn_stats(out=stats[:, 1, :], in_=y[:, 512:])
                        mv = ph5sb.tile([P, 2], F32, tag="mv", name=f"mv_{t}")
                        nc.vector.bn_aggr(out=mv[:], in_=stats[:])
                        nc.scalar.activation(out=mv[:, 1:2], in_=mv[:, 1:2], func=AF.Sqrt,
                                             bias=eps_sb[:], scale=1.0)
                        nc.vector.reciprocal(mv[:, 1:2], mv[:, 1:2])
                        yo = ph5sb.tile([P, D], F32, tag="yo", name=f"yo_{t}")
                        nc.gpsimd.tensor_scalar(yo[:], y[:], mv[:, 0:1], mv[:, 1:2],
                                                OP.subtract, OP.mult)
                        if apply_gamma:
                            nc.vector.tensor_tensor(yo[:], yo[:], gamma_b[:], OP.mult)
                        if apply_beta:
                            nc.vector.tensor_tensor(yo[:], yo[:], beta_b[:], OP.add)
                        nc.sync.dma_start(out_d.ap()[ts, :], yo[:])

            wqp_cm.__exit__(None, None, None)

    nc.compile()
    return nc


def kernel(x, Wq, bq, Wk, bk, Wv, bv, Wo, bo, gamma, beta):
    global LAST_RESULT
    x = np.asarray(x, dtype=np.float32)
    f32 = np.float32
    bf16 = ml_dtypes.bfloat16

    apply_bias = any(np.any(np.asarray(b)) for b in (bq, bk, bv, bo))
    apply_gamma = not np.all(np.asarray(gamma) == 1.0)
    apply_beta = bool(np.any(np.asarray(beta)))

    nc = _build(apply_bias, apply_gamma, apply_beta)

    def split(W):
        W = np.asarray(W, dtype=f32)
        hi = W.astype(bf16)
        lo = (W - hi.astype(f32)).astype(bf16)
        return hi, lo

    wq_h, wq_l = split(Wq)
    wk_f = np.ascontiguousarray(np.asarray(Wk, dtype=f32))
    wv_f = np.ascontiguousarray(np.asarray(Wv, dtype=f32))
    wo_h, _ = split(Wo)
    e_sel = np.zeros((2, P), dtype=bf16)
    e_sel[0, :64] = 1
    e_sel[1, 64:] = 1

    in_maps = []
    for c in range(NCORES):
        b, half = c // 2, c % 2
        xs = x[b, half * TOK:(half + 1) * TOK]          # [2048, 1024]
        xhi = xs.astype(bf16)
        xlo = (xs - xhi.astype(f32)).astype(bf16)
        m = {
            "xtf": np.ascontiguousarray(xs.T),
            "xthi": np.ascontiguousarray(xhi.T),
            "xtlo": np.ascontiguousarray(xlo.T),
            "wqh": wq_h, "wql": wq_l,
            "wk": wk_f, "wv": wv_f, "woh": wo_h,
            "xres": np.ascontiguousarray(xs),
            "e_sel": e_sel,
        }
        if apply_bias:
            m.update(bq=np.asarray(bq, f32), bk=np.asarray(bk, f32),
                     bv=np.asarray(bv, f32), bo=np.asarray(bo, f32))
        if apply_gamma:
            m["gamma"] = np.asarray(gamma, f32)
        if apply_beta:
            m["beta"] = np.asarray(beta, f32)
        in_maps.append(m)

    import os
    try:
        LAST_RESULT = run_bass_kernel_spmd(nc, in_maps, core_ids=list(range(NCORES)))
    except ModuleNotFoundError:
        # no antenv.axon_hooks in this container -> NTFF tracing unavailable
        os.environ["BASS_NEVER_TRACE"] = "1"
        LAST_RESULT = run_bass_kernel_spmd(nc, in_maps, core_ids=list(range(NCORES)))
    out = np.empty((B, N, D), dtype=np.float32)
    for c in range(NCORES):
        b, half = c // 2, c % 2
        out[b, half * TOK:(half + 1) * TOK] = LAST_RESULT.results[c]["out"]
    return out


# revision 53
# speedup vs baseline: 1.0183x; 1.0075x over previous
"""Distributed Trainium2 kernel for the linear-attention transformer block.

Math (per batch element b):
  Q = elu(x @ Wq + bq), K = elu(x @ Wk + bk), V = x @ Wv + bv   (per-head d=64)
  KV_h = K_h^T V_h  [64,64];  Ksum_h = sum_n K_h[n]  [64]
  attn_h = (Q_h @ KV_h) / (Q_h . Ksum_h)
  out = LayerNorm(x + attn @ Wo + bo) * gamma + beta

Sharding: 16384 tokens over 8 cores (2048 each; core c owns batch c//2,
half c%2). Each core computes Q/K/V only for its tokens, partial KV/Ksum,
then a 266KB AllReduce over core pairs {2b, 2b+1} completes the KV stats;
attention + output projection + LayerNorm finish locally.

Precision: the 1/(Q.Ksum) denominators pass near zero (min |den| on the
nominal instance is ~0.35) and amplify projection noise into sign flips,
so the Q projection uses a 3-term bf16 split (x_hi@W_hi + x_hi@W_lo +
x_lo@W_hi; per-element err ~5e-6). The K/V projections run as single
float32r matmuls (full PE rate at N=512; measured HW err ~1.5e-4), which
keeps the Ksum-side denominator error ~0.07 rms - flip-safe with >5 sigma
margin. Ksum and the denominator run in f32. ~1.7e-3 global rel err.
"""

import sys

sys.path.insert(0, "/opt/trn_rl_repo")

import numpy as np
import ml_dtypes

import concourse.bass as bass
import concourse.bass_isa as bass_isa
import concourse.mybir as mybir
import concourse.tile as tile
from concourse import bacc
from concourse.bass_utils import run_bass_kernel_spmd

AF = mybir.ActivationFunctionType
OP = mybir.AluOpType
F32 = mybir.dt.float32
F32R = mybir.dt.float32r
BF16 = mybir.dt.bfloat16

B, N, D = 4, 4096, 1024
H, HD = 16, 64
TOK = 2048            # tokens per core
NCORES = 8
LN_EPS = 1e-3
P = 128
KC = D // P           # 8 contraction chunks
TC = TOK // P         # 16 token chunks of 128
TQ = TOK // 512       # 4 token chunks of 512
WARM_MM = 10          # PE clock-gate warmup matmuls

LAST_RESULT = None    # BassKernelResults of the most recent run (for test.py)
DEBUG_TAPS = False    # set True (e.g. from debug.py) to add intermediate outputs


def _build(apply_bias, apply_gamma, apply_beta):
    nc = bacc.Bacc("TRN2", target_bir_lowering=False, debug=False, num_devices=NCORES)

    def din(name, shape, dtype=BF16):
        return nc.dram_tensor(name, shape, dtype, kind="ExternalInput")

    xtf = din("xtf", [D, TOK], F32R)
    xthi = din("xthi", [D, TOK])
    xtlo = din("xtlo", [D, TOK])
    wqh = din("wqh", [D, D])
    wql = din("wql", [D, D])
    wk = din("wk", [D, D], F32R)
    wv = din("wv", [D, D], F32R)
    woh = din("woh", [D, D])
    xres = din("xres", [TOK, D], F32)
    e_sel = din("e_sel", [2, P])
    if apply_bias:
        bq_d = din("bq", [D], F32)
        bk_d = din("bk", [D], F32)
        bv_d = din("bv", [D], F32)
        bo_d = din("bo", [D], F32)
    if apply_gamma:
        gamma_d = din("gamma", [D], F32)
    if apply_beta:
        beta_d = din("beta", [D], F32)
    out_d = nc.dram_tensor("out", [TOK, D], F32, kind="ExternalOutput")
    if DEBUG_TAPS:
        dbg_k0 = nc.dram_tensor("dbg_k0", [P, 512], F32, kind="ExternalOutput")
        dbg_kv = nc.dram_tensor("dbg_kv", [P, 512], F32, kind="ExternalOutput")
        dbg_ksum = nc.dram_tensor("dbg_ksum", [P, KC], F32, kind="ExternalOutput")
        dbg_ar = nc.dram_tensor("dbg_ar", [P, 520], F32, kind="ExternalOutput")
        dbg_qt0 = nc.dram_tensor("dbg_qt0", [P, TOK], F32, kind="ExternalOutput")
        dbg_at = nc.dram_tensor("dbg_at", [P, KC, TOK], F32, kind="ExternalOutput")

    r8 = lambda t: t.ap().rearrange("(ko p) n -> p ko n", p=P)

    def bcast_row(dram_vec, sb_tile):
        # DMA-broadcast a [D] vector to [P, D] (stride-0 partition dim).
        src = bass.AP(
            tensor=dram_vec.ap().tensor,
            offset=dram_vec.ap().offset,
            ap=[[0, P]] + list(dram_vec.ap().ap),
        )
        nc.sync.dma_start(out=sb_tile, in_=src)

    with tile.TileContext(nc) as tc:
        with (
            tc.tile_pool(name="smalls", bufs=1) as smalls,
            tc.tile_pool(name="dram", bufs=1, space="DRAM") as dram,
        ):
            e_sb = smalls.tile([2, P], BF16)
            nc.sync.dma_start(e_sb[:], e_sel.ap())
            ones_sb = smalls.tile([P, 1], F32)
            nc.vector.memset(ones_sb[:], 1.0)
            eps_sb = smalls.tile([P, 1], F32)
            nc.vector.memset(eps_sb[:], LN_EPS)
            if apply_bias:
                bq_sb = smalls.tile([P, KC], F32)   # per-partition layout for Q^T
                nc.sync.dma_start(bq_sb[:], bq_d.ap().rearrange("(ko p) -> p ko", p=P))
                bk_b = smalls.tile([P, D], F32)
                bv_b = smalls.tile([P, D], F32)
                bo_b = smalls.tile([P, D], F32)
                bcast_row(bk_d, bk_b[:])
                bcast_row(bv_d, bv_b[:])
                bcast_row(bo_d, bo_b[:])
            if apply_gamma:
                gamma_b = smalls.tile([P, D], F32)
                bcast_row(gamma_d, gamma_b[:])
            if apply_beta:
                beta_b = smalls.tile([P, D], F32)
                bcast_row(beta_d, beta_b[:])

            if DEBUG_TAPS:
                kv_loc = smalls.tile([P, 512], F32)
                ksum_loc = smalls.tile([P, KC], F32)

            # PE warmup: matmuls on zeroed tiles release the HAM clock
            # gate (1.2->2.4 GHz) while the first input DMAs are in flight.
            with (
                tc.tile_pool(name="warmsb", bufs=1) as warmsb,
                tc.tile_pool(name="warmps", bufs=2, space="PSUM") as warmps,
            ):
                warm_a = warmsb.tile([P, P], BF16)
                warm_b = warmsb.tile([P, 512], BF16)
                nc.gpsimd.memset(warm_a[:], 0.0)
                nc.gpsimd.memset(warm_b[:], 0.0)
                for w in range(WARM_MM):
                    wp = warmps.tile([P, 512], F32, tag="warm", name=f"warm_{w}")
                    nc.tensor.matmul(wp[:], warm_a[:], warm_b[:], start=True, stop=True)

            # Prefetch the first two Q-weight slices; their DMAs have no deps
            # and fill otherwise-idle DMA time during phase 1.
            wqp_cm = tc.tile_pool(name="wqp", bufs=3)
            wqp = wqp_cm.__enter__()
            wq_tiles = {}

            def load_wq(hp):
                msl = slice(hp * P, (hp + 1) * P)
                wq_t = wqp.tile([P, KC, 2, P], BF16, tag="wq", name=f"wq_{hp}")
                nc.sync.dma_start(wq_t[:, :, 0, :], r8(wqh)[:, :, msl])
                nc.sync.dma_start(wq_t[:, :, 1, :], r8(wql)[:, :, msl])
                wq_tiles[hp] = wq_t

            # ================= Phase 1: K, V, partial KV + Ksum =================
            # K = x @ Wk and V = x @ Wv as single f32r matmuls (full PE rate
            # at N=512). x^T is resident in f32; the four psum accumulators
            # (K/V x dh halves) share each stationary x^T chunk.
            with (
                tc.tile_pool(name="ph1x", bufs=1) as ph1x,
                tc.tile_pool(name="wkv", bufs=1) as wkv,
                tc.tile_pool(name="kvps_pool", bufs=2, space="PSUM") as kvps_pool,
                tc.tile_pool(name="ph1ps", bufs=6, space="PSUM") as ph1ps,
                tc.tile_pool(name="ph1sb", bufs=4) as ph1sb,
            ):
                xtf_sb = ph1x.tile([P, KC, TOK], F32R)
                wk_sb = wkv.tile([P, KC, D], F32R)
                wv_sb = wkv.tile([P, KC, D], F32R)
                # DMA order: first t=0..1 x chunks + Wk (needed by t=0 K
                # matmuls), then Wv, then remaining x token-sliced t-major.
                for k in range(KC):
                    nc.sync.dma_start(wk_sb[:, k, :], r8(wk)[:, k, :])
                    nc.sync.dma_start(xtf_sb[:, k, 0:2 * P], r8(xtf)[:, k, 0:2 * P])
                for k in range(KC):
                    nc.sync.dma_start(wv_sb[:, k, :], r8(wv)[:, k, :])
                for t in range(2, TC):
                    ts = slice(t * P, (t + 1) * P)
                    for k in range(KC):
                        nc.sync.dma_start(xtf_sb[:, k, ts], r8(xtf)[:, k, ts])
                load_wq(0)
                load_wq(1)

                # SBUF accumulator (DVE-add per token chunk; interleaved
                # multi-chunk PSUM accumulation groups proved unreliable).
                # Layout per dh half: [dh*260, dh*260+256) = KV, [+256, +260) = Ksum.
                acc = smalls.tile([P, 520], F32)
                nc.vector.memset(acc[:], 0.0)

                for t in range(TC):
                    ts = slice(t * P, (t + 1) * P)
                    kps = {}
                    vps = {}
                    for dh in range(2):
                        kps[dh] = ph1ps.tile([P, 512], F32, tag="proj",
                                             name=f"kps_{t}_{dh}")
                        vps[dh] = ph1ps.tile([P, 512], F32, tag="proj",
                                             name=f"vps_{t}_{dh}")
                    for k in range(KC):
                        st, sp = (k == 0), (k == KC - 1)
                        for dh in range(2):
                            dsl = slice(dh * 512, (dh + 1) * 512)
                            nc.tensor.matmul(kps[dh][:], xtf_sb[:, k, ts],
                                             wk_sb[:, k, dsl], start=st, stop=sp)
                            nc.tensor.matmul(vps[dh][:], xtf_sb[:, k, ts],
                                             wv_sb[:, k, dsl], start=st, stop=sp)
                    kb_chunks = []
                    kvs_tiles = {}
                    for dh in range(2):
                        dsl = slice(dh * 512, (dh + 1) * 512)
                        if apply_bias:
                            kraw = ph1sb.tile([P, 512], F32, tag="kraw", name=f"kraw_{t}_{dh}")
                            nc.vector.tensor_tensor(kraw[:], kps[dh][:], bk_b[:, dsl], OP.add)
                            ksrc = kraw
                        else:
                            ksrc = kps[dh]
                        kmin = ph1sb.tile([P, 512], F32, tag="kmin", name=f"kmin_{t}_{dh}")
                        nc.vector.tensor_scalar(kmin[:], ksrc[:], 0.0, None, OP.min)
                        kexp = ph1sb.tile([P, 512], F32, tag="kexp", name=f"kexp_{t}_{dh}")
                        nc.scalar.activation(kexp[:], kmin[:], AF.Exp)
                        kmax = ph1sb.tile([P, 512], F32, tag="kmax", name=f"kmax_{t}_{dh}")
                        nc.vector.tensor_scalar(kmax[:], ksrc[:], 0.0, -1.0, OP.max, OP.add)
                        kf = ph1sb.tile([P, 512], F32, tag="kf", name=f"kf_{t}_{dh}")
                        nc.vector.tensor_tensor(kf[:], kmax[:], kexp[:], OP.add)
                        kb = ph1sb.tile([P, 512], BF16, tag="kb", name=f"kb_{t}_{dh}")
                        nc.vector.tensor_copy(kb[:], kf[:])
                        if DEBUG_TAPS and t == 0 and dh == 0:
                            nc.sync.dma_start(dbg_k0.ap(), kf[:])
                        kb_chunks.append(kb)
                        # Ksum column blocks (f32 matmul against ones) go into
                        # cols [256, 260) of the shared kvs_t psum tile.
                        kvs_t = kvps_pool.tile([P, 260], F32, tag="kvs_t",
                                               name=f"kvs_t_{t}_{dh}", bufs=1)
                        kvs_tiles[dh] = kvs_t
                        for j in range(4):
                            nc.tensor.matmul(
                                kvs_t[:, 256 + j:257 + j], kf[:, j * P:(j + 1) * P],
                                ones_sb[:], start=True, stop=True, skip_group_check=True)
                    for dh in range(2):
                        dsl = slice(dh * 512, (dh + 1) * 512)
                        vb = ph1sb.tile([P, 512], BF16, tag="vb", name=f"vb_{t}_{dh}")
                        if apply_bias:
                            nc.vector.tensor_tensor(vb[:], vps[dh][:], bv_b[:, dsl], OP.add)
                        else:
                            nc.any.tensor_copy(vb[:], vps[dh][:])
                        kb = kb_chunks[dh]
                        kvs_t = kvs_tiles[dh]
                        for hh in range(8):
                            h = dh * 8 + hh
                            pr = (h % 2) * 64
                            fc = (h // 2) * 64 - dh * 256
                            nc.tensor.matmul(
                                kvs_t[pr:pr + 64, fc:fc + 64],
                                kb[:, hh * 64:(hh + 1) * 64],
                                vb[:, hh * 64:(hh + 1) * 64],
                                start=True, stop=True,
                                tile_position=(0, pr), skip_group_check=True)
                        nc.vector.tensor_tensor(
                            acc[:, dh * 260:(dh + 1) * 260],
                            acc[:, dh * 260:(dh + 1) * 260], kvs_t[:], OP.add)

                if DEBUG_TAPS:
                    nc.vector.tensor_copy(kv_loc[:, :256], acc[:, :256])
                    nc.vector.tensor_copy(kv_loc[:, 256:], acc[:, 260:516])
                    nc.vector.tensor_copy(ksum_loc[:, :4], acc[:, 256:260])
                    nc.vector.tensor_copy(ksum_loc[:, 4:], acc[:, 516:520])
                    nc.sync.dma_start(dbg_kv.ap(), kv_loc[:])
                    nc.sync.dma_start(dbg_ksum.ap(), ksum_loc[:])

            # ========== Phases 2-4: AllReduce; Q^T; attention (pipelined) ==========
            with (
                tc.tile_pool(name="qx", bufs=1) as qx,
                tc.tile_pool(name="late", bufs=1) as late,
            ):
                # bf16 hi/lo x^T for the Q 3-term split; DMA'd into the SBUF
                # space phase 1 just freed, overlapping the AllReduce.
                xthi_sb = qx.tile([P, KC, TOK], BF16)
                xtlo_sb = qx.tile([P, KC, TOK], BF16)
                for k in range(KC):
                    nc.sync.dma_start(xthi_sb[:, k, :], r8(xthi)[:, k, :])
                    nc.sync.dma_start(xtlo_sb[:, k, :], r8(xtlo)[:, k, :])

                at_sb = late.tile([P, KC, TOK], BF16)
                woh_sb = late.tile([P, KC, D], BF16)

                with (
                    tc.tile_pool(name="qtp", bufs=4) as qtp,
                    tc.tile_pool(name="ph3ps", bufs=4, space="PSUM") as ph3ps,
                    tc.tile_pool(name="ph3sb", bufs=3) as ph3sb,
                    tc.tile_pool(name="ph4ps_a", bufs=2, space="PSUM") as ph4ps_a,
                    tc.tile_pool(name="ph4sb", bufs=2) as ph4sb,
                ):
                    qt_tiles = {}

                    # -- AllReduce of the packed KV/Ksum accumulator --
                    cc_in = dram.tile([P, 520], F32)
                    cc_out = dram.tile([P, 520], F32)
                    nc.sync.dma_start(cc_in[:], acc[:])
                    nc.gpsimd.collective_compute(
                        "AllReduce", OP.add,
                        replica_groups=[[0, 1], [2, 3], [4, 5], [6, 7]],
                        ins=[cc_in[:].opt()], outs=[cc_out[:].opt()])
                    ar_sb = smalls.tile([P, 520], F32)
                    nc.sync.dma_start(ar_sb[:], cc_out[:])
                    if DEBUG_TAPS:
                        nc.sync.dma_start(dbg_ar.ap(), ar_sb[:])
                    kv_bf = smalls.tile([P, 512], BF16)
                    nc.any.tensor_copy(kv_bf[:, :256], ar_sb[:, :256])
                    nc.any.tensor_copy(kv_bf[:, 256:], ar_sb[:, 260:516])
                    # kdp col hp: Ksum_{2hp}/2 on partitions 0-63 and
                    # Ksum_{2hp+1}/2 on 64-127 -- aligned with qt's partition
                    # layout. den halves come from two full-128 partition
                    # reduces (the gpsimd ucode only supports base partition
                    # 0): A = sum(qt*kdp) = (den_e+den_o)/2 over all 128,
                    # B = sum(qt*kdps) = (den_e-den_o)/2 with kdps = +-kdp,
                    # so den_e = A+B (rows 0-63) and den_o = A-B (rows 64+).
                    kdp = smalls.tile([P, KC], F32)
                    for h in range(H):
                        pr = (h % 2) * 64
                        c = h // 2
                        sc = 256 + c if c < 4 else 516 + (c - 4)
                        nc.vector.tensor_scalar(
                            kdp[pr:pr + 64, h // 2:h // 2 + 1],
                            ar_sb[pr:pr + 64, sc:sc + 1], 0.5, None, OP.mult)
                    sgn = smalls.tile([P, 1], F32)
                    nc.vector.memset(sgn[0:64, :], 1.0)
                    nc.vector.memset(sgn[64:128, :], -1.0)
                    kdps = smalls.tile([P, KC], F32)
                    nc.vector.tensor_scalar(kdps[:], kdp[:], sgn[:, 0:1], None, OP.mult)
                    # Block-diagonal KV stationary per head pair: one 128-wide
                    # matmul computes both heads' attention numerators.
                    kvq = smalls.tile([P, KC, P], BF16)
                    nc.any.memset(kvq[:], 0.0)
                    for hp in range(KC):
                        nc.any.tensor_copy(kvq[0:64, hp, 0:64],
                                           kv_bf[0:64, hp * 64:(hp + 1) * 64])
                        nc.any.tensor_copy(kvq[64:128, hp, 64:128],
                                           kv_bf[64:128, hp * 64:(hp + 1) * 64])

                    for k in range(KC):
                        nc.sync.dma_start(woh_sb[:, k, :], r8(woh)[:, k, :])

                    def q_proj(hp):
                        wq_t = wq_tiles.pop(hp)
                        qt = qtp.tile([P, TOK], F32, tag="qt", name=f"qt_{hp}")
                        qt_tiles[hp] = qt
                        # k-outer over all four tq psum tiles: each arriving
                        # x^T chunk immediately feeds 12 matmuls, so the
                        # hp=0 wave overlaps the xthi/xtlo DMA chunk-by-chunk.
                        qps_t = [ph3ps.tile([P, 512], F32, tag="qps",
                                            name=f"qps_{hp}_{tq}") for tq in range(TQ)]
                        for k in range(KC):
                            st, sp = (k == 0), (k == KC - 1)
                            for tq in range(TQ):
                                tsl = slice(tq * 512, (tq + 1) * 512)
                                nc.tensor.matmul(qps_t[tq][:], wq_t[:, k, 0, :],
                                                 xthi_sb[:, k, tsl], start=st, stop=False)
                                nc.tensor.matmul(qps_t[tq][:], wq_t[:, k, 1, :],
                                                 xthi_sb[:, k, tsl], start=False, stop=False)
                                nc.tensor.matmul(qps_t[tq][:], wq_t[:, k, 0, :],
                                                 xtlo_sb[:, k, tsl], start=False, stop=sp)
                        for tq in range(TQ):
                            tsl = slice(tq * 512, (tq + 1) * 512)
                            qps = qps_t[tq]
                            if apply_bias:
                                qraw = ph3sb.tile([P, 512], F32, tag="qraw",
                                                  name=f"qraw_{hp}_{tq}")
                                nc.vector.tensor_scalar(qraw[:], qps[:], bq_sb[:, hp:hp + 1],
                                                        None, OP.add)
                                qsrc = qraw
                            else:
                                qsrc = qps
                            qmin = ph3sb.tile([P, 512], F32, tag="qmin", name=f"qmin_{hp}_{tq}")
                            nc.vector.tensor_scalar(qmin[:], qsrc[:], 0.0, None, OP.min)
                            qexp = ph3sb.tile([P, 512], F32, tag="qexp", name=f"qexp_{hp}_{tq}")
                            nc.scalar.activation(qexp[:], qmin[:], AF.Exp)
                            qmax = ph3sb.tile([P, 512], F32, tag="qmax", name=f"qmax_{hp}_{tq}")
                            nc.vector.tensor_scalar(qmax[:], qsrc[:], 0.0, -1.0, OP.max, OP.add)
                            nc.vector.tensor_tensor(qt[:, tsl], qmax[:], qexp[:], OP.add)

                    def attention(hp):
                        qt = qt_tiles.pop(hp)
                        if DEBUG_TAPS and hp == 0:
                            nc.sync.dma_start(dbg_qt0.ap(), qt[:])
                        for tq in range(TQ):
                            tsl = slice(tq * 512, (tq + 1) * 512)
                            # den on gpsimd+DVE (keeps PE free); see kdp note.
                            prod = ph4sb.tile([P, 512], F32, tag="prod",
                                              name=f"prod_{hp}_{tq}")
                            nc.vector.tensor_scalar(prod[:], qt[:, tsl],
                                                    kdp[:, hp:hp + 1], None, OP.mult)
                            sprd = ph4sb.tile([P, 512], F32, tag="sprd",
                                              name=f"sprd_{hp}_{tq}")
                            nc.vector.tensor_scalar(sprd[:], qt[:, tsl],
                                                    kdps[:, hp:hp + 1], None, OP.mult)
                            denA = ph4sb.tile([P, 512], F32, tag="denA",
                                              name=f"denA_{hp}_{tq}")
                            denB = ph4sb.tile([P, 512], F32, tag="denB",
                                              name=f"denB_{hp}_{tq}")
                            nc.gpsimd.partition_all_reduce(
                                denA[:], prod[:], channels=128,
                                reduce_op=bass_isa.ReduceOp.add)
                            nc.gpsimd.partition_all_reduce(
                                denB[:], sprd[:], channels=128,
                                reduce_op=bass_isa.ReduceOp.add)
                            nc.vector.tensor_tensor(denA[0:64, :], denA[0:64, :],
                                                    denB[0:64, :], OP.add)
                            nc.vector.tensor_tensor(denA[64:128, :], denA[64:128, :],
                                                    denB[64:128, :], OP.subtract)
                            nc.vector.reciprocal(denA[:], denA[:])
                            qbf = ph4sb.tile([P, 512], BF16, tag="qbf", name=f"qbf_{hp}_{tq}")
                            nc.vector.tensor_copy(qbf[:], qt[:, tsl])
                            aps = ph4ps_a.tile([P, 512], F32, tag="aps", name=f"aps_{hp}_{tq}")
                            nc.tensor.matmul(aps[:], kvq[:, hp, :], qbf[:],
                                             start=True, stop=True)
                            nc.vector.tensor_tensor(at_sb[:, hp, tsl], aps[:], denA[:], OP.mult)

                    # depth-2 software pipeline: attention(hp) runs two Q chunks
                    # behind, so the AllReduce hides under ~3 Q projections.
                    # Software pipeline: depth 2 while the AllReduce is in
                    # flight, catching up to depth 1 at hp=4 so only
                    # attention(7)'s den chain trails into phase 5.
                    att_sched = {2: [0], 3: [1], 4: [2, 3], 5: [4], 6: [5], 7: [6]}
                    q_proj(0)
                    for hp in range(1, KC):
                        if hp + 1 < KC:
                            load_wq(hp + 1)
                        q_proj(hp)
                        for a in att_sched.get(hp, []):
                            attention(a)
                    attention(KC - 1)

                if DEBUG_TAPS:
                    with tc.tile_pool(name="dbgat", bufs=2) as dbgat:
                        for c in range(KC):
                            atf = dbgat.tile([P, TOK], F32, tag="atf", name=f"atf_{c}")
                            nc.vector.tensor_copy(atf[:], at_sb[:, c, :])
                            nc.sync.dma_start(dbg_at.ap()[:, c, :], atf[:])

                # ===== Phase 5: output projection + residual + LayerNorm =====
                with (
                    tc.tile_pool(name="ph5ps", bufs=4, space="PSUM") as ph5ps,
                    tc.tile_pool(name="ph5sb", bufs=4) as ph5sb,
                ):
                    for t in range(TC):
                        ts = slice(t * P, (t + 1) * P)
                        y = ph5sb.tile([P, D], F32, tag="y", name=f"y_{t}")
                        xr = ph5sb.tile([P, D], F32, tag="xr", name=f"xr_{t}")
                        nc.sync.dma_start(xr[:], xres.ap()[ts, :])
                        ops = ph5ps.tile([P, D], F32, tag="ops", name=f"ops_{t}")
                        for dh in range(2):
                            dsl = slice(dh * 512, (dh + 1) * 512)
                            for c in range(KC):
                                nc.tensor.matmul(ops[:, dsl], at_sb[:, c, ts], woh_sb[:, c, dsl],
                                                 start=(c == 0), stop=(c == KC - 1))
                        nc.vector.tensor_tensor(y[:], ops[:], xr[:], OP.add)
                        if apply_bias:
                            nc.vector.tensor_tensor(y[:], y[:], bo_b[:], OP.add)
                        stats = ph5sb.tile([P, 2, 6], F32, tag="stats", name=f"stats_{t}")
                        nc.vector.bn_stats(out=stats[:, 0, :], in_=y[:, :512])
                        nc.vector.bn_stats(out=stats[:, 1, :], in_=y[:, 512:])
                        mv = ph5sb.tile([P, 2], F32, tag="mv", name=f"mv_{t}")
                        nc.vector.bn_aggr(out=mv[:], in_=stats[:])
                        nc.scalar.activation(out=mv[:, 1:2], in_=mv[:, 1:2], func=AF.Sqrt,
                                             bias=eps_sb[:], scale=1.0)
                        nc.vector.reciprocal(mv[:, 1:2], mv[:, 1:2])
                        yo = ph5sb.tile([P, D], F32, tag="yo", name=f"yo_{t}")
                        nc.gpsimd.tensor_scalar(yo[:], y[:], mv[:, 0:1], mv[:, 1:2],
                                                OP.subtract, OP.mult)
                        if apply_gamma:
                            nc.vector.tensor_tensor(yo[:], yo[:], gamma_b[:], OP.mult)
                        if apply_beta:
                            nc.vector.tensor_tensor(yo[:], yo[:], beta_b[:], OP.add)
                        nc.sync.dma_start(out_d.ap()[ts, :], yo[:])

            wqp_cm.__exit__(None, None, None)

    nc.compile()
    return nc


def kernel(x, Wq, bq, Wk, bk, Wv, bv, Wo, bo, gamma, beta):
    global LAST_RESULT
    x = np.asarray(x, dtype=np.float32)
    f32 = np.float32
    bf16 = ml_dtypes.bfloat16

    apply_bias = any(np.any(np.asarray(b)) for b in (bq, bk, bv, bo))
    apply_gamma = not np.all(np.asarray(gamma) == 1.0)
    apply_beta = bool(np.any(np.asarray(beta)))

    nc = _build(apply_bias, apply_gamma, apply_beta)

    def split(W):
        W = np.asarray(W, dtype=f32)
        hi = W.astype(bf16)
        lo = (W - hi.astype(f32)).astype(bf16)
        return hi, lo

    wq_h, wq_l = split(Wq)
    wk_f = np.ascontiguousarray(np.asarray(Wk, dtype=f32))
    wv_f = np.ascontiguousarray(np.asarray(Wv, dtype=f32))
    wo_h, _ = split(Wo)
    e_sel = np.zeros((2, P), dtype=bf16)
    e_sel[0, :64] = 1
    e_sel[1, 64:] = 1

    in_maps = []
    for c in range(NCORES):
        b, half = c // 2, c % 2
        xs = x[b, half * TOK:(half + 1) * TOK]          # [2048, 1024]
        xhi = xs.astype(bf16)
        xlo = (xs - xhi.astype(f32)).astype(bf16)
        m = {
            "xtf": np.ascontiguousarray(xs.T),
            "xthi": np.ascontiguousarray(xhi.T),
            "xtlo": np.ascontiguousarray(xlo.T),
            "wqh": wq_h, "wql": wq_l,
            "wk": wk_f, "wv": wv_f, "woh": wo_h,
            "xres": np.ascontiguousarray(xs),
            "e_sel": e_sel,
        }
        if apply_bias:
            m.update(bq=np.asarray(bq, f32), bk=np.asarray(bk, f32),
                     bv=np.asarray(bv, f32), bo=np.asarray(bo, f32))
        if apply_gamma:
            m["gamma"] = np.asarray(gamma, f32)
        if apply_beta:
            m["beta"] = np.asarray(beta, f32)
        in_maps.append(m)

    import os
    try:
        LAST_RESULT = run_bass_kernel_spmd(nc, in_maps, core_ids=list(range(NCORES)))
    except ModuleNotFoundError:
        # no antenv.axon_hooks in this container -> NTFF tracing unavailable
        os.environ["BASS_NEVER_TRACE"] = "1"
        LAST_RESULT = run_bass_kernel_spmd(nc, in_maps, core_ids=list(range(NCORES)))
    out = np.empty((B, N, D), dtype=np.float32)
    for c in range(NCORES):
        b, half = c // 2, c % 2
        out[b, half * TOK:(half + 1) * TOK] = LAST_RESULT.results[c]["out"]
    return out


# revision 55
# speedup vs baseline: 1.0222x; 1.0038x over previous
"""Distributed Trainium2 kernel for the linear-attention transformer block.

Math (per batch element b):
  Q = elu(x @ Wq + bq), K = elu(x @ Wk + bk), V = x @ Wv + bv   (per-head d=64)
  KV_h = K_h^T V_h  [64,64];  Ksum_h = sum_n K_h[n]  [64]
  attn_h = (Q_h @ KV_h) / (Q_h . Ksum_h)
  out = LayerNorm(x + attn @ Wo + bo) * gamma + beta

Sharding: 16384 tokens over 8 cores (2048 each; core c owns batch c//2,
half c%2). Each core computes Q/K/V only for its tokens, partial KV/Ksum,
then a 266KB AllReduce over core pairs {2b, 2b+1} completes the KV stats;
attention + output projection + LayerNorm finish locally.

Precision: the 1/(Q.Ksum) denominators pass near zero (min |den| on the
nominal instance is ~0.35) and amplify projection noise into sign flips,
so the Q projection uses a 3-term bf16 split (x_hi@W_hi + x_hi@W_lo +
x_lo@W_hi; per-element err ~5e-6). The K/V projections run as single
float32r matmuls (full PE rate at N=512; measured HW err ~1.5e-4), which
keeps the Ksum-side denominator error ~0.07 rms - flip-safe with >5 sigma
margin. Ksum and the denominator run in f32. ~1.7e-3 global rel err.
"""

import sys

sys.path.insert(0, "/opt/trn_rl_repo")

import numpy as np
import ml_dtypes

import concourse.bass as bass
import concourse.bass_isa as bass_isa
import concourse.mybir as mybir
import concourse.tile as tile
from concourse import bacc
from concourse.bass_utils import run_bass_kernel_spmd

AF = mybir.ActivationFunctionType
OP = mybir.AluOpType
F32 = mybir.dt.float32
F32R = mybir.dt.float32r
BF16 = mybir.dt.bfloat16

B, N, D = 4, 4096, 1024
H, HD = 16, 64
TOK = 2048            # tokens per core
NCORES = 8
LN_EPS = 1e-3
P = 128
KC = D // P           # 8 contraction chunks
TC = TOK // P         # 16 token chunks of 128
TQ = TOK // 512       # 4 token chunks of 512
WARM_MM = 10          # PE clock-gate warmup matmuls

LAST_RESULT = None    # BassKernelResults of the most recent run (for test.py)
DEBUG_TAPS = False    # set True (e.g. from debug.py) to add intermediate outputs


def _build(apply_bias, apply_gamma, apply_beta):
    nc = bacc.Bacc("TRN2", target_bir_lowering=False, debug=False, num_devices=NCORES)

    def din(name, shape, dtype=BF16):
        return nc.dram_tensor(name, shape, dtype, kind="ExternalInput")

    xtf = din("xtf", [D, TOK], F32R)
    xthi = din("xthi", [D, TOK])
    xtlo = din("xtlo", [D, TOK])
    wqh = din("wqh", [D, D])
    wql = din("wql", [D, D])
    wk = din("wk", [D, D], F32R)
    wv = din("wv", [D, D], F32R)
    woh = din("woh", [D, D])
    xres = din("xres", [TOK, D], F32)
    e_sel = din("e_sel", [2, P])
    if apply_bias:
        bq_d = din("bq", [D], F32)
        bk_d = din("bk", [D], F32)
        bv_d = din("bv", [D], F32)
        bo_d = din("bo", [D], F32)
    if apply_gamma:
        gamma_d = din("gamma", [D], F32)
    if apply_beta:
        beta_d = din("beta", [D], F32)
    out_d = nc.dram_tensor("out", [TOK, D], F32, kind="ExternalOutput")
    if DEBUG_TAPS:
        dbg_k0 = nc.dram_tensor("dbg_k0", [P, 512], F32, kind="ExternalOutput")
        dbg_kv = nc.dram_tensor("dbg_kv", [P, 512], F32, kind="ExternalOutput")
        dbg_ksum = nc.dram_tensor("dbg_ksum", [P, KC], F32, kind="ExternalOutput")
        dbg_ar = nc.dram_tensor("dbg_ar", [P, 520], F32, kind="ExternalOutput")
        dbg_qt0 = nc.dram_tensor("dbg_qt0", [P, TOK], F32, kind="ExternalOutput")
        dbg_at = nc.dram_tensor("dbg_at", [P, KC, TOK], F32, kind="ExternalOutput")

    r8 = lambda t: t.ap().rearrange("(ko p) n -> p ko n", p=P)

    def bcast_row(dram_vec, sb_tile):
        # DMA-broadcast a [D] vector to [P, D] (stride-0 partition dim).
        src = bass.AP(
            tensor=dram_vec.ap().tensor,
            offset=dram_vec.ap().offset,
            ap=[[0, P]] + list(dram_vec.ap().ap),
        )
        nc.sync.dma_start(out=sb_tile, in_=src)

    with tile.TileContext(nc) as tc:
        with (
            tc.tile_pool(name="smalls", bufs=1) as smalls,
            tc.tile_pool(name="dram", bufs=1, space="DRAM") as dram,
        ):
            e_sb = smalls.tile([2, P], BF16)
            nc.sync.dma_start(e_sb[:], e_sel.ap())
            ones_sb = smalls.tile([P, 1], F32)
            nc.vector.memset(ones_sb[:], 1.0)
            eps_sb = smalls.tile([P, 1], F32)
            nc.vector.memset(eps_sb[:], LN_EPS)
            if apply_bias:
                bq_sb = smalls.tile([P, KC], F32)   # per-partition layout for Q^T
                nc.sync.dma_start(bq_sb[:], bq_d.ap().rearrange("(ko p) -> p ko", p=P))
                bk_b = smalls.tile([P, D], F32)
                bv_b = smalls.tile([P, D], F32)
                bo_b = smalls.tile([P, D], F32)
                bcast_row(bk_d, bk_b[:])
                bcast_row(bv_d, bv_b[:])
                bcast_row(bo_d, bo_b[:])
            if apply_gamma:
                gamma_b = smalls.tile([P, D], F32)
                bcast_row(gamma_d, gamma_b[:])
            if apply_beta:
                beta_b = smalls.tile([P, D], F32)
                bcast_row(beta_d, beta_b[:])

            if DEBUG_TAPS:
                kv_loc = smalls.tile([P, 512], F32)
                ksum_loc = smalls.tile([P, KC], F32)

            # PE warmup: matmuls on zeroed tiles release the HAM clock
            # gate (1.2->2.4 GHz) while the first input DMAs are in flight.
            with (
                tc.tile_pool(name="warmsb", bufs=1) as warmsb,
                tc.tile_pool(name="warmps", bufs=2, space="PSUM") as warmps,
            ):
                warm_a = warmsb.tile([P, P], BF16)
                warm_b = warmsb.tile([P, 512], BF16)
                nc.gpsimd.memset(warm_a[:], 0.0)
                nc.gpsimd.memset(warm_b[:], 0.0)
                for w in range(WARM_MM):
                    wp = warmps.tile([P, 512], F32, tag="warm", name=f"warm_{w}")
                    nc.tensor.matmul(wp[:], warm_a[:], warm_b[:], start=True, stop=True)

            # Prefetch the first two Q-weight slices; their DMAs have no deps
            # and fill otherwise-idle DMA time during phase 1.
            wqp_cm = tc.tile_pool(name="wqp", bufs=3)
            wqp = wqp_cm.__enter__()
            wq_tiles = {}

            def load_wq(hp):
                msl = slice(hp * P, (hp + 1) * P)
                wq_t = wqp.tile([P, KC, 2, P], BF16, tag="wq", name=f"wq_{hp}")
                nc.sync.dma_start(wq_t[:, :, 0, :], r8(wqh)[:, :, msl])
                nc.sync.dma_start(wq_t[:, :, 1, :], r8(wql)[:, :, msl])
                wq_tiles[hp] = wq_t

            # ================= Phase 1: K, V, partial KV + Ksum =================
            # K = x @ Wk and V = x @ Wv as single f32r matmuls (full PE rate
            # at N=512). x^T is resident in f32; the four psum accumulators
            # (K/V x dh halves) share each stationary x^T chunk.
            with (
                tc.tile_pool(name="ph1x", bufs=1) as ph1x,
                tc.tile_pool(name="wkv", bufs=1) as wkv,
                tc.tile_pool(name="kvps_pool", bufs=2, space="PSUM") as kvps_pool,
                tc.tile_pool(name="ph1ps", bufs=6, space="PSUM") as ph1ps,
                tc.tile_pool(name="ph1sb", bufs=4) as ph1sb,
            ):
                xtf_sb = ph1x.tile([P, KC, TOK], F32R)
                wk_sb = wkv.tile([P, KC, D], F32R)
                wv_sb = wkv.tile([P, KC, D], F32R)
                # DMA order: first t=0..1 x chunks + Wk (needed by t=0 K
                # matmuls), then Wv, then remaining x token-sliced t-major.
                for k in range(KC):
                    nc.sync.dma_start(wk_sb[:, k, :], r8(wk)[:, k, :])
                    nc.sync.dma_start(xtf_sb[:, k, 0:2 * P], r8(xtf)[:, k, 0:2 * P])
                for k in range(KC):
                    nc.sync.dma_start(wv_sb[:, k, :], r8(wv)[:, k, :])
                for t in range(2, TC):
                    ts = slice(t * P, (t + 1) * P)
                    for k in range(KC):
                        nc.sync.dma_start(xtf_sb[:, k, ts], r8(xtf)[:, k, ts])
                load_wq(0)
                load_wq(1)

                # SBUF accumulator (DVE-add per token chunk; interleaved
                # multi-chunk PSUM accumulation groups proved unreliable).
                # Layout per dh half: [dh*260, dh*260+256) = KV, [+256, +260) = Ksum.
                acc = smalls.tile([P, 520], F32)
                nc.vector.memset(acc[:], 0.0)

                for t in range(TC):
                    ts = slice(t * P, (t + 1) * P)
                    kps = {}
                    vps = {}
                    for dh in range(2):
                        kps[dh] = ph1ps.tile([P, 512], F32, tag="proj",
                                             name=f"kps_{t}_{dh}")
                        vps[dh] = ph1ps.tile([P, 512], F32, tag="proj",
                                             name=f"vps_{t}_{dh}")
                    for k in range(KC):
                        st, sp = (k == 0), (k == KC - 1)
                        for dh in range(2):
                            dsl = slice(dh * 512, (dh + 1) * 512)
                            nc.tensor.matmul(kps[dh][:], xtf_sb[:, k, ts],
                                             wk_sb[:, k, dsl], start=st, stop=sp)
                            nc.tensor.matmul(vps[dh][:], xtf_sb[:, k, ts],
                                             wv_sb[:, k, dsl], start=st, stop=sp)
                    kb_chunks = []
                    kvs_tiles = {}
                    for dh in range(2):
                        dsl = slice(dh * 512, (dh + 1) * 512)
                        if apply_bias:
                            kraw = ph1sb.tile([P, 512], F32, tag="kraw", name=f"kraw_{t}_{dh}")
                            nc.vector.tensor_tensor(kraw[:], kps[dh][:], bk_b[:, dsl], OP.add)
                            ksrc = kraw
                        else:
                            ksrc = kps[dh]
                        kmin = ph1sb.tile([P, 512], F32, tag="kmin", name=f"kmin_{t}_{dh}")
                        nc.vector.tensor_scalar(kmin[:], ksrc[:], 0.0, None, OP.min)
                        kexp = ph1sb.tile([P, 512], F32, tag="kexp", name=f"kexp_{t}_{dh}")
                        nc.scalar.activation(kexp[:], kmin[:], AF.Exp)
                        kmax = ph1sb.tile([P, 512], F32, tag="kmax", name=f"kmax_{t}_{dh}")
                        nc.vector.tensor_scalar(kmax[:], ksrc[:], 0.0, -1.0, OP.max, OP.add)
                        kf = ph1sb.tile([P, 512], F32, tag="kf", name=f"kf_{t}_{dh}")
                        nc.vector.tensor_tensor(kf[:], kmax[:], kexp[:], OP.add)
                        kb = ph1sb.tile([P, 512], BF16, tag="kb", name=f"kb_{t}_{dh}")
                        nc.vector.tensor_copy(kb[:], kf[:])
                        if DEBUG_TAPS and t == 0 and dh == 0:
                            nc.sync.dma_start(dbg_k0.ap(), kf[:])
                        kb_chunks.append(kb)
                        # Ksum column blocks (f32 matmul against ones) go into
                        # cols [256, 260) of the shared kvs_t psum tile.
                        kvs_t = kvps_pool.tile([P, 260], F32, tag="kvs_t",
                                               name=f"kvs_t_{t}_{dh}", bufs=1)
                        kvs_tiles[dh] = kvs_t
                        for j in range(4):
                            nc.tensor.matmul(
                                kvs_t[:, 256 + j:257 + j], kf[:, j * P:(j + 1) * P],
                                ones_sb[:], start=True, stop=True, skip_group_check=True)
                    for dh in range(2):
                        dsl = slice(dh * 512, (dh + 1) * 512)
                        vb = ph1sb.tile([P, 512], BF16, tag="vb", name=f"vb_{t}_{dh}")
                        if apply_bias:
                            nc.vector.tensor_tensor(vb[:], vps[dh][:], bv_b[:, dsl], OP.add)
                        else:
                            nc.any.tensor_copy(vb[:], vps[dh][:])
                        kb = kb_chunks[dh]
                        kvs_t = kvs_tiles[dh]
                        for hh in range(8):
                            h = dh * 8 + hh
                            pr = (h % 2) * 64
                            fc = (h // 2) * 64 - dh * 256
                            nc.tensor.matmul(
                                kvs_t[pr:pr + 64, fc:fc + 64],
                                kb[:, hh * 64:(hh + 1) * 64],
                                vb[:, hh * 64:(hh + 1) * 64],
                                start=True, stop=True,
                                tile_position=(0, pr), skip_group_check=True)
                        nc.vector.tensor_tensor(
                            acc[:, dh * 260:(dh + 1) * 260],
                            acc[:, dh * 260:(dh + 1) * 260], kvs_t[:], OP.add)

                if DEBUG_TAPS:
                    nc.vector.tensor_copy(kv_loc[:, :256], acc[:, :256])
                    nc.vector.tensor_copy(kv_loc[:, 256:], acc[:, 260:516])
                    nc.vector.tensor_copy(ksum_loc[:, :4], acc[:, 256:260])
                    nc.vector.tensor_copy(ksum_loc[:, 4:], acc[:, 516:520])
                    nc.sync.dma_start(dbg_kv.ap(), kv_loc[:])
                    nc.sync.dma_start(dbg_ksum.ap(), ksum_loc[:])

            # ========== Phases 2-4: AllReduce; Q^T; attention (pipelined) ==========
            with (
                tc.tile_pool(name="qx", bufs=1) as qx,
                tc.tile_pool(name="late", bufs=1) as late,
            ):
                # bf16 hi/lo x^T for the Q 3-term split; DMA'd into the SBUF
                # space phase 1 just freed, overlapping the AllReduce.
                xthi_sb = qx.tile([P, KC, TOK], BF16)
                xtlo_sb = qx.tile([P, KC, TOK], BF16)
                for k in range(KC):
                    nc.sync.dma_start(xthi_sb[:, k, :], r8(xthi)[:, k, :])
                    nc.sync.dma_start(xtlo_sb[:, k, :], r8(xtlo)[:, k, :])

                at_sb = late.tile([P, KC, TOK], BF16)
                woh_sb = late.tile([P, KC, D], BF16)

                with (
                    tc.tile_pool(name="qtp", bufs=4) as qtp,
                    tc.tile_pool(name="ph3ps", bufs=4, space="PSUM") as ph3ps,
                    tc.tile_pool(name="ph3sb", bufs=2) as ph3sb,
                    tc.tile_pool(name="ph4ps_a", bufs=2, space="PSUM") as ph4ps_a,
                    tc.tile_pool(name="ph4sb", bufs=3) as ph4sb,
                ):
                    qt_tiles = {}

                    # -- AllReduce of the packed KV/Ksum accumulator --
                    cc_in = dram.tile([P, 520], F32)
                    cc_out = dram.tile([P, 520], F32)
                    nc.sync.dma_start(cc_in[:], acc[:])
                    nc.gpsimd.collective_compute(
                        "AllReduce", OP.add,
                        replica_groups=[[0, 1], [2, 3], [4, 5], [6, 7]],
                        ins=[cc_in[:].opt()], outs=[cc_out[:].opt()])
                    ar_sb = smalls.tile([P, 520], F32)
                    nc.sync.dma_start(ar_sb[:], cc_out[:])
                    if DEBUG_TAPS:
                        nc.sync.dma_start(dbg_ar.ap(), ar_sb[:])
                    kv_bf = smalls.tile([P, 512], BF16)
                    nc.any.tensor_copy(kv_bf[:, :256], ar_sb[:, :256])
                    nc.any.tensor_copy(kv_bf[:, 256:], ar_sb[:, 260:516])
                    # kdp col hp: Ksum_{2hp}/2 on partitions 0-63 and
                    # Ksum_{2hp+1}/2 on 64-127 -- aligned with qt's partition
                    # layout. den halves come from two full-128 partition
                    # reduces (the gpsimd ucode only supports base partition
                    # 0): A = sum(qt*kdp) = (den_e+den_o)/2 over all 128,
                    # B = sum(qt*kdps) = (den_e-den_o)/2 with kdps = +-kdp,
                    # so den_e = A+B (rows 0-63) and den_o = A-B (rows 64+).
                    kdp = smalls.tile([P, KC], F32)
                    for h in range(H):
                        pr = (h % 2) * 64
                        c = h // 2
                        sc = 256 + c if c < 4 else 516 + (c - 4)
                        nc.vector.tensor_scalar(
                            kdp[pr:pr + 64, h // 2:h // 2 + 1],
                            ar_sb[pr:pr + 64, sc:sc + 1], 0.5, None, OP.mult)
                    sgn = smalls.tile([P, 1], F32)
                    nc.vector.memset(sgn[0:64, :], 1.0)
                    nc.vector.memset(sgn[64:128, :], -1.0)
                    kdps = smalls.tile([P, KC], F32)
                    nc.vector.tensor_scalar(kdps[:], kdp[:], sgn[:, 0:1], None, OP.mult)
                    # Block-diagonal KV stationary per head pair: one 128-wide
                    # matmul computes both heads' attention numerators.
                    kvq = smalls.tile([P, KC, P], BF16)
                    nc.any.memset(kvq[:], 0.0)
                    for hp in range(KC):
                        nc.any.tensor_copy(kvq[0:64, hp, 0:64],
                                           kv_bf[0:64, hp * 64:(hp + 1) * 64])
                        nc.any.tensor_copy(kvq[64:128, hp, 64:128],
                                           kv_bf[64:128, hp * 64:(hp + 1) * 64])

                    for k in range(KC):
                        nc.sync.dma_start(woh_sb[:, k, :], r8(woh)[:, k, :])

                    def q_proj(hp):
                        wq_t = wq_tiles.pop(hp)
                        qt = qtp.tile([P, TOK], F32, tag="qt", name=f"qt_{hp}")
                        qt_tiles[hp] = qt
                        # k-outer over all four tq psum tiles: each arriving
                        # x^T chunk immediately feeds 12 matmuls, so the
                        # hp=0 wave overlaps the xthi/xtlo DMA chunk-by-chunk.
                        qps_t = [ph3ps.tile([P, 512], F32, tag="qps",
                                            name=f"qps_{hp}_{tq}") for tq in range(TQ)]
                        for k in range(KC):
                            st, sp = (k == 0), (k == KC - 1)
                            for tq in range(TQ):
                                tsl = slice(tq * 512, (tq + 1) * 512)
                                nc.tensor.matmul(qps_t[tq][:], wq_t[:, k, 0, :],
                                                 xthi_sb[:, k, tsl], start=st, stop=False)
                                nc.tensor.matmul(qps_t[tq][:], wq_t[:, k, 1, :],
                                                 xthi_sb[:, k, tsl], start=False, stop=False)
                                nc.tensor.matmul(qps_t[tq][:], wq_t[:, k, 0, :],
                                                 xtlo_sb[:, k, tsl], start=False, stop=sp)
                        for tq in range(TQ):
                            tsl = slice(tq * 512, (tq + 1) * 512)
                            qps = qps_t[tq]
                            if apply_bias:
                                qraw = ph3sb.tile([P, 512], F32, tag="qraw",
                                                  name=f"qraw_{hp}_{tq}")
                                nc.vector.tensor_scalar(qraw[:], qps[:], bq_sb[:, hp:hp + 1],
                                                        None, OP.add)
                                qsrc = qraw
                            else:
                                qsrc = qps
                            qmin = ph3sb.tile([P, 512], F32, tag="qmin", name=f"qmin_{hp}_{tq}")
                            nc.vector.tensor_scalar(qmin[:], qsrc[:], 0.0, None, OP.min)
                            qexp = ph3sb.tile([P, 512], F32, tag="qexp", name=f"qexp_{hp}_{tq}")
                            nc.scalar.activation(qexp[:], qmin[:], AF.Exp)
                            qmax = ph3sb.tile([P, 512], F32, tag="qmax", name=f"qmax_{hp}_{tq}")
                            nc.vector.tensor_scalar(qmax[:], qsrc[:], 0.0, -1.0, OP.max, OP.add)
                            nc.vector.tensor_tensor(qt[:, tsl], qmax[:], qexp[:], OP.add)

                    def attention(hp):
                        qt = qt_tiles.pop(hp)
                        if DEBUG_TAPS and hp == 0:
                            nc.sync.dma_start(dbg_qt0.ap(), qt[:])
                        for tq in range(TQ):
                            tsl = slice(tq * 512, (tq + 1) * 512)
                            # den on gpsimd+DVE (keeps PE free); see kdp note.
                            prod = ph4sb.tile([P, 512], F32, tag="prod",
                                              name=f"prod_{hp}_{tq}")
                            nc.vector.tensor_scalar(prod[:], qt[:, tsl],
                                                    kdp[:, hp:hp + 1], None, OP.mult)
                            sprd = ph4sb.tile([P, 512], F32, tag="sprd",
                                              name=f"sprd_{hp}_{tq}")
                            nc.vector.tensor_scalar(sprd[:], qt[:, tsl],
                                                    kdps[:, hp:hp + 1], None, OP.mult)
                            denA = ph4sb.tile([P, 512], F32, tag="denA",
                                              name=f"denA_{hp}_{tq}")
                            denB = ph4sb.tile([P, 512], F32, tag="denB",
                                              name=f"denB_{hp}_{tq}")
                            nc.gpsimd.partition_all_reduce(
                                denA[:], prod[:], channels=128,
                                reduce_op=bass_isa.ReduceOp.add)
                            nc.gpsimd.partition_all_reduce(
                                denB[:], sprd[:], channels=128,
                                reduce_op=bass_isa.ReduceOp.add)
                            nc.vector.tensor_tensor(denA[0:64, :], denA[0:64, :],
                                                    denB[0:64, :], OP.add)
                            nc.vector.tensor_tensor(denA[64:128, :], denA[64:128, :],
                                                    denB[64:128, :], OP.subtract)
                            nc.vector.reciprocal(denA[:], denA[:])
                            qbf = ph4sb.tile([P, 512], BF16, tag="qbf", name=f"qbf_{hp}_{tq}")
                            nc.vector.tensor_copy(qbf[:], qt[:, tsl])
                            aps = ph4ps_a.tile([P, 512], F32, tag="aps", name=f"aps_{hp}_{tq}")
                            nc.tensor.matmul(aps[:], kvq[:, hp, :], qbf[:],
                                             start=True, stop=True)
                            nc.vector.tensor_tensor(at_sb[:, hp, tsl], aps[:], denA[:], OP.mult)

                    # depth-2 software pipeline: attention(hp) runs two Q chunks
                    # behind, so the AllReduce hides under ~3 Q projections.
                    # Software pipeline: depth 2 while the AllReduce is in
                    # flight, catching up to depth 1 at hp=4 so only
                    # attention(7)'s den chain trails into phase 5.
                    att_sched = {2: [0], 3: [1], 4: [2, 3], 5: [4], 6: [5], 7: [6]}
                    q_proj(0)
                    for hp in range(1, KC):
                        if hp + 1 < KC:
                            load_wq(hp + 1)
                        q_proj(hp)
                        for a in att_sched.get(hp, []):
                            attention(a)
                    attention(KC - 1)

                if DEBUG_TAPS:
                    with tc.tile_pool(name="dbgat", bufs=2) as dbgat:
                        for c in range(KC):
                            atf = dbgat.tile([P, TOK], F32, tag="atf", name=f"atf_{c}")
                            nc.vector.tensor_copy(atf[:], at_sb[:, c, :])
                            nc.sync.dma_start(dbg_at.ap()[:, c, :], atf[:])

                # ===== Phase 5: output projection + residual + LayerNorm =====
                with (
                    tc.tile_pool(name="ph5ps", bufs=4, space="PSUM") as ph5ps,
                    tc.tile_pool(name="ph5sb", bufs=4) as ph5sb,
                ):
                    for t in range(TC):
                        ts = slice(t * P, (t + 1) * P)
                        y = ph5sb.tile([P, D], F32, tag="y", name=f"y_{t}")
                        xr = ph5sb.tile([P, D], F32, tag="xr", name=f"xr_{t}")
                        nc.sync.dma_start(xr[:], xres.ap()[ts, :])
                        ops = ph5ps.tile([P, D], F32, tag="ops", name=f"ops_{t}")
                        for dh in range(2):
                            dsl = slice(dh * 512, (dh + 1) * 512)
                            for c in range(KC):
                                nc.tensor.matmul(ops[:, dsl], at_sb[:, c, ts], woh_sb[:, c, dsl],
                                                 start=(c == 0), stop=(c == KC - 1))
                        nc.vector.tensor_tensor(y[:], ops[:], xr[:], OP.add)
                        if apply_bias:
                            nc.vector.tensor_tensor(y[:], y[:], bo_b[:], OP.add)
                        stats = ph5sb.tile([P, 2, 6], F32, tag="stats", name=f"stats_{t}")
                        nc.vector.bn_stats(out=stats[:, 0, :], in_=y[:, :512])
                        nc.vector.bn_stats(out=stats[:, 1, :], in_=y[:, 512:])
                        mv = ph5sb.tile([P, 2], F32, tag="mv", name=f"mv_{t}")
                        nc.vector.bn_aggr(out=mv[:], in_=stats[:])
                        nc.scalar.activation(out=mv[:, 1:2], in_=mv[:, 1:2], func=AF.Sqrt,
                                             bias=eps_sb[:], scale=1.0)
                        nc.vector.reciprocal(mv[:, 1:2], mv[:, 1:2])
                        yo = ph5sb.tile([P, D], F32, tag="yo", name=f"yo_{t}")
                        nc.gpsimd.tensor_scalar(yo[:], y[:], mv[:, 0:1], mv[:, 1:2],
                                                OP.subtract, OP.mult)
                        if apply_gamma:
                            nc.vector.tensor_tensor(yo[:], yo[:], gamma_b[:], OP.mult)
                        if apply_beta:
                            nc.vector.tensor_tensor(yo[:], yo[:], beta_b[:], OP.add)
                        nc.sync.dma_start(out_d.ap()[ts, :], yo[:])

            wqp_cm.__exit__(None, None, None)

    nc.compile()
    return nc


def kernel(x, Wq, bq, Wk, bk, Wv, bv, Wo, bo, gamma, beta):
    global LAST_RESULT
    x = np.asarray(x, dtype=np.float32)
    f32 = np.float32
    bf16 = ml_dtypes.bfloat16

    apply_bias = any(np.any(np.asarray(b)) for b in (bq, bk, bv, bo))
    apply_gamma = not np.all(np.asarray(gamma) == 1.0)
    apply_beta = bool(np.any(np.asarray(beta)))

    nc = _build(apply_bias, apply_gamma, apply_beta)

    def split(W):
        W = np.asarray(W, dtype=f32)
        hi = W.astype(bf16)
        lo = (W - hi.astype(f32)).astype(bf16)
        return hi, lo

    wq_h, wq_l = split(Wq)
    wk_f = np.ascontiguousarray(np.asarray(Wk, dtype=f32))
    wv_f = np.ascontiguousarray(np.asarray(Wv, dtype=f32))
    wo_h, _ = split(Wo)
    e_sel = np.zeros((2, P), dtype=bf16)
    e_sel[0, :64] = 1
    e_sel[1, 64:] = 1

    in_maps = []
    for c in range(NCORES):
        b, half = c // 2, c % 2
        xs = x[b, half * TOK:(half + 1) * TOK]          # [2048, 1024]
        xhi = xs.astype(bf16)
        xlo = (xs - xhi.astype(f32)).astype(bf16)
        m = {
            "xtf": np.ascontiguousarray(xs.T),
            "xthi": np.ascontiguousarray(xhi.T),
            "xtlo": np.ascontiguousarray(xlo.T),
            "wqh": wq_h, "wql": wq_l,
            "wk": wk_f, "wv": wv_f, "woh": wo_h,
            "xres": np.ascontiguousarray(xs),
            "e_sel": e_sel,
        }
        if apply_bias:
            m.update(bq=np.asarray(bq, f32), bk=np.asarray(bk, f32),
                     bv=np.asarray(bv, f32), bo=np.asarray(bo, f32))
        if apply_gamma:
            m["gamma"] = np.asarray(gamma, f32)
        if apply_beta:
            m["beta"] = np.asarray(beta, f32)
        in_maps.append(m)

    import os
    try:
        LAST_RESULT = run_bass_kernel_spmd(nc, in_maps, core_ids=list(range(NCORES)))
    except ModuleNotFoundError:
        # no antenv.axon_hooks in this container -> NTFF tracing unavailable
        os.environ["BASS_NEVER_TRACE"] = "1"
        LAST_RESULT = run_bass_kernel_spmd(nc, in_maps, core_ids=list(range(NCORES)))
    out = np.empty((B, N, D), dtype=np.float32)
    for c in range(NCORES):
        b, half = c // 2, c % 2
        out[b, half * TOK:(half + 1) * TOK] = LAST_RESULT.results[c]["out"]
    return out


# revision 57
# speedup vs baseline: 1.0289x; 1.0065x over previous
"""Distributed Trainium2 kernel for the linear-attention transformer block.

Math (per batch element b):
  Q = elu(x @ Wq + bq), K = elu(x @ Wk + bk), V = x @ Wv + bv   (per-head d=64)
  KV_h = K_h^T V_h  [64,64];  Ksum_h = sum_n K_h[n]  [64]
  attn_h = (Q_h @ KV_h) / (Q_h . Ksum_h)
  out = LayerNorm(x + attn @ Wo + bo) * gamma + beta

Sharding: 16384 tokens over 8 cores (2048 each; core c owns batch c//2,
half c%2). Each core computes Q/K/V only for its tokens, partial KV/Ksum,
then a 266KB AllReduce over core pairs {2b, 2b+1} completes the KV stats;
attention + output projection + LayerNorm finish locally.

Precision: the 1/(Q.Ksum) denominators pass near zero (min |den| on the
nominal instance is ~0.35) and amplify projection noise into sign flips,
so the Q projection uses a 3-term bf16 split (x_hi@W_hi + x_hi@W_lo +
x_lo@W_hi; per-element err ~5e-6). The K/V projections run as single
float32r matmuls (full PE rate at N=512; measured HW err ~1.5e-4), which
keeps the Ksum-side denominator error ~0.07 rms - flip-safe with >5 sigma
margin. Ksum and the denominator run in f32. ~1.7e-3 global rel err.
"""

import sys

sys.path.insert(0, "/opt/trn_rl_repo")

import numpy as np
import ml_dtypes

import concourse.bass as bass
import concourse.bass_isa as bass_isa
import concourse.mybir as mybir
import concourse.tile as tile
from concourse import bacc
from concourse.bass_utils import run_bass_kernel_spmd

AF = mybir.ActivationFunctionType
OP = mybir.AluOpType
F32 = mybir.dt.float32
F32R = mybir.dt.float32r
BF16 = mybir.dt.bfloat16

B, N, D = 4, 4096, 1024
H, HD = 16, 64
TOK = 2048            # tokens per core
NCORES = 8
LN_EPS = 1e-3
P = 128
KC = D // P           # 8 contraction chunks
TC = TOK // P         # 16 token chunks of 128
TQ = TOK // 512       # 4 token chunks of 512
WARM_MM = 10          # PE clock-gate warmup matmuls

LAST_RESULT = None    # BassKernelResults of the most recent run (for test.py)
DEBUG_TAPS = False    # set True (e.g. from debug.py) to add intermediate outputs


def _build(apply_bias, apply_gamma, apply_beta):
    nc = bacc.Bacc("TRN2", target_bir_lowering=False, debug=False, num_devices=NCORES)

    def din(name, shape, dtype=BF16):
        return nc.dram_tensor(name, shape, dtype, kind="ExternalInput")

    xtf = din("xtf", [D, TOK], F32R)
    xthi = din("xthi", [D, TOK])
    xtlo = din("xtlo", [D, TOK])
    wqh = din("wqh", [D, D])
    wql = din("wql", [D, D])
    wk = din("wk", [D, D], F32R)
    wv = din("wv", [D, D], F32R)
    woh = din("woh", [D, D])
    xres = din("xres", [TOK, D], F32)
    e_sel = din("e_sel", [2, P])
    if apply_bias:
        bq_d = din("bq", [D], F32)
        bk_d = din("bk", [D], F32)
        bv_d = din("bv", [D], F32)
        bo_d = din("bo", [D], F32)
    if apply_gamma:
        gamma_d = din("gamma", [D], F32)
    if apply_beta:
        beta_d = din("beta", [D], F32)
    out_d = nc.dram_tensor("out", [TOK, D], F32, kind="ExternalOutput")
    if DEBUG_TAPS:
        dbg_k0 = nc.dram_tensor("dbg_k0", [P, 512], F32, kind="ExternalOutput")
        dbg_kv = nc.dram_tensor("dbg_kv", [P, 512], F32, kind="ExternalOutput")
        dbg_ksum = nc.dram_tensor("dbg_ksum", [P, KC], F32, kind="ExternalOutput")
        dbg_ar = nc.dram_tensor("dbg_ar", [P, 520], F32, kind="ExternalOutput")
        dbg_qt0 = nc.dram_tensor("dbg_qt0", [P, TOK], F32, kind="ExternalOutput")
        dbg_at = nc.dram_tensor("dbg_at", [P, KC, TOK], F32, kind="ExternalOutput")

    r8 = lambda t: t.ap().rearrange("(ko p) n -> p ko n", p=P)

    def bcast_row(dram_vec, sb_tile):
        # DMA-broadcast a [D] vector to [P, D] (stride-0 partition dim).
        src = bass.AP(
            tensor=dram_vec.ap().tensor,
            offset=dram_vec.ap().offset,
            ap=[[0, P]] + list(dram_vec.ap().ap),
        )
        nc.sync.dma_start(out=sb_tile, in_=src)

    with tile.TileContext(nc) as tc:
        with (
            tc.tile_pool(name="smalls", bufs=1) as smalls,
            tc.tile_pool(name="dram", bufs=1, space="DRAM") as dram,
        ):
            e_sb = smalls.tile([2, P], BF16)
            nc.sync.dma_start(e_sb[:], e_sel.ap())
            ones_sb = smalls.tile([P, 1], F32)
            nc.vector.memset(ones_sb[:], 1.0)
            eps_sb = smalls.tile([P, 1], F32)
            nc.vector.memset(eps_sb[:], LN_EPS)
            if apply_bias:
                bq_sb = smalls.tile([P, KC], F32)   # per-partition layout for Q^T
                nc.sync.dma_start(bq_sb[:], bq_d.ap().rearrange("(ko p) -> p ko", p=P))
                bk_b = smalls.tile([P, D], F32)
                bv_b = smalls.tile([P, D], F32)
                bo_b = smalls.tile([P, D], F32)
                bcast_row(bk_d, bk_b[:])
                bcast_row(bv_d, bv_b[:])
                bcast_row(bo_d, bo_b[:])
            if apply_gamma:
                gamma_b = smalls.tile([P, D], F32)
                bcast_row(gamma_d, gamma_b[:])
            if apply_beta:
                beta_b = smalls.tile([P, D], F32)
                bcast_row(beta_d, beta_b[:])

            if DEBUG_TAPS:
                kv_loc = smalls.tile([P, 512], F32)
                ksum_loc = smalls.tile([P, KC], F32)

            # PE warmup: matmuls on zeroed tiles release the HAM clock
            # gate (1.2->2.4 GHz) while the first input DMAs are in flight.
            with (
                tc.tile_pool(name="warmsb", bufs=1) as warmsb,
                tc.tile_pool(name="warmps", bufs=2, space="PSUM") as warmps,
            ):
                warm_a = warmsb.tile([P, P], BF16)
                warm_b = warmsb.tile([P, 512], BF16)
                nc.gpsimd.memset(warm_a[:], 0.0)
                nc.gpsimd.memset(warm_b[:], 0.0)
                for w in range(WARM_MM):
                    wp = warmps.tile([P, 512], F32, tag="warm", name=f"warm_{w}")
                    nc.tensor.matmul(wp[:], warm_a[:], warm_b[:], start=True, stop=True)

            # Prefetch the first two Q-weight slices; their DMAs have no deps
            # and fill otherwise-idle DMA time during phase 1.
            wqp_cm = tc.tile_pool(name="wqp", bufs=3)
            wqp = wqp_cm.__enter__()
            wq_tiles = {}

            def load_wq(hp):
                msl = slice(hp * P, (hp + 1) * P)
                wq_t = wqp.tile([P, KC, 2, P], BF16, tag="wq", name=f"wq_{hp}")
                nc.sync.dma_start(wq_t[:, :, 0, :], r8(wqh)[:, :, msl])
                nc.sync.dma_start(wq_t[:, :, 1, :], r8(wql)[:, :, msl])
                wq_tiles[hp] = wq_t

            # ================= Phase 1: K, V, partial KV + Ksum =================
            # K = x @ Wk and V = x @ Wv as single f32r matmuls (full PE rate
            # at N=512). x^T is resident in f32; the four psum accumulators
            # (K/V x dh halves) share each stationary x^T chunk.
            with (
                tc.tile_pool(name="ph1x", bufs=1) as ph1x,
                tc.tile_pool(name="wkv", bufs=1) as wkv,
                tc.tile_pool(name="kvps_pool", bufs=1, space="PSUM") as kvps_pool,
                tc.tile_pool(name="ph1ps", bufs=7, space="PSUM") as ph1ps,
                tc.tile_pool(name="ph1sb", bufs=4) as ph1sb,
            ):
                xtf_sb = ph1x.tile([P, KC, TOK], F32R)
                wk_sb = wkv.tile([P, KC, D], F32R)
                wv_sb = wkv.tile([P, KC, D], F32R)
                # DMA order: first t=0..1 x chunks + Wk (needed by t=0 K
                # matmuls), then Wv, then remaining x token-sliced t-major.
                for k in range(KC):
                    nc.sync.dma_start(wk_sb[:, k, :], r8(wk)[:, k, :])
                    nc.sync.dma_start(xtf_sb[:, k, 0:2 * P], r8(xtf)[:, k, 0:2 * P])
                for k in range(KC):
                    nc.sync.dma_start(wv_sb[:, k, :], r8(wv)[:, k, :])
                for t in range(2, TC):
                    ts = slice(t * P, (t + 1) * P)
                    for k in range(KC):
                        nc.sync.dma_start(xtf_sb[:, k, ts], r8(xtf)[:, k, ts])
                load_wq(0)
                load_wq(1)

                # SBUF accumulator (DVE-add per token chunk; interleaved
                # multi-chunk PSUM accumulation groups proved unreliable).
                # Layout per dh half: [dh*260, dh*260+256) = KV, [+256, +260) = Ksum.
                acc = smalls.tile([P, 520], F32)
                nc.vector.memset(acc[:], 0.0)

                for t in range(TC):
                    ts = slice(t * P, (t + 1) * P)
                    kps = {}
                    vps = {}
                    for dh in range(2):
                        kps[dh] = ph1ps.tile([P, 512], F32, tag="proj",
                                             name=f"kps_{t}_{dh}")
                        vps[dh] = ph1ps.tile([P, 512], F32, tag="proj",
                                             name=f"vps_{t}_{dh}")
                    for k in range(KC):
                        st, sp = (k == 0), (k == KC - 1)
                        for dh in range(2):
                            dsl = slice(dh * 512, (dh + 1) * 512)
                            nc.tensor.matmul(kps[dh][:], xtf_sb[:, k, ts],
                                             wk_sb[:, k, dsl], start=st, stop=sp)
                            nc.tensor.matmul(vps[dh][:], xtf_sb[:, k, ts],
                                             wv_sb[:, k, dsl], start=st, stop=sp)
                    kb_chunks = []
                    kvs_tiles = {}
                    for dh in range(2):
                        dsl = slice(dh * 512, (dh + 1) * 512)
                        if apply_bias:
                            kraw = ph1sb.tile([P, 512], F32, tag="kraw", name=f"kraw_{t}_{dh}")
                            nc.vector.tensor_tensor(kraw[:], kps[dh][:], bk_b[:, dsl], OP.add)
                            ksrc = kraw
                        else:
                            ksrc = kps[dh]
                        kmin = ph1sb.tile([P, 512], F32, tag="kmin", name=f"kmin_{t}_{dh}")
                        nc.vector.tensor_scalar(kmin[:], ksrc[:], 0.0, None, OP.min)
                        kexp = ph1sb.tile([P, 512], F32, tag="kexp", name=f"kexp_{t}_{dh}")
                        nc.scalar.activation(kexp[:], kmin[:], AF.Exp)
                        kmax = ph1sb.tile([P, 512], F32, tag="kmax", name=f"kmax_{t}_{dh}")
                        nc.vector.tensor_scalar(kmax[:], ksrc[:], 0.0, -1.0, OP.max, OP.add)
                        kf = ph1sb.tile([P, 512], F32, tag="kf", name=f"kf_{t}_{dh}")
                        nc.vector.tensor_tensor(kf[:], kmax[:], kexp[:], OP.add)
                        kb = ph1sb.tile([P, 512], BF16, tag="kb", name=f"kb_{t}_{dh}")
                        nc.vector.tensor_copy(kb[:], kf[:])
                        if DEBUG_TAPS and t == 0 and dh == 0:
                            nc.sync.dma_start(dbg_k0.ap(), kf[:])
                        kb_chunks.append(kb)
                        # Ksum column blocks (f32 matmul against ones) go into
                        # cols [256, 260) of the shared kvs_t psum tile.
                        kvs_t = kvps_pool.tile([P, 260], F32, tag="kvs_t",
                                               name=f"kvs_t_{t}_{dh}", bufs=1)
                        kvs_tiles[dh] = kvs_t
                        for j in range(4):
                            nc.tensor.matmul(
                                kvs_t[:, 256 + j:257 + j], kf[:, j * P:(j + 1) * P],
                                ones_sb[:], start=True, stop=True, skip_group_check=True)
                    for dh in range(2):
                        dsl = slice(dh * 512, (dh + 1) * 512)
                        vb = ph1sb.tile([P, 512], BF16, tag="vb", name=f"vb_{t}_{dh}")
                        if apply_bias:
                            nc.vector.tensor_tensor(vb[:], vps[dh][:], bv_b[:, dsl], OP.add)
                        else:
                            nc.any.tensor_copy(vb[:], vps[dh][:])
                        kb = kb_chunks[dh]
                        kvs_t = kvs_tiles[dh]
                        for hh in range(8):
                            h = dh * 8 + hh
                            pr = (h % 2) * 64
                            fc = (h // 2) * 64 - dh * 256
                            nc.tensor.matmul(
                                kvs_t[pr:pr + 64, fc:fc + 64],
                                kb[:, hh * 64:(hh + 1) * 64],
                                vb[:, hh * 64:(hh + 1) * 64],
                                start=True, stop=True,
                                tile_position=(0, pr), skip_group_check=True)
                        nc.vector.tensor_tensor(
                            acc[:, dh * 260:(dh + 1) * 260],
                            acc[:, dh * 260:(dh + 1) * 260], kvs_t[:], OP.add)

                if DEBUG_TAPS:
                    nc.vector.tensor_copy(kv_loc[:, :256], acc[:, :256])
                    nc.vector.tensor_copy(kv_loc[:, 256:], acc[:, 260:516])
                    nc.vector.tensor_copy(ksum_loc[:, :4], acc[:, 256:260])
                    nc.vector.tensor_copy(ksum_loc[:, 4:], acc[:, 516:520])
                    nc.sync.dma_start(dbg_kv.ap(), kv_loc[:])
                    nc.sync.dma_start(dbg_ksum.ap(), ksum_loc[:])

            # ========== Phases 2-4: AllReduce; Q^T; attention (pipelined) ==========
            with (
                tc.tile_pool(name="qx", bufs=1) as qx,
                tc.tile_pool(name="late", bufs=1) as late,
            ):
                # bf16 hi/lo x^T for the Q 3-term split; DMA'd into the SBUF
                # space phase 1 just freed, overlapping the AllReduce.
                xthi_sb = qx.tile([P, KC, TOK], BF16)
                xtlo_sb = qx.tile([P, KC, TOK], BF16)
                for k in range(KC):
                    nc.sync.dma_start(xthi_sb[:, k, :], r8(xthi)[:, k, :])
                    nc.sync.dma_start(xtlo_sb[:, k, :], r8(xtlo)[:, k, :])

                at_sb = late.tile([P, KC, TOK], BF16)
                woh_sb = late.tile([P, KC, D], BF16)

                with (
                    tc.tile_pool(name="qtp", bufs=4) as qtp,
                    tc.tile_pool(name="ph3ps", bufs=4, space="PSUM") as ph3ps,
                    tc.tile_pool(name="ph3sb", bufs=2) as ph3sb,
                    tc.tile_pool(name="ph4ps_a", bufs=3, space="PSUM") as ph4ps_a,
                    tc.tile_pool(name="ph4sb", bufs=3) as ph4sb,
                ):
                    qt_tiles = {}

                    # -- AllReduce of the packed KV/Ksum accumulator --
                    cc_in = dram.tile([P, 520], F32)
                    cc_out = dram.tile([P, 520], F32)
                    nc.sync.dma_start(cc_in[:], acc[:])
                    nc.gpsimd.collective_compute(
                        "AllReduce", OP.add,
                        replica_groups=[[0, 1], [2, 3], [4, 5], [6, 7]],
                        ins=[cc_in[:].opt()], outs=[cc_out[:].opt()])
                    ar_sb = smalls.tile([P, 520], F32)
                    nc.sync.dma_start(ar_sb[:], cc_out[:])
                    if DEBUG_TAPS:
                        nc.sync.dma_start(dbg_ar.ap(), ar_sb[:])
                    kv_bf = smalls.tile([P, 512], BF16)
                    nc.any.tensor_copy(kv_bf[:, :256], ar_sb[:, :256])
                    nc.any.tensor_copy(kv_bf[:, 256:], ar_sb[:, 260:516])
                    # kdp col hp: Ksum_{2hp}/2 on partitions 0-63 and
                    # Ksum_{2hp+1}/2 on 64-127 -- aligned with qt's partition
                    # layout. den halves come from two full-128 partition
                    # reduces (the gpsimd ucode only supports base partition
                    # 0): A = sum(qt*kdp) = (den_e+den_o)/2 over all 128,
                    # B = sum(qt*kdps) = (den_e-den_o)/2 with kdps = +-kdp,
                    # so den_e = A+B (rows 0-63) and den_o = A-B (rows 64+).
                    kdp = smalls.tile([P, KC], F32)
                    for h in range(H):
                        pr = (h % 2) * 64
                        c = h // 2
                        sc = 256 + c if c < 4 else 516 + (c - 4)
                        nc.vector.tensor_scalar(
                            kdp[pr:pr + 64, h // 2:h // 2 + 1],
                            ar_sb[pr:pr + 64, sc:sc + 1], 0.5, None, OP.mult)
                    sgn = smalls.tile([P, 1], F32)
                    nc.vector.memset(sgn[0:64, :], 1.0)
                    nc.vector.memset(sgn[64:128, :], -1.0)
                    kdps = smalls.tile([P, KC], F32)
                    nc.vector.tensor_scalar(kdps[:], kdp[:], sgn[:, 0:1], None, OP.mult)
                    # Block-diagonal KV stationary per head pair: one 128-wide
                    # matmul computes both heads' attention numerators.
                    kvq = smalls.tile([P, KC, P], BF16)
                    nc.any.memset(kvq[:], 0.0)
                    for hp in range(KC):
                        nc.any.tensor_copy(kvq[0:64, hp, 0:64],
                                           kv_bf[0:64, hp * 64:(hp + 1) * 64])
                        nc.any.tensor_copy(kvq[64:128, hp, 64:128],
                                           kv_bf[64:128, hp * 64:(hp + 1) * 64])

                    for k in range(KC):
                        nc.sync.dma_start(woh_sb[:, k, :], r8(woh)[:, k, :])

                    def q_proj(hp):
                        wq_t = wq_tiles.pop(hp)
                        qt = qtp.tile([P, TOK], F32, tag="qt", name=f"qt_{hp}")
                        qt_tiles[hp] = qt
                        # k-outer over all four tq psum tiles: each arriving
                        # x^T chunk immediately feeds 12 matmuls, so the
                        # hp=0 wave overlaps the xthi/xtlo DMA chunk-by-chunk.
                        qps_t = [ph3ps.tile([P, 512], F32, tag="qps",
                                            name=f"qps_{hp}_{tq}") for tq in range(TQ)]
                        for k in range(KC):
                            st, sp = (k == 0), (k == KC - 1)
                            for tq in range(TQ):
                                tsl = slice(tq * 512, (tq + 1) * 512)
                                nc.tensor.matmul(qps_t[tq][:], wq_t[:, k, 0, :],
                                                 xthi_sb[:, k, tsl], start=st, stop=False)
                                nc.tensor.matmul(qps_t[tq][:], wq_t[:, k, 1, :],
                                                 xthi_sb[:, k, tsl], start=False, stop=False)
                                nc.tensor.matmul(qps_t[tq][:], wq_t[:, k, 0, :],
                                                 xtlo_sb[:, k, tsl], start=False, stop=sp)
                        for tq in range(TQ):
                            tsl = slice(tq * 512, (tq + 1) * 512)
                            qps = qps_t[tq]
                            if apply_bias:
                                qraw = ph3sb.tile([P, 512], F32, tag="qraw",
                                                  name=f"qraw_{hp}_{tq}")
                                nc.vector.tensor_scalar(qraw[:], qps[:], bq_sb[:, hp:hp + 1],
                                                        None, OP.add)
                                qsrc = qraw
                            else:
                                qsrc = qps
                            qmin = ph3sb.tile([P, 512], F32, tag="qmin", name=f"qmin_{hp}_{tq}")
                            nc.vector.tensor_scalar(qmin[:], qsrc[:], 0.0, None, OP.min)
                            qexp = ph3sb.tile([P, 512], F32, tag="qexp", name=f"qexp_{hp}_{tq}")
                            nc.scalar.activation(qexp[:], qmin[:], AF.Exp)
                            qmax = ph3sb.tile([P, 512], F32, tag="qmax", name=f"qmax_{hp}_{tq}")
                            nc.vector.tensor_scalar(qmax[:], qsrc[:], 0.0, -1.0, OP.max, OP.add)
                            nc.vector.tensor_tensor(qt[:, tsl], qmax[:], qexp[:], OP.add)

                    def attention(hp):
                        qt = qt_tiles.pop(hp)
                        if DEBUG_TAPS and hp == 0:
                            nc.sync.dma_start(dbg_qt0.ap(), qt[:])
                        for tq in range(TQ):
                            tsl = slice(tq * 512, (tq + 1) * 512)
                            # den on gpsimd+DVE (keeps PE free); see kdp note.
                            prod = ph4sb.tile([P, 512], F32, tag="prod",
                                              name=f"prod_{hp}_{tq}")
                            nc.vector.tensor_scalar(prod[:], qt[:, tsl],
                                                    kdp[:, hp:hp + 1], None, OP.mult)
                            sprd = ph4sb.tile([P, 512], F32, tag="sprd",
                                              name=f"sprd_{hp}_{tq}")
                            nc.vector.tensor_scalar(sprd[:], qt[:, tsl],
                                                    kdps[:, hp:hp + 1], None, OP.mult)
                            denA = ph4sb.tile([P, 512], F32, tag="denA",
                                              name=f"denA_{hp}_{tq}")
                            denB = ph4sb.tile([P, 512], F32, tag="denB",
                                              name=f"denB_{hp}_{tq}")
                            nc.gpsimd.partition_all_reduce(
                                denA[:], prod[:], channels=128,
                                reduce_op=bass_isa.ReduceOp.add)
                            nc.gpsimd.partition_all_reduce(
                                denB[:], sprd[:], channels=128,
                                reduce_op=bass_isa.ReduceOp.add)
                            nc.vector.tensor_tensor(denA[0:64, :], denA[0:64, :],
                                                    denB[0:64, :], OP.add)
                            nc.vector.tensor_tensor(denA[64:128, :], denA[64:128, :],
                                                    denB[64:128, :], OP.subtract)
                            nc.vector.reciprocal(denA[:], denA[:])
                            qbf = ph4sb.tile([P, 512], BF16, tag="qbf", name=f"qbf_{hp}_{tq}")
                            nc.vector.tensor_copy(qbf[:], qt[:, tsl])
                            aps = ph4ps_a.tile([P, 512], F32, tag="aps", name=f"aps_{hp}_{tq}")
                            nc.tensor.matmul(aps[:], kvq[:, hp, :], qbf[:],
                                             start=True, stop=True)
                            nc.vector.tensor_tensor(at_sb[:, hp, tsl], aps[:], denA[:], OP.mult)

                    # depth-2 software pipeline: attention(hp) runs two Q chunks
                    # behind, so the AllReduce hides under ~3 Q projections.
                    # Software pipeline: depth 2 while the AllReduce is in
                    # flight, catching up to depth 1 at hp=4 so only
                    # attention(7)'s den chain trails into phase 5.
                    att_sched = {2: [0], 3: [1], 4: [2, 3], 5: [4], 6: [5], 7: [6]}
                    q_proj(0)
                    for hp in range(1, KC):
                        if hp + 1 < KC:
                            load_wq(hp + 1)
                        q_proj(hp)
                        for a in att_sched.get(hp, []):
                            attention(a)
                    attention(KC - 1)

                if DEBUG_TAPS:
                    with tc.tile_pool(name="dbgat", bufs=2) as dbgat:
                        for c in range(KC):
                            atf = dbgat.tile([P, TOK], F32, tag="atf", name=f"atf_{c}")
                            nc.vector.tensor_copy(atf[:], at_sb[:, c, :])
                            nc.sync.dma_start(dbg_at.ap()[:, c, :], atf[:])

                # ===== Phase 5: output projection + residual + LayerNorm =====
                with (
                    tc.tile_pool(name="ph5ps", bufs=4, space="PSUM") as ph5ps,
                    tc.tile_pool(name="ph5sb", bufs=4) as ph5sb,
                ):
                    for t in range(TC):
                        ts = slice(t * P, (t + 1) * P)
                        y = ph5sb.tile([P, D], F32, tag="y", name=f"y_{t}")
                        xr = ph5sb.tile([P, D], F32, tag="xr", name=f"xr_{t}")
                        nc.sync.dma_start(xr[:], xres.ap()[ts, :])
                        ops = ph5ps.tile([P, D], F32, tag="ops", name=f"ops_{t}")
                        for dh in range(2):
                            dsl = slice(dh * 512, (dh + 1) * 512)
                            for c in range(KC):
                                nc.tensor.matmul(ops[:, dsl], at_sb[:, c, ts], woh_sb[:, c, dsl],
                                                 start=(c == 0), stop=(c == KC - 1))
                        nc.vector.tensor_tensor(y[:], ops[:], xr[:], OP.add)
                        if apply_bias:
                            nc.vector.tensor_tensor(y[:], y[:], bo_b[:], OP.add)
                        stats = ph5sb.tile([P, 2, 6], F32, tag="stats", name=f"stats_{t}")
                        nc.vector.bn_stats(out=stats[:, 0, :], in_=y[:, :512])
                        nc.vector.bn_stats(out=stats[:, 1, :], in_=y[:, 512:])
                        mv = ph5sb.tile([P, 2], F32, tag="mv", name=f"mv_{t}")
                        nc.vector.bn_aggr(out=mv[:], in_=stats[:])
                        nc.scalar.activation(out=mv[:, 1:2], in_=mv[:, 1:2], func=AF.Sqrt,
                                             bias=eps_sb[:], scale=1.0)
                        nc.vector.reciprocal(mv[:, 1:2], mv[:, 1:2])
                        yo = ph5sb.tile([P, D], F32, tag="yo", name=f"yo_{t}")
                        nc.gpsimd.tensor_scalar(yo[:], y[:], mv[:, 0:1], mv[:, 1:2],
                                                OP.subtract, OP.mult)
                        if apply_gamma:
                            nc.vector.tensor_tensor(yo[:], yo[:], gamma_b[:], OP.mult)
                        if apply_beta:
                            nc.vector.tensor_tensor(yo[:], yo[:], beta_b[:], OP.add)
                        nc.sync.dma_start(out_d.ap()[ts, :], yo[:])

            wqp_cm.__exit__(None, None, None)

    nc.compile()
    return nc


def kernel(x, Wq, bq, Wk, bk, Wv, bv, Wo, bo, gamma, beta):
    global LAST_RESULT
    x = np.asarray(x, dtype=np.float32)
    f32 = np.float32
    bf16 = ml_dtypes.bfloat16

    apply_bias = any(np.any(np.asarray(b)) for b in (bq, bk, bv, bo))
    apply_gamma = not np.all(np.asarray(gamma) == 1.0)
    apply_beta = bool(np.any(np.asarray(beta)))

    nc = _build(apply_bias, apply_gamma, apply_beta)

    def split(W):
        W = np.asarray(W, dtype=f32)
        hi = W.astype(bf16)
        lo = (W - hi.astype(f32)).astype(bf16)
        return hi, lo

    wq_h, wq_l = split(Wq)
    wk_f = np.ascontiguousarray(np.asarray(Wk, dtype=f32))
    wv_f = np.ascontiguousarray(np.asarray(Wv, dtype=f32))
    wo_h, _ = split(Wo)
    e_sel = np.zeros((2, P), dtype=bf16)
    e_sel[0, :64] = 1
    e_sel[1, 64:] = 1

    in_maps = []
    for c in range(NCORES):
        b, half = c // 2, c % 2
        xs = x[b, half * TOK:(half + 1) * TOK]          # [2048, 1024]
        xhi = xs.astype(bf16)
        xlo = (xs - xhi.astype(f32)).astype(bf16)
        m = {
            "xtf": np.ascontiguousarray(xs.T),
            "xthi": np.ascontiguousarray(xhi.T),
            "xtlo": np.ascontiguousarray(xlo.T),
            "wqh": wq_h, "wql": wq_l,
            "wk": wk_f, "wv": wv_f, "woh": wo_h,
            "xres": np.ascontiguousarray(xs),
            "e_sel": e_sel,
        }
        if apply_bias:
            m.update(bq=np.asarray(bq, f32), bk=np.asarray(bk, f32),
                     bv=np.asarray(bv, f32), bo=np.asarray(bo, f32))
        if apply_gamma:
            m["gamma"] = np.asarray(gamma, f32)
        if apply_beta:
            m["beta"] = np.asarray(beta, f32)
        in_maps.append(m)

    import os
    try:
        LAST_RESULT = run_bass_kernel_spmd(nc, in_maps, core_ids=list(range(NCORES)))
    except ModuleNotFoundError:
        # no antenv.axon_hooks in this container -> NTFF tracing unavailable
        os.environ["BASS_NEVER_TRACE"] = "1"
        LAST_RESULT = run_bass_kernel_spmd(nc, in_maps, core_ids=list(range(NCORES)))
    out = np.empty((B, N, D), dtype=np.float32)
    for c in range(NCORES):
        b, half = c // 2, c % 2
        out[b, half * TOK:(half + 1) * TOK] = LAST_RESULT.results[c]["out"]
    return out


# revision 61
# speedup vs baseline: 1.0368x; 1.0078x over previous
"""Distributed Trainium2 kernel for the linear-attention transformer block.

Math (per batch element b):
  Q = elu(x @ Wq + bq), K = elu(x @ Wk + bk), V = x @ Wv + bv   (per-head d=64)
  KV_h = K_h^T V_h  [64,64];  Ksum_h = sum_n K_h[n]  [64]
  attn_h = (Q_h @ KV_h) / (Q_h . Ksum_h)
  out = LayerNorm(x + attn @ Wo + bo) * gamma + beta

Sharding: 16384 tokens over 8 cores (2048 each; core c owns batch c//2,
half c%2). Each core computes Q/K/V only for its tokens, partial KV/Ksum,
then a 266KB AllReduce over core pairs {2b, 2b+1} completes the KV stats;
attention + output projection + LayerNorm finish locally.

Precision: the 1/(Q.Ksum) denominators pass near zero (min |den| on the
nominal instance is ~0.35) and amplify projection noise into sign flips,
so the Q projection uses a 3-term bf16 split (x_hi@W_hi + x_hi@W_lo +
x_lo@W_hi; per-element err ~5e-6). The K/V projections run as single
float32r matmuls (full PE rate at N=512; measured HW err ~1.5e-4), which
keeps the Ksum-side denominator error ~0.07 rms - flip-safe with >5 sigma
margin. Ksum and the denominator run in f32. ~1.7e-3 global rel err.
"""

import sys

sys.path.insert(0, "/opt/trn_rl_repo")

import numpy as np
import ml_dtypes

import concourse.bass as bass
import concourse.bass_isa as bass_isa
import concourse.mybir as mybir
import concourse.tile as tile
from concourse import bacc
from concourse.bass_utils import run_bass_kernel_spmd

AF = mybir.ActivationFunctionType
OP = mybir.AluOpType
F32 = mybir.dt.float32
F32R = mybir.dt.float32r
BF16 = mybir.dt.bfloat16

B, N, D = 4, 4096, 1024
H, HD = 16, 64
TOK = 2048            # tokens per core
NCORES = 8
LN_EPS = 1e-3
P = 128
KC = D // P           # 8 contraction chunks
TC = TOK // P         # 16 token chunks of 128
TQ = TOK // 512       # 4 token chunks of 512
WARM_MM = 10          # PE clock-gate warmup matmuls

LAST_RESULT = None    # BassKernelResults of the most recent run (for test.py)
DEBUG_TAPS = False    # set True (e.g. from debug.py) to add intermediate outputs


def _build(apply_bias, apply_gamma, apply_beta):
    nc = bacc.Bacc("TRN2", target_bir_lowering=False, debug=False, num_devices=NCORES)

    def din(name, shape, dtype=BF16):
        return nc.dram_tensor(name, shape, dtype, kind="ExternalInput")

    xtf = din("xtf", [D, TOK], F32R)
    xthi = din("xthi", [D, TOK])
    xtlo = din("xtlo", [D, TOK])
    wqh = din("wqh", [D, D])
    wql = din("wql", [D, D])
    wk = din("wk", [D, D], F32R)
    wv = din("wv", [D, D], F32R)
    woh = din("woh", [D, D])
    xres = din("xres", [TOK, D], F32)
    e_sel = din("e_sel", [2, P])
    if apply_bias:
        bq_d = din("bq", [D], F32)
        bk_d = din("bk", [D], F32)
        bv_d = din("bv", [D], F32)
        bo_d = din("bo", [D], F32)
    if apply_gamma:
        gamma_d = din("gamma", [D], F32)
    if apply_beta:
        beta_d = din("beta", [D], F32)
    out_d = nc.dram_tensor("out", [TOK, D], F32, kind="ExternalOutput")
    if DEBUG_TAPS:
        dbg_k0 = nc.dram_tensor("dbg_k0", [P, 512], F32, kind="ExternalOutput")
        dbg_kv = nc.dram_tensor("dbg_kv", [P, 512], F32, kind="ExternalOutput")
        dbg_ksum = nc.dram_tensor("dbg_ksum", [P, KC], F32, kind="ExternalOutput")
        dbg_ar = nc.dram_tensor("dbg_ar", [P, 520], F32, kind="ExternalOutput")
        dbg_qt0 = nc.dram_tensor("dbg_qt0", [P, TOK], F32, kind="ExternalOutput")
        dbg_at = nc.dram_tensor("dbg_at", [P, KC, TOK], F32, kind="ExternalOutput")

    r8 = lambda t: t.ap().rearrange("(ko p) n -> p ko n", p=P)

    def bcast_row(dram_vec, sb_tile):
        # DMA-broadcast a [D] vector to [P, D] (stride-0 partition dim).
        src = bass.AP(
            tensor=dram_vec.ap().tensor,
            offset=dram_vec.ap().offset,
            ap=[[0, P]] + list(dram_vec.ap().ap),
        )
        nc.sync.dma_start(out=sb_tile, in_=src)

    with tile.TileContext(nc) as tc:
        with (
            tc.tile_pool(name="smalls", bufs=1) as smalls,
            tc.tile_pool(name="dram", bufs=1, space="DRAM") as dram,
        ):
            e_sb = smalls.tile([2, P], BF16)
            nc.sync.dma_start(e_sb[:], e_sel.ap())
            ones_sb = smalls.tile([P, 1], F32)
            nc.vector.memset(ones_sb[:], 1.0)
            eps_sb = smalls.tile([P, 1], F32)
            nc.vector.memset(eps_sb[:], LN_EPS)
            if apply_bias:
                bq_sb = smalls.tile([P, KC], F32)   # per-partition layout for Q^T
                nc.sync.dma_start(bq_sb[:], bq_d.ap().rearrange("(ko p) -> p ko", p=P))
                bk_b = smalls.tile([P, D], F32)
                bv_b = smalls.tile([P, D], F32)
                bo_b = smalls.tile([P, D], F32)
                bcast_row(bk_d, bk_b[:])
                bcast_row(bv_d, bv_b[:])
                bcast_row(bo_d, bo_b[:])
            if apply_gamma:
                gamma_b = smalls.tile([P, D], F32)
                bcast_row(gamma_d, gamma_b[:])
            if apply_beta:
                beta_b = smalls.tile([P, D], F32)
                bcast_row(beta_d, beta_b[:])

            if DEBUG_TAPS:
                kv_loc = smalls.tile([P, 512], F32)
                ksum_loc = smalls.tile([P, KC], F32)

            # PE warmup: matmuls on zeroed tiles release the HAM clock
            # gate (1.2->2.4 GHz) while the first input DMAs are in flight.
            with (
                tc.tile_pool(name="warmsb", bufs=1) as warmsb,
                tc.tile_pool(name="warmps", bufs=2, space="PSUM") as warmps,
            ):
                warm_a = warmsb.tile([P, P], BF16)
                warm_b = warmsb.tile([P, 512], BF16)
                nc.gpsimd.memset(warm_a[:], 0.0)
                nc.gpsimd.memset(warm_b[:], 0.0)
                for w in range(WARM_MM):
                    wp = warmps.tile([P, 512], F32, tag="warm", name=f"warm_{w}")
                    nc.tensor.matmul(wp[:], warm_a[:], warm_b[:], start=True, stop=True)

            # Prefetch the first two Q-weight slices; their DMAs have no deps
            # and fill otherwise-idle DMA time during phase 1.
            wqp_cm = tc.tile_pool(name="wqp", bufs=3)
            wqp = wqp_cm.__enter__()
            wq_tiles = {}

            def load_wq(hp):
                msl = slice(hp * P, (hp + 1) * P)
                wq_t = wqp.tile([P, KC, 2, P], BF16, tag="wq", name=f"wq_{hp}")
                nc.sync.dma_start(wq_t[:, :, 0, :], r8(wqh)[:, :, msl])
                nc.sync.dma_start(wq_t[:, :, 1, :], r8(wql)[:, :, msl])
                wq_tiles[hp] = wq_t

            # ================= Phase 1: K, V, partial KV + Ksum =================
            # K = x @ Wk and V = x @ Wv as single f32r matmuls (full PE rate
            # at N=512). x^T is resident in f32; the four psum accumulators
            # (K/V x dh halves) share each stationary x^T chunk.
            with (
                tc.tile_pool(name="ph1x", bufs=1) as ph1x,
                tc.tile_pool(name="wkv", bufs=1) as wkv,
                tc.tile_pool(name="kvps_pool", bufs=1, space="PSUM") as kvps_pool,
                tc.tile_pool(name="ph1ps", bufs=7, space="PSUM") as ph1ps,
                tc.tile_pool(name="ph1sb", bufs=4) as ph1sb,
            ):
                xtf_sb = ph1x.tile([P, KC, TOK], F32R)
                wk_sb = wkv.tile([P, KC, D], F32R)
                wv_sb = wkv.tile([P, KC, D], F32R)
                # DMA order: first t=0..1 x chunks + Wk (needed by t=0 K
                # matmuls), then Wv, then remaining x token-sliced t-major.
                for k in range(KC):
                    nc.sync.dma_start(wk_sb[:, k, :], r8(wk)[:, k, :])
                    nc.sync.dma_start(xtf_sb[:, k, 0:2 * P], r8(xtf)[:, k, 0:2 * P])
                for k in range(KC):
                    nc.sync.dma_start(wv_sb[:, k, :], r8(wv)[:, k, :])
                for t in range(2, TC):
                    ts = slice(t * P, (t + 1) * P)
                    for k in range(KC):
                        nc.sync.dma_start(xtf_sb[:, k, ts], r8(xtf)[:, k, ts])
                load_wq(0)
                load_wq(1)

                # SBUF accumulator (DVE-add per token chunk; interleaved
                # multi-chunk PSUM accumulation groups proved unreliable).
                # Layout per dh half: [dh*260, dh*260+256) = KV, [+256, +260) = Ksum.
                acc = smalls.tile([P, 520], F32)
                nc.vector.memset(acc[:], 0.0)

                for t in range(TC):
                    ts = slice(t * P, (t + 1) * P)
                    kps = {}
                    vps = {}
                    for dh in range(2):
                        kps[dh] = ph1ps.tile([P, 512], F32, tag="proj",
                                             name=f"kps_{t}_{dh}")
                        vps[dh] = ph1ps.tile([P, 512], F32, tag="proj",
                                             name=f"vps_{t}_{dh}")
                    for k in range(KC):
                        st, sp = (k == 0), (k == KC - 1)
                        for dh in range(2):
                            dsl = slice(dh * 512, (dh + 1) * 512)
                            nc.tensor.matmul(kps[dh][:], xtf_sb[:, k, ts],
                                             wk_sb[:, k, dsl], start=st, stop=sp)
                            nc.tensor.matmul(vps[dh][:], xtf_sb[:, k, ts],
                                             wv_sb[:, k, dsl], start=st, stop=sp)
                    kb_chunks = []
                    kvs_tiles = {}
                    for dh in range(2):
                        dsl = slice(dh * 512, (dh + 1) * 512)
                        if apply_bias:
                            kraw = ph1sb.tile([P, 512], F32, tag="kraw", name=f"kraw_{t}_{dh}")
                            nc.vector.tensor_tensor(kraw[:], kps[dh][:], bk_b[:, dsl], OP.add)
                            ksrc = kraw
                        else:
                            ksrc = kps[dh]
                        kmin = ph1sb.tile([P, 512], F32, tag="kmin", name=f"kmin_{t}_{dh}")
                        nc.vector.tensor_scalar(kmin[:], ksrc[:], 0.0, None, OP.min)
                        kexp = ph1sb.tile([P, 512], F32, tag="kexp", name=f"kexp_{t}_{dh}")
                        nc.scalar.activation(kexp[:], kmin[:], AF.Exp)
                        kmax = ph1sb.tile([P, 512], F32, tag="kmax", name=f"kmax_{t}_{dh}")
                        nc.vector.tensor_scalar(kmax[:], ksrc[:], 0.0, -1.0, OP.max, OP.add)
                        kf = ph1sb.tile([P, 512], F32, tag="kf", name=f"kf_{t}_{dh}")
                        nc.vector.tensor_tensor(kf[:], kmax[:], kexp[:], OP.add)
                        kb = ph1sb.tile([P, 512], BF16, tag="kb", name=f"kb_{t}_{dh}")
                        nc.vector.tensor_copy(kb[:], kf[:])
                        if DEBUG_TAPS and t == 0 and dh == 0:
                            nc.sync.dma_start(dbg_k0.ap(), kf[:])
                        kb_chunks.append(kb)
                        # Ksum column blocks (f32 matmul against ones) go into
                        # cols [256, 260) of the shared kvs_t psum tile.
                        kvs_t = kvps_pool.tile([P, 260], F32, tag="kvs_t",
                                               name=f"kvs_t_{t}_{dh}", bufs=1)
                        kvs_tiles[dh] = kvs_t
                        for j in range(4):
                            nc.tensor.matmul(
                                kvs_t[:, 256 + j:257 + j], kf[:, j * P:(j + 1) * P],
                                ones_sb[:], start=True, stop=True, skip_group_check=True)
                    for dh in range(2):
                        dsl = slice(dh * 512, (dh + 1) * 512)
                        vb = ph1sb.tile([P, 512], BF16, tag="vb", name=f"vb_{t}_{dh}")
                        if apply_bias:
                            nc.vector.tensor_tensor(vb[:], vps[dh][:], bv_b[:, dsl], OP.add)
                        else:
                            nc.any.tensor_copy(vb[:], vps[dh][:])
                        kb = kb_chunks[dh]
                        kvs_t = kvs_tiles[dh]
                        for hh in range(8):
                            h = dh * 8 + hh
                            pr = (h % 2) * 64
                            fc = (h // 2) * 64 - dh * 256
                            nc.tensor.matmul(
                                kvs_t[pr:pr + 64, fc:fc + 64],
                                kb[:, hh * 64:(hh + 1) * 64],
                                vb[:, hh * 64:(hh + 1) * 64],
                                start=True, stop=True,
                                tile_position=(0, pr), skip_group_check=True)
                        nc.vector.tensor_tensor(
                            acc[:, dh * 260:(dh + 1) * 260],
                            acc[:, dh * 260:(dh + 1) * 260], kvs_t[:], OP.add)

                if DEBUG_TAPS:
                    nc.vector.tensor_copy(kv_loc[:, :256], acc[:, :256])
                    nc.vector.tensor_copy(kv_loc[:, 256:], acc[:, 260:516])
                    nc.vector.tensor_copy(ksum_loc[:, :4], acc[:, 256:260])
                    nc.vector.tensor_copy(ksum_loc[:, 4:], acc[:, 516:520])
                    nc.sync.dma_start(dbg_kv.ap(), kv_loc[:])
                    nc.sync.dma_start(dbg_ksum.ap(), ksum_loc[:])

            # ========== Phases 2-4: AllReduce; Q^T; attention (pipelined) ==========
            with (
                tc.tile_pool(name="qx", bufs=1) as qx,
                tc.tile_pool(name="late", bufs=1) as late,
            ):
                # bf16 hi/lo x^T for the Q 3-term split; DMA'd into the SBUF
                # space phase 1 just freed, overlapping the AllReduce.
                xthi_sb = qx.tile([P, KC, TOK], BF16)
                xtlo_sb = qx.tile([P, KC, TOK], BF16)
                for k in range(KC):
                    nc.sync.dma_start(xthi_sb[:, k, :], r8(xthi)[:, k, :])
                    nc.sync.dma_start(xtlo_sb[:, k, :], r8(xtlo)[:, k, :])

                at_sb = late.tile([P, KC, TOK], BF16)
                woh_sb = late.tile([P, KC, D], BF16)

                with (
                    tc.tile_pool(name="qtp", bufs=4) as qtp,
                    tc.tile_pool(name="ph3ps", bufs=4, space="PSUM") as ph3ps,
                    tc.tile_pool(name="ph3sb", bufs=2) as ph3sb,
                    tc.tile_pool(name="ph4ps_a", bufs=2, space="PSUM") as ph4ps_a,
                    tc.tile_pool(name="ph4sb", bufs=3) as ph4sb,
                ):
                    qt_tiles = {}

                    # -- AllReduce of the packed KV/Ksum accumulator --
                    cc_in = dram.tile([P, 520], F32)
                    cc_out = dram.tile([P, 520], F32)
                    nc.sync.dma_start(cc_in[:], acc[:])
                    nc.gpsimd.collective_compute(
                        "AllReduce", OP.add,
                        replica_groups=[[0, 1], [2, 3], [4, 5], [6, 7]],
                        ins=[cc_in[:].opt()], outs=[cc_out[:].opt()])
                    ar_sb = smalls.tile([P, 520], F32)
                    nc.sync.dma_start(ar_sb[:], cc_out[:])
                    if DEBUG_TAPS:
                        nc.sync.dma_start(dbg_ar.ap(), ar_sb[:])
                    kv_bf = smalls.tile([P, 512], BF16)
                    nc.any.tensor_copy(kv_bf[:, :256], ar_sb[:, :256])
                    nc.any.tensor_copy(kv_bf[:, 256:], ar_sb[:, 260:516])
                    # kdp col hp: Ksum_{2hp}/2 on partitions 0-63 and
                    # Ksum_{2hp+1}/2 on 64-127 -- aligned with qt's partition
                    # layout. den halves come from two full-128 partition
                    # reduces (the gpsimd ucode only supports base partition
                    # 0): A = sum(qt*kdp) = (den_e+den_o)/2 over all 128,
                    # B = sum(qt*kdps) = (den_e-den_o)/2 with kdps = +-kdp,
                    # so den_e = A+B (rows 0-63) and den_o = A-B (rows 64+).
                    kdp = smalls.tile([P, KC], F32)
                    for h in range(H):
                        pr = (h % 2) * 64
                        c = h // 2
                        sc = 256 + c if c < 4 else 516 + (c - 4)
                        nc.vector.tensor_scalar(
                            kdp[pr:pr + 64, h // 2:h // 2 + 1],
                            ar_sb[pr:pr + 64, sc:sc + 1], 0.5, None, OP.mult)
                    sgn = smalls.tile([P, 1], F32)
                    nc.vector.memset(sgn[0:64, :], 1.0)
                    nc.vector.memset(sgn[64:128, :], -1.0)
                    kdps = smalls.tile([P, KC], F32)
                    nc.vector.tensor_scalar(kdps[:], kdp[:], sgn[:, 0:1], None, OP.mult)
                    # Block-diagonal KV stationary per head pair: one 128-wide
                    # matmul computes both heads' attention numerators.
                    # kd2 col h: Ksum_h (unhalved) at rows (h%2)*64, for the
                    # PE-matmul den path used by the last two head pairs.
                    kd2 = smalls.tile([P, H], F32)
                    nc.any.memset(kd2[:], 0.0)
                    for h in range(H):
                        pr = (h % 2) * 64
                        c = h // 2
                        sc = 256 + c if c < 4 else 516 + (c - 4)
                        nc.any.tensor_copy(kd2[pr:pr + 64, h:h + 1],
                                           ar_sb[pr:pr + 64, sc:sc + 1])
                    kvq = smalls.tile([P, KC, P], BF16)
                    nc.any.memset(kvq[:], 0.0)
                    for hp in range(KC):
                        nc.any.tensor_copy(kvq[0:64, hp, 0:64],
                                           kv_bf[0:64, hp * 64:(hp + 1) * 64])
                        nc.any.tensor_copy(kvq[64:128, hp, 64:128],
                                           kv_bf[64:128, hp * 64:(hp + 1) * 64])

                    for k in range(KC):
                        nc.sync.dma_start(woh_sb[:, k, :], r8(woh)[:, k, :])

                    def q_proj(hp):
                        wq_t = wq_tiles.pop(hp)
                        qt = qtp.tile([P, TOK], F32, tag="qt", name=f"qt_{hp}")
                        qt_tiles[hp] = qt
                        # k-outer over all four tq psum tiles: each arriving
                        # x^T chunk immediately feeds 12 matmuls, so the
                        # hp=0 wave overlaps the xthi/xtlo DMA chunk-by-chunk.
                        qps_t = [ph3ps.tile([P, 512], F32, tag="qps",
                                            name=f"qps_{hp}_{tq}") for tq in range(TQ)]
                        for k in range(KC):
                            st, sp = (k == 0), (k == KC - 1)
                            for tq in range(TQ):
                                tsl = slice(tq * 512, (tq + 1) * 512)
                                nc.tensor.matmul(qps_t[tq][:], wq_t[:, k, 0, :],
                                                 xthi_sb[:, k, tsl], start=st, stop=False)
                                nc.tensor.matmul(qps_t[tq][:], wq_t[:, k, 1, :],
                                                 xthi_sb[:, k, tsl], start=False, stop=False)
                                nc.tensor.matmul(qps_t[tq][:], wq_t[:, k, 0, :],
                                                 xtlo_sb[:, k, tsl], start=False, stop=sp)
                        for tq in range(TQ):
                            tsl = slice(tq * 512, (tq + 1) * 512)
                            qps = qps_t[tq]
                            if apply_bias:
                                qraw = ph3sb.tile([P, 512], F32, tag="qraw",
                                                  name=f"qraw_{hp}_{tq}")
                                nc.vector.tensor_scalar(qraw[:], qps[:], bq_sb[:, hp:hp + 1],
                                                        None, OP.add)
                                qsrc = qraw
                            else:
                                qsrc = qps
                            qmin = ph3sb.tile([P, 512], F32, tag="qmin", name=f"qmin_{hp}_{tq}")
                            nc.vector.tensor_scalar(qmin[:], qsrc[:], 0.0, None, OP.min)
                            qexp = ph3sb.tile([P, 512], F32, tag="qexp", name=f"qexp_{hp}_{tq}")
                            nc.scalar.activation(qexp[:], qmin[:], AF.Exp)
                            qmax = ph3sb.tile([P, 512], F32, tag="qmax", name=f"qmax_{hp}_{tq}")
                            nc.vector.tensor_scalar(qmax[:], qsrc[:], 0.0, -1.0, OP.max, OP.add)
                            nc.vector.tensor_tensor(qt[:, tsl], qmax[:], qexp[:], OP.add)

                    def attention(hp, pe_den=False):
                        qt = qt_tiles.pop(hp)
                        if DEBUG_TAPS and hp == 0:
                            nc.sync.dma_start(dbg_qt0.ap(), qt[:])
                        for tq in range(TQ):
                            tsl = slice(tq * 512, (tq + 1) * 512)
                            if pe_den:
                                # Tail head-pairs: the PE is idling here and
                                # the den gates phase 5, so compute it with
                                # the short f32-matmul + broadcast chain.
                                dps = ph3ps.tile([2, 512], F32, tag="dps",
                                                 name=f"dps_{hp}_{tq}", bufs=1)
                                nc.tensor.matmul(dps[:], kd2[:, 2 * hp:2 * hp + 2],
                                                 qt[:, tsl], start=True, stop=True)
                                zrf = ph4sb.tile([2, 512], F32, tag="sprd",
                                                 name=f"zrf_{hp}_{tq}")
                                nc.vector.reciprocal(zrf[:], dps[:])
                                zr = ph4sb.tile([2, 512], BF16, tag="qbf",
                                                name=f"zr_{hp}_{tq}")
                                nc.vector.tensor_copy(zr[:], zrf[:])
                                zps = ph3ps.tile([P, 512], F32, tag="zps",
                                                 name=f"zps_{hp}_{tq}", bufs=1)
                                nc.tensor.matmul(zps[:], e_sb[:], zr[:],
                                                 start=True, stop=True)
                                denA = ph4sb.tile([P, 512], F32, tag="denA",
                                                  name=f"denA_{hp}_{tq}")
                                nc.vector.tensor_copy(denA[:], zps[:])
                                qbf = ph4sb.tile([P, 512], BF16, tag="qbf",
                                                 name=f"qbf_{hp}_{tq}")
                                nc.vector.tensor_copy(qbf[:], qt[:, tsl])
                                aps = ph4ps_a.tile([P, 512], F32, tag="aps",
                                                   name=f"aps_{hp}_{tq}")
                                nc.tensor.matmul(aps[:], kvq[:, hp, :], qbf[:],
                                                 start=True, stop=True)
                                nc.vector.tensor_tensor(at_sb[:, hp, tsl], aps[:],
                                                        denA[:], OP.mult)
                                continue
                            # den on gpsimd+DVE (keeps PE free); see kdp note.
                            prod = ph4sb.tile([P, 512], F32, tag="prod",
                                              name=f"prod_{hp}_{tq}")
                            nc.vector.tensor_scalar(prod[:], qt[:, tsl],
                                                    kdp[:, hp:hp + 1], None, OP.mult)
                            sprd = ph4sb.tile([P, 512], F32, tag="sprd",
                                              name=f"sprd_{hp}_{tq}")
                            nc.vector.tensor_scalar(sprd[:], qt[:, tsl],
                                                    kdps[:, hp:hp + 1], None, OP.mult)
                            denA = ph4sb.tile([P, 512], F32, tag="denA",
                                              name=f"denA_{hp}_{tq}")
                            denB = ph4sb.tile([P, 512], F32, tag="denB",
                                              name=f"denB_{hp}_{tq}")
                            nc.gpsimd.partition_all_reduce(
                                denA[:], prod[:], channels=128,
                                reduce_op=bass_isa.ReduceOp.add)
                            nc.gpsimd.partition_all_reduce(
                                denB[:], sprd[:], channels=128,
                                reduce_op=bass_isa.ReduceOp.add)
                            nc.vector.tensor_tensor(denA[0:64, :], denA[0:64, :],
                                                    denB[0:64, :], OP.add)
                            nc.vector.tensor_tensor(denA[64:128, :], denA[64:128, :],
                                                    denB[64:128, :], OP.subtract)
                            nc.vector.reciprocal(denA[:], denA[:])
                            qbf = ph4sb.tile([P, 512], BF16, tag="qbf", name=f"qbf_{hp}_{tq}")
                            nc.vector.tensor_copy(qbf[:], qt[:, tsl])
                            aps = ph4ps_a.tile([P, 512], F32, tag="aps", name=f"aps_{hp}_{tq}")
                            nc.tensor.matmul(aps[:], kvq[:, hp, :], qbf[:],
                                             start=True, stop=True)
                            nc.vector.tensor_tensor(at_sb[:, hp, tsl], aps[:], denA[:], OP.mult)

                    # depth-2 software pipeline: attention(hp) runs two Q chunks
                    # behind, so the AllReduce hides under ~3 Q projections.
                    # Software pipeline: depth 2 while the AllReduce is in
                    # flight, catching up to depth 1 at hp=4 so only
                    # attention(7)'s den chain trails into phase 5.
                    att_sched = {2: [0], 3: [1], 4: [2, 3], 5: [4], 6: [5], 7: [6]}
                    q_proj(0)
                    for hp in range(1, KC):
                        if hp + 1 < KC:
                            load_wq(hp + 1)
                        q_proj(hp)
                        for a in att_sched.get(hp, []):
                            attention(a, pe_den=(a >= KC - 2))
                    attention(KC - 1, pe_den=True)

                if DEBUG_TAPS:
                    with tc.tile_pool(name="dbgat", bufs=2) as dbgat:
                        for c in range(KC):
                            atf = dbgat.tile([P, TOK], F32, tag="atf", name=f"atf_{c}")
                            nc.vector.tensor_copy(atf[:], at_sb[:, c, :])
                            nc.sync.dma_start(dbg_at.ap()[:, c, :], atf[:])

                # ===== Phase 5: output projection + residual + LayerNorm =====
                with (
                    tc.tile_pool(name="ph5ps", bufs=4, space="PSUM") as ph5ps,
                    tc.tile_pool(name="ph5sb", bufs=4) as ph5sb,
                ):
                    for t in range(TC):
                        ts = slice(t * P, (t + 1) * P)
                        y = ph5sb.tile([P, D], F32, tag="y", name=f"y_{t}")
                        xr = ph5sb.tile([P, D], F32, tag="xr", name=f"xr_{t}")
                        nc.sync.dma_start(xr[:], xres.ap()[ts, :])
                        ops = ph5ps.tile([P, D], F32, tag="ops", name=f"ops_{t}")
                        for dh in range(2):
                            dsl = slice(dh * 512, (dh + 1) * 512)
                            for c in range(KC):
                                nc.tensor.matmul(ops[:, dsl], at_sb[:, c, ts], woh_sb[:, c, dsl],
                                                 start=(c == 0), stop=(c == KC - 1))
                        nc.vector.tensor_tensor(y[:], ops[:], xr[:], OP.add)
                        if apply_bias:
                            nc.vector.tensor_tensor(y[:], y[:], bo_b[:], OP.add)
                        stats = ph5sb.tile([P, 2, 6], F32, tag="stats", name=f"stats_{t}")
                        nc.vector.bn_stats(out=stats[:, 0, :], in_=y[:, :512])
                        nc.vector.bn_stats(out=stats[:, 1, :], in_=y[:, 512:])
                        mv = ph5sb.tile([P, 2], F32, tag="mv", name=f"mv_{t}")
                        nc.vector.bn_aggr(out=mv[:], in_=stats[:])
                        nc.scalar.activation(out=mv[:, 1:2], in_=mv[:, 1:2], func=AF.Sqrt,
                                             bias=eps_sb[:], scale=1.0)
                        nc.vector.reciprocal(mv[:, 1:2], mv[:, 1:2])
                        yo = ph5sb.tile([P, D], F32, tag="yo", name=f"yo_{t}")
                        nc.gpsimd.tensor_scalar(yo[:], y[:], mv[:, 0:1], mv[:, 1:2],
                                                OP.subtract, OP.mult)
                        if apply_gamma:
                            nc.vector.tensor_tensor(yo[:], yo[:], gamma_b[:], OP.mult)
                        if apply_beta:
                            nc.vector.tensor_tensor(yo[:], yo[:], beta_b[:], OP.add)
                        nc.sync.dma_start(out_d.ap()[ts, :], yo[:])

            wqp_cm.__exit__(None, None, None)

    nc.compile()
    return nc


def kernel(x, Wq, bq, Wk, bk, Wv, bv, Wo, bo, gamma, beta):
    global LAST_RESULT
    x = np.asarray(x, dtype=np.float32)
    f32 = np.float32
    bf16 = ml_dtypes.bfloat16

    apply_bias = any(np.any(np.asarray(b)) for b in (bq, bk, bv, bo))
    apply_gamma = not np.all(np.asarray(gamma) == 1.0)
    apply_beta = bool(np.any(np.asarray(beta)))

    nc = _build(apply_bias, apply_gamma, apply_beta)

    def split(W):
        W = np.asarray(W, dtype=f32)
        hi = W.astype(bf16)
        lo = (W - hi.astype(f32)).astype(bf16)
        return hi, lo

    wq_h, wq_l = split(Wq)
    wk_f = np.ascontiguousarray(np.asarray(Wk, dtype=f32))
    wv_f = np.ascontiguousarray(np.asarray(Wv, dtype=f32))
    wo_h, _ = split(Wo)
    e_sel = np.zeros((2, P), dtype=bf16)
    e_sel[0, :64] = 1
    e_sel[1, 64:] = 1

    in_maps = []
    for c in range(NCORES):
        b, half = c // 2, c % 2
        xs = x[b, half * TOK:(half + 1) * TOK]          # [2048, 1024]
        xhi = xs.astype(bf16)
        xlo = (xs - xhi.astype(f32)).astype(bf16)
        m = {
            "xtf": np.ascontiguousarray(xs.T),
            "xthi": np.ascontiguousarray(xhi.T),
            "xtlo": np.ascontiguousarray(xlo.T),
            "wqh": wq_h, "wql": wq_l,
            "wk": wk_f, "wv": wv_f, "woh": wo_h,
            "xres": np.ascontiguousarray(xs),
            "e_sel": e_sel,
        }
        if apply_bias:
            m.update(bq=np.asarray(bq, f32), bk=np.asarray(bk, f32),
                     bv=np.asarray(bv, f32), bo=np.asarray(bo, f32))
        if apply_gamma:
            m["gamma"] = np.asarray(gamma, f32)
        if apply_beta:
            m["beta"] = np.asarray(beta, f32)
        in_maps.append(m)

    import os
    try:
        LAST_RESULT = run_bass_kernel_spmd(nc, in_maps, core_ids=list(range(NCORES)))
    except ModuleNotFoundError:
        # no antenv.axon_hooks in this container -> NTFF tracing unavailable
        os.environ["BASS_NEVER_TRACE"] = "1"
        LAST_RESULT = run_bass_kernel_spmd(nc, in_maps, core_ids=list(range(NCORES)))
    out = np.empty((B, N, D), dtype=np.float32)
    for c in range(NCORES):
        b, half = c // 2, c % 2
        out[b, half * TOK:(half + 1) * TOK] = LAST_RESULT.results[c]["out"]
    return out


# revision 62
# speedup vs baseline: 1.0586x; 1.0210x over previous
"""Distributed Trainium2 kernel for the linear-attention transformer block.

Math (per batch element b):
  Q = elu(x @ Wq + bq), K = elu(x @ Wk + bk), V = x @ Wv + bv   (per-head d=64)
  KV_h = K_h^T V_h  [64,64];  Ksum_h = sum_n K_h[n]  [64]
  attn_h = (Q_h @ KV_h) / (Q_h . Ksum_h)
  out = LayerNorm(x + attn @ Wo + bo) * gamma + beta

Sharding: 16384 tokens over 8 cores (2048 each; core c owns batch c//2,
half c%2). Each core computes Q/K/V only for its tokens, partial KV/Ksum,
then a 266KB AllReduce over core pairs {2b, 2b+1} completes the KV stats;
attention + output projection + LayerNorm finish locally.

Precision: the 1/(Q.Ksum) denominators pass near zero (min |den| on the
nominal instance is ~0.35) and amplify projection noise into sign flips,
so the Q projection uses a 3-term bf16 split (x_hi@W_hi + x_hi@W_lo +
x_lo@W_hi; per-element err ~5e-6). The K/V projections run as single
float32r matmuls (full PE rate at N=512; measured HW err ~1.5e-4), which
keeps the Ksum-side denominator error ~0.07 rms - flip-safe with >5 sigma
margin. Ksum and the denominator run in f32. ~1.7e-3 global rel err.
"""

import sys

sys.path.insert(0, "/opt/trn_rl_repo")

import numpy as np
import ml_dtypes

import concourse.bass as bass
import concourse.bass_isa as bass_isa
import concourse.mybir as mybir
import concourse.tile as tile
from concourse import bacc
from concourse.bass_utils import run_bass_kernel_spmd

AF = mybir.ActivationFunctionType
OP = mybir.AluOpType
F32 = mybir.dt.float32
F32R = mybir.dt.float32r
BF16 = mybir.dt.bfloat16

B, N, D = 4, 4096, 1024
H, HD = 16, 64
TOK = 2048            # tokens per core
NCORES = 8
LN_EPS = 1e-3
P = 128
KC = D // P           # 8 contraction chunks
TC = TOK // P         # 16 token chunks of 128
TQ = TOK // 512       # 4 token chunks of 512
WARM_MM = 10          # PE clock-gate warmup matmuls

LAST_RESULT = None    # BassKernelResults of the most recent run (for test.py)
DEBUG_TAPS = False    # set True (e.g. from debug.py) to add intermediate outputs


def _build(apply_bias, apply_gamma, apply_beta):
    nc = bacc.Bacc("TRN2", target_bir_lowering=False, debug=False, num_devices=NCORES)

    def din(name, shape, dtype=BF16):
        return nc.dram_tensor(name, shape, dtype, kind="ExternalInput")

    xtf = din("xtf", [D, TOK], F32R)
    xthi = din("xthi", [D, TOK])
    xtlo = din("xtlo", [D, TOK])
    wqh = din("wqh", [D, D])
    wql = din("wql", [D, D])
    wk = din("wk", [D, D], F32R)
    wv = din("wv", [D, D], F32R)
    woh = din("woh", [D, D])
    xres = din("xres", [TOK, D], F32)
    e_sel = din("e_sel", [2, P])
    if apply_bias:
        bq_d = din("bq", [D], F32)
        bk_d = din("bk", [D], F32)
        bv_d = din("bv", [D], F32)
        bo_d = din("bo", [D], F32)
    if apply_gamma:
        gamma_d = din("gamma", [D], F32)
    if apply_beta:
        beta_d = din("beta", [D], F32)
    out_d = nc.dram_tensor("out", [TOK, D], F32, kind="ExternalOutput")
    if DEBUG_TAPS:
        dbg_k0 = nc.dram_tensor("dbg_k0", [P, 512], F32, kind="ExternalOutput")
        dbg_kv = nc.dram_tensor("dbg_kv", [P, 512], F32, kind="ExternalOutput")
        dbg_ksum = nc.dram_tensor("dbg_ksum", [P, KC], F32, kind="ExternalOutput")
        dbg_ar = nc.dram_tensor("dbg_ar", [P, 520], F32, kind="ExternalOutput")
        dbg_qt0 = nc.dram_tensor("dbg_qt0", [P, TOK], F32, kind="ExternalOutput")
        dbg_at = nc.dram_tensor("dbg_at", [P, KC, TOK], F32, kind="ExternalOutput")

    r8 = lambda t: t.ap().rearrange("(ko p) n -> p ko n", p=P)

    def bcast_row(dram_vec, sb_tile):
        # DMA-broadcast a [D] vector to [P, D] (stride-0 partition dim).
        src = bass.AP(
            tensor=dram_vec.ap().tensor,
            offset=dram_vec.ap().offset,
            ap=[[0, P]] + list(dram_vec.ap().ap),
        )
        nc.sync.dma_start(out=sb_tile, in_=src)

    with tile.TileContext(nc) as tc:
        with (
            tc.tile_pool(name="smalls", bufs=1) as smalls,
            tc.tile_pool(name="dram", bufs=1, space="DRAM") as dram,
        ):
            e_sb = smalls.tile([2, P], BF16)
            nc.sync.dma_start(e_sb[:], e_sel.ap())
            ones_sb = smalls.tile([P, 1], F32)
            nc.vector.memset(ones_sb[:], 1.0)
            eps_sb = smalls.tile([P, 1], F32)
            nc.vector.memset(eps_sb[:], LN_EPS)
            if apply_bias:
                bq_sb = smalls.tile([P, KC], F32)   # per-partition layout for Q^T
                nc.sync.dma_start(bq_sb[:], bq_d.ap().rearrange("(ko p) -> p ko", p=P))
                bk_b = smalls.tile([P, D], F32)
                bv_b = smalls.tile([P, D], F32)
                bo_b = smalls.tile([P, D], F32)
                bcast_row(bk_d, bk_b[:])
                bcast_row(bv_d, bv_b[:])
                bcast_row(bo_d, bo_b[:])
            if apply_gamma:
                gamma_b = smalls.tile([P, D], F32)
                bcast_row(gamma_d, gamma_b[:])
            if apply_beta:
                beta_b = smalls.tile([P, D], F32)
                bcast_row(beta_d, beta_b[:])

            if DEBUG_TAPS:
                kv_loc = smalls.tile([P, 512], F32)
                ksum_loc = smalls.tile([P, KC], F32)

            # PE warmup: matmuls on zeroed tiles release the HAM clock
            # gate (1.2->2.4 GHz) while the first input DMAs are in flight.
            with (
                tc.tile_pool(name="warmsb", bufs=1) as warmsb,
                tc.tile_pool(name="warmps", bufs=2, space="PSUM") as warmps,
            ):
                warm_a = warmsb.tile([P, P], BF16)
                warm_b = warmsb.tile([P, 512], BF16)
                nc.gpsimd.memset(warm_a[:], 0.0)
                nc.gpsimd.memset(warm_b[:], 0.0)
                for w in range(WARM_MM):
                    wp = warmps.tile([P, 512], F32, tag="warm", name=f"warm_{w}")
                    nc.tensor.matmul(wp[:], warm_a[:], warm_b[:], start=True, stop=True)

            # Prefetch the first two Q-weight slices; their DMAs have no deps
            # and fill otherwise-idle DMA time during phase 1.
            wqp_cm = tc.tile_pool(name="wqp", bufs=3)
            wqp = wqp_cm.__enter__()
            wq_tiles = {}

            def load_wq(hp):
                msl = slice(hp * P, (hp + 1) * P)
                wq_t = wqp.tile([P, KC, 2, P], BF16, tag="wq", name=f"wq_{hp}")
                nc.sync.dma_start(wq_t[:, :, 0, :], r8(wqh)[:, :, msl])
                nc.sync.dma_start(wq_t[:, :, 1, :], r8(wql)[:, :, msl])
                wq_tiles[hp] = wq_t

            # ================= Phase 1: K, V, partial KV + Ksum =================
            # K = x @ Wk and V = x @ Wv as single f32r matmuls (full PE rate
            # at N=512). x^T is resident in f32; the four psum accumulators
            # (K/V x dh halves) share each stationary x^T chunk.
            with (
                tc.tile_pool(name="ph1x", bufs=1) as ph1x,
                tc.tile_pool(name="wkv", bufs=1) as wkv,
                tc.tile_pool(name="kvps_pool", bufs=1, space="PSUM") as kvps_pool,
                tc.tile_pool(name="ph1ps", bufs=7, space="PSUM") as ph1ps,
                tc.tile_pool(name="ph1sb", bufs=4) as ph1sb,
            ):
                xtf_sb = ph1x.tile([P, KC, TOK], F32R)
                wk_sb = wkv.tile([P, KC, D], F32R)
                wv_sb = wkv.tile([P, KC, D], F32R)
                # DMA order: first t=0..1 x chunks + Wk (needed by t=0 K
                # matmuls), then Wv, then remaining x token-sliced t-major.
                for k in range(KC):
                    nc.sync.dma_start(wk_sb[:, k, :], r8(wk)[:, k, :])
                    nc.sync.dma_start(xtf_sb[:, k, 0:2 * P], r8(xtf)[:, k, 0:2 * P])
                for k in range(KC):
                    nc.sync.dma_start(wv_sb[:, k, :], r8(wv)[:, k, :])
                for t in range(2, TC):
                    ts = slice(t * P, (t + 1) * P)
                    for k in range(KC):
                        nc.sync.dma_start(xtf_sb[:, k, ts], r8(xtf)[:, k, ts])
                load_wq(0)
                load_wq(1)

                # SBUF accumulator (DVE-add per token chunk; interleaved
                # multi-chunk PSUM accumulation groups proved unreliable).
                # Layout per dh half: [dh*260, dh*260+256) = KV, [+256, +260) = Ksum.
                acc = smalls.tile([P, 520], F32)
                nc.vector.memset(acc[:], 0.0)

                for t in range(TC):
                    ts = slice(t * P, (t + 1) * P)
                    kps = {}
                    vps = {}
                    for dh in range(2):
                        kps[dh] = ph1ps.tile([P, 512], F32, tag="proj",
                                             name=f"kps_{t}_{dh}")
                        vps[dh] = ph1ps.tile([P, 512], F32, tag="proj",
                                             name=f"vps_{t}_{dh}")
                    for k in range(KC):
                        st, sp = (k == 0), (k == KC - 1)
                        for dh in range(2):
                            dsl = slice(dh * 512, (dh + 1) * 512)
                            nc.tensor.matmul(kps[dh][:], xtf_sb[:, k, ts],
                                             wk_sb[:, k, dsl], start=st, stop=sp)
                            nc.tensor.matmul(vps[dh][:], xtf_sb[:, k, ts],
                                             wv_sb[:, k, dsl], start=st, stop=sp)
                    kb_chunks = []
                    kvs_tiles = {}
                    for dh in range(2):
                        dsl = slice(dh * 512, (dh + 1) * 512)
                        if apply_bias:
                            kraw = ph1sb.tile([P, 512], F32, tag="kraw", name=f"kraw_{t}_{dh}")
                            nc.vector.tensor_tensor(kraw[:], kps[dh][:], bk_b[:, dsl], OP.add)
                            ksrc = kraw
                        else:
                            ksrc = kps[dh]
                        kmin = ph1sb.tile([P, 512], F32, tag="kmin", name=f"kmin_{t}_{dh}")
                        nc.vector.tensor_scalar(kmin[:], ksrc[:], 0.0, None, OP.min)
                        kexp = ph1sb.tile([P, 512], F32, tag="kexp", name=f"kexp_{t}_{dh}")
                        nc.scalar.activation(kexp[:], kmin[:], AF.Exp)
                        kmax = ph1sb.tile([P, 512], F32, tag="kmax", name=f"kmax_{t}_{dh}")
                        nc.vector.tensor_scalar(kmax[:], ksrc[:], 0.0, -1.0, OP.max, OP.add)
                        kf = ph1sb.tile([P, 512], F32, tag="kf", name=f"kf_{t}_{dh}")
                        nc.vector.tensor_tensor(kf[:], kmax[:], kexp[:], OP.add)
                        kb = ph1sb.tile([P, 512], BF16, tag="kb", name=f"kb_{t}_{dh}")
                        nc.vector.tensor_copy(kb[:], kf[:])
                        if DEBUG_TAPS and t == 0 and dh == 0:
                            nc.sync.dma_start(dbg_k0.ap(), kf[:])
                        kb_chunks.append(kb)
                        # Ksum column blocks (f32 matmul against ones) go into
                        # cols [256, 260) of the shared kvs_t psum tile.
                        kvs_t = kvps_pool.tile([P, 260], F32, tag="kvs_t",
                                               name=f"kvs_t_{t}_{dh}", bufs=1)
                        kvs_tiles[dh] = kvs_t
                        for j in range(4):
                            nc.tensor.matmul(
                                kvs_t[:, 256 + j:257 + j], kf[:, j * P:(j + 1) * P],
                                ones_sb[:], start=True, stop=True, skip_group_check=True)
                    for dh in range(2):
                        dsl = slice(dh * 512, (dh + 1) * 512)
                        vb = ph1sb.tile([P, 512], BF16, tag="vb", name=f"vb_{t}_{dh}")
                        if apply_bias:
                            nc.vector.tensor_tensor(vb[:], vps[dh][:], bv_b[:, dsl], OP.add)
                        else:
                            nc.any.tensor_copy(vb[:], vps[dh][:])
                        kb = kb_chunks[dh]
                        kvs_t = kvs_tiles[dh]
                        for hh in range(8):
                            h = dh * 8 + hh
                            pr = (h % 2) * 64
                            fc = (h // 2) * 64 - dh * 256
                            nc.tensor.matmul(
                                kvs_t[pr:pr + 64, fc:fc + 64],
                                kb[:, hh * 64:(hh + 1) * 64],
                                vb[:, hh * 64:(hh + 1) * 64],
                                start=True, stop=True,
                                tile_position=(0, pr), skip_group_check=True)
                        nc.vector.tensor_tensor(
                            acc[:, dh * 260:(dh + 1) * 260],
                            acc[:, dh * 260:(dh + 1) * 260], kvs_t[:], OP.add)

                if DEBUG_TAPS:
                    nc.vector.tensor_copy(kv_loc[:, :256], acc[:, :256])
                    nc.vector.tensor_copy(kv_loc[:, 256:], acc[:, 260:516])
                    nc.vector.tensor_copy(ksum_loc[:, :4], acc[:, 256:260])
                    nc.vector.tensor_copy(ksum_loc[:, 4:], acc[:, 516:520])
                    nc.sync.dma_start(dbg_kv.ap(), kv_loc[:])
                    nc.sync.dma_start(dbg_ksum.ap(), ksum_loc[:])

            # ========== Phases 2-4: AllReduce; Q^T; attention (pipelined) ==========
            with (
                tc.tile_pool(name="qx", bufs=1) as qx,
                tc.tile_pool(name="late", bufs=1) as late,
            ):
                # bf16 hi/lo x^T for the Q 3-term split; DMA'd into the SBUF
                # space phase 1 just freed, overlapping the AllReduce.
                xthi_sb = qx.tile([P, KC, TOK], BF16)
                xtlo_sb = qx.tile([P, KC, TOK], BF16)
                for k in range(KC):
                    nc.sync.dma_start(xthi_sb[:, k, :], r8(xthi)[:, k, :])
                    nc.sync.dma_start(xtlo_sb[:, k, :], r8(xtlo)[:, k, :])

                at_sb = late.tile([P, KC, TOK], BF16)
                woh_sb = late.tile([P, KC, D], BF16)

                with (
                    tc.tile_pool(name="qtp", bufs=4) as qtp,
                    tc.tile_pool(name="ph3ps", bufs=4, space="PSUM") as ph3ps,
                    tc.tile_pool(name="ph3sb", bufs=2) as ph3sb,
                    tc.tile_pool(name="ph4ps_a", bufs=2, space="PSUM") as ph4ps_a,
                    tc.tile_pool(name="ph4sb", bufs=3) as ph4sb,
                ):
                    qt_tiles = {}

                    # -- AllReduce of the packed KV/Ksum accumulator --
                    cc_in = dram.tile([P, 520], F32)
                    cc_out = dram.tile([P, 520], F32)
                    nc.sync.dma_start(cc_in[:], acc[:])
                    nc.gpsimd.collective_compute(
                        "AllReduce", OP.add,
                        replica_groups=[[0, 1], [2, 3], [4, 5], [6, 7]],
                        ins=[cc_in[:].opt()], outs=[cc_out[:].opt()])
                    ar_sb = smalls.tile([P, 520], F32)
                    nc.sync.dma_start(ar_sb[:], cc_out[:])
                    if DEBUG_TAPS:
                        nc.sync.dma_start(dbg_ar.ap(), ar_sb[:])
                    kv_bf = smalls.tile([P, 512], BF16)
                    nc.any.tensor_copy(kv_bf[:, :256], ar_sb[:, :256])
                    nc.any.tensor_copy(kv_bf[:, 256:], ar_sb[:, 260:516])
                    # kdp col hp: Ksum_{2hp}/2 on partitions 0-63 and
                    # Ksum_{2hp+1}/2 on 64-127 -- aligned with qt's partition
                    # layout. den halves come from two full-128 partition
                    # reduces (the gpsimd ucode only supports base partition
                    # 0): A = sum(qt*kdp) = (den_e+den_o)/2 over all 128,
                    # B = sum(qt*kdps) = (den_e-den_o)/2 with kdps = +-kdp,
                    # so den_e = A+B (rows 0-63) and den_o = A-B (rows 64+).
                    kdp = smalls.tile([P, KC], F32)
                    for h in range(H):
                        pr = (h % 2) * 64
                        c = h // 2
                        sc = 256 + c if c < 4 else 516 + (c - 4)
                        nc.vector.tensor_scalar(
                            kdp[pr:pr + 64, h // 2:h // 2 + 1],
                            ar_sb[pr:pr + 64, sc:sc + 1], 0.5, None, OP.mult)
                    sgn = smalls.tile([P, 1], F32)
                    nc.vector.memset(sgn[0:64, :], 1.0)
                    nc.vector.memset(sgn[64:128, :], -1.0)
                    kdps = smalls.tile([P, KC], F32)
                    nc.vector.tensor_scalar(kdps[:], kdp[:], sgn[:, 0:1], None, OP.mult)
                    # Block-diagonal KV stationary per head pair: one 128-wide
                    # matmul computes both heads' attention numerators.
                    # kd2 col h: Ksum_h (unhalved) at rows (h%2)*64, for the
                    # PE-matmul den path used by the last two head pairs.
                    kd2 = smalls.tile([P, H], F32)
                    nc.any.memset(kd2[:], 0.0)
                    for h in range(H):
                        pr = (h % 2) * 64
                        c = h // 2
                        sc = 256 + c if c < 4 else 516 + (c - 4)
                        nc.any.tensor_copy(kd2[pr:pr + 64, h:h + 1],
                                           ar_sb[pr:pr + 64, sc:sc + 1])
                    kvq = smalls.tile([P, KC, P], BF16)
                    nc.any.memset(kvq[:], 0.0)
                    for hp in range(KC):
                        nc.any.tensor_copy(kvq[0:64, hp, 0:64],
                                           kv_bf[0:64, hp * 64:(hp + 1) * 64])
                        nc.any.tensor_copy(kvq[64:128, hp, 64:128],
                                           kv_bf[64:128, hp * 64:(hp + 1) * 64])

                    for k in range(KC):
                        nc.sync.dma_start(woh_sb[:, k, :], r8(woh)[:, k, :])

                    def q_proj(hp):
                        wq_t = wq_tiles.pop(hp)
                        qt = qtp.tile([P, TOK], F32, tag="qt", name=f"qt_{hp}")
                        qt_tiles[hp] = qt
                        # k-outer over all four tq psum tiles: each arriving
                        # x^T chunk immediately feeds 12 matmuls, so the
                        # hp=0 wave overlaps the xthi/xtlo DMA chunk-by-chunk.
                        qps_t = [ph3ps.tile([P, 512], F32, tag="qps",
                                            name=f"qps_{hp}_{tq}") for tq in range(TQ)]
                        for k in range(KC):
                            st, sp = (k == 0), (k == KC - 1)
                            for tq in range(TQ):
                                tsl = slice(tq * 512, (tq + 1) * 512)
                                nc.tensor.matmul(qps_t[tq][:], wq_t[:, k, 0, :],
                                                 xthi_sb[:, k, tsl], start=st, stop=False)
                                nc.tensor.matmul(qps_t[tq][:], wq_t[:, k, 1, :],
                                                 xthi_sb[:, k, tsl], start=False, stop=False)
                                nc.tensor.matmul(qps_t[tq][:], wq_t[:, k, 0, :],
                                                 xtlo_sb[:, k, tsl], start=False, stop=sp)
                        for tq in range(TQ):
                            tsl = slice(tq * 512, (tq + 1) * 512)
                            qps = qps_t[tq]
                            if apply_bias:
                                qraw = ph3sb.tile([P, 512], F32, tag="qraw",
                                                  name=f"qraw_{hp}_{tq}")
                                nc.vector.tensor_scalar(qraw[:], qps[:], bq_sb[:, hp:hp + 1],
                                                        None, OP.add)
                                qsrc = qraw
                            else:
                                qsrc = qps
                            qmin = ph3sb.tile([P, 512], F32, tag="qmin", name=f"qmin_{hp}_{tq}")
                            nc.vector.tensor_scalar(qmin[:], qsrc[:], 0.0, None, OP.min)
                            qexp = ph3sb.tile([P, 512], F32, tag="qexp", name=f"qexp_{hp}_{tq}")
                            nc.scalar.activation(qexp[:], qmin[:], AF.Exp)
                            qmax = ph3sb.tile([P, 512], F32, tag="qmax", name=f"qmax_{hp}_{tq}")
                            nc.vector.tensor_scalar(qmax[:], qsrc[:], 0.0, -1.0, OP.max, OP.add)
                            nc.vector.tensor_tensor(qt[:, tsl], qmax[:], qexp[:], OP.add)

                    def attention(hp, pe_den=False):
                        qt = qt_tiles.pop(hp)
                        if DEBUG_TAPS and hp == 0:
                            nc.sync.dma_start(dbg_qt0.ap(), qt[:])
                        for tq in range(TQ):
                            tsl = slice(tq * 512, (tq + 1) * 512)
                            if pe_den:
                                # Tail head-pairs: the PE is idling here and
                                # the den gates phase 5, so compute it with
                                # the short f32-matmul + broadcast chain.
                                dps = ph3ps.tile([2, 512], F32, tag="dps",
                                                 name=f"dps_{hp}_{tq}", bufs=1)
                                nc.tensor.matmul(dps[:], kd2[:, 2 * hp:2 * hp + 2],
                                                 qt[:, tsl], start=True, stop=True)
                                zrf = ph4sb.tile([2, 512], F32, tag="sprd",
                                                 name=f"zrf_{hp}_{tq}")
                                nc.vector.reciprocal(zrf[:], dps[:])
                                zr = ph4sb.tile([2, 512], BF16, tag="qbf",
                                                name=f"zr_{hp}_{tq}")
                                nc.vector.tensor_copy(zr[:], zrf[:])
                                zps = ph3ps.tile([P, 512], F32, tag="zps",
                                                 name=f"zps_{hp}_{tq}", bufs=1)
                                nc.tensor.matmul(zps[:], e_sb[:], zr[:],
                                                 start=True, stop=True)
                                denA = ph4sb.tile([P, 512], F32, tag="denA",
                                                  name=f"denA_{hp}_{tq}")
                                nc.vector.tensor_copy(denA[:], zps[:])
                                qbf = ph4sb.tile([P, 512], BF16, tag="qbf",
                                                 name=f"qbf_{hp}_{tq}")
                                nc.vector.tensor_copy(qbf[:], qt[:, tsl])
                                aps = ph4ps_a.tile([P, 512], F32, tag="aps",
                                                   name=f"aps_{hp}_{tq}")
                                nc.tensor.matmul(aps[:], kvq[:, hp, :], qbf[:],
                                                 start=True, stop=True)
                                nc.vector.tensor_tensor(at_sb[:, hp, tsl], aps[:],
                                                        denA[:], OP.mult)
                                continue
                            # den on gpsimd+DVE (keeps PE free); see kdp note.
                            prod = ph4sb.tile([P, 512], F32, tag="prod",
                                              name=f"prod_{hp}_{tq}")
                            nc.vector.tensor_scalar(prod[:], qt[:, tsl],
                                                    kdp[:, hp:hp + 1], None, OP.mult)
                            sprd = ph4sb.tile([P, 512], F32, tag="sprd",
                                              name=f"sprd_{hp}_{tq}")
                            nc.vector.tensor_scalar(sprd[:], qt[:, tsl],
                                                    kdps[:, hp:hp + 1], None, OP.mult)
                            denA = ph4sb.tile([P, 512], F32, tag="denA",
                                              name=f"denA_{hp}_{tq}")
                            denB = ph4sb.tile([P, 512], F32, tag="denB",
                                              name=f"denB_{hp}_{tq}")
                            nc.gpsimd.partition_all_reduce(
                                denA[:], prod[:], channels=128,
                                reduce_op=bass_isa.ReduceOp.add)
                            nc.gpsimd.partition_all_reduce(
                                denB[:], sprd[:], channels=128,
                                reduce_op=bass_isa.ReduceOp.add)
                            nc.vector.tensor_tensor(denA[0:64, :], denA[0:64, :],
                                                    denB[0:64, :], OP.add)
                            nc.vector.tensor_tensor(denA[64:128, :], denA[64:128, :],
                                                    denB[64:128, :], OP.subtract)
                            nc.vector.reciprocal(denA[:], denA[:])
                            qbf = ph4sb.tile([P, 512], BF16, tag="qbf", name=f"qbf_{hp}_{tq}")
                            nc.vector.tensor_copy(qbf[:], qt[:, tsl])
                            aps = ph4ps_a.tile([P, 512], F32, tag="aps", name=f"aps_{hp}_{tq}")
                            nc.tensor.matmul(aps[:], kvq[:, hp, :], qbf[:],
                                             start=True, stop=True)
                            nc.vector.tensor_tensor(at_sb[:, hp, tsl], aps[:], denA[:], OP.mult)

                    # depth-2 software pipeline: attention(hp) runs two Q chunks
                    # behind, so the AllReduce hides under ~3 Q projections.
                    # Software pipeline: depth 2 while the AllReduce is in
                    # flight, catching up to depth 1 at hp=4 so only
                    # attention(7)'s den chain trails into phase 5.
                    att_sched = {2: [0], 3: [1], 4: [2, 3], 5: [4], 6: [5], 7: [6]}
                    q_proj(0)
                    for hp in range(1, KC):
                        if hp + 1 < KC:
                            load_wq(hp + 1)
                        q_proj(hp)
                        for a in att_sched.get(hp, []):
                            attention(a, pe_den=(a <= 1 or a >= KC - 2))
                    attention(KC - 1, pe_den=True)

                if DEBUG_TAPS:
                    with tc.tile_pool(name="dbgat", bufs=2) as dbgat:
                        for c in range(KC):
                            atf = dbgat.tile([P, TOK], F32, tag="atf", name=f"atf_{c}")
                            nc.vector.tensor_copy(atf[:], at_sb[:, c, :])
                            nc.sync.dma_start(dbg_at.ap()[:, c, :], atf[:])

                # ===== Phase 5: output projection + residual + LayerNorm =====
                with (
                    tc.tile_pool(name="ph5ps", bufs=4, space="PSUM") as ph5ps,
                    tc.tile_pool(name="ph5sb", bufs=4) as ph5sb,
                ):
                    for t in range(TC):
                        ts = slice(t * P, (t + 1) * P)
                        y = ph5sb.tile([P, D], F32, tag="y", name=f"y_{t}")
                        xr = ph5sb.tile([P, D], F32, tag="xr", name=f"xr_{t}")
                        nc.sync.dma_start(xr[:], xres.ap()[ts, :])
                        ops = ph5ps.tile([P, D], F32, tag="ops", name=f"ops_{t}")
                        for dh in range(2):
                            dsl = slice(dh * 512, (dh + 1) * 512)
                            for c in range(KC):
                                nc.tensor.matmul(ops[:, dsl], at_sb[:, c, ts], woh_sb[:, c, dsl],
                                                 start=(c == 0), stop=(c == KC - 1))
                        nc.vector.tensor_tensor(y[:], ops[:], xr[:], OP.add)
                        if apply_bias:
                            nc.vector.tensor_tensor(y[:], y[:], bo_b[:], OP.add)
                        stats = ph5sb.tile([P, 2, 6], F32, tag="stats", name=f"stats_{t}")
                        nc.vector.bn_stats(out=stats[:, 0, :], in_=y[:, :512])
                        nc.vector.bn_stats(out=stats[:, 1, :], in_=y[:, 512:])
                        mv = ph5sb.tile([P, 2], F32, tag="mv", name=f"mv_{t}")
                        nc.vector.bn_aggr(out=mv[:], in_=stats[:])
                        nc.scalar.activation(out=mv[:, 1:2], in_=mv[:, 1:2], func=AF.Sqrt,
                                             bias=eps_sb[:], scale=1.0)
                        nc.vector.reciprocal(mv[:, 1:2], mv[:, 1:2])
                        yo = ph5sb.tile([P, D], F32, tag="yo", name=f"yo_{t}")
                        nc.gpsimd.tensor_scalar(yo[:], y[:], mv[:, 0:1], mv[:, 1:2],
                                                OP.subtract, OP.mult)
                        if apply_gamma:
                            nc.vector.tensor_tensor(yo[:], yo[:], gamma_b[:], OP.mult)
                        if apply_beta:
                            nc.vector.tensor_tensor(yo[:], yo[:], beta_b[:], OP.add)
                        nc.sync.dma_start(out_d.ap()[ts, :], yo[:])

            wqp_cm.__exit__(None, None, None)

    nc.compile()
    return nc


def kernel(x, Wq, bq, Wk, bk, Wv, bv, Wo, bo, gamma, beta):
    global LAST_RESULT
    x = np.asarray(x, dtype=np.float32)
    f32 = np.float32
    bf16 = ml_dtypes.bfloat16

    apply_bias = any(np.any(np.asarray(b)) for b in (bq, bk, bv, bo))
    apply_gamma = not np.all(np.asarray(gamma) == 1.0)
    apply_beta = bool(np.any(np.asarray(beta)))

    nc = _build(apply_bias, apply_gamma, apply_beta)

    def split(W):
        W = np.asarray(W, dtype=f32)
        hi = W.astype(bf16)
        lo = (W - hi.astype(f32)).astype(bf16)
        return hi, lo

    wq_h, wq_l = split(Wq)
    wk_f = np.ascontiguousarray(np.asarray(Wk, dtype=f32))
    wv_f = np.ascontiguousarray(np.asarray(Wv, dtype=f32))
    wo_h, _ = split(Wo)
    e_sel = np.zeros((2, P), dtype=bf16)
    e_sel[0, :64] = 1
    e_sel[1, 64:] = 1

    in_maps = []
    for c in range(NCORES):
        b, half = c // 2, c % 2
        xs = x[b, half * TOK:(half + 1) * TOK]          # [2048, 1024]
        xhi = xs.astype(bf16)
        xlo = (xs - xhi.astype(f32)).astype(bf16)
        m = {
            "xtf": np.ascontiguousarray(xs.T),
            "xthi": np.ascontiguousarray(xhi.T),
            "xtlo": np.ascontiguousarray(xlo.T),
            "wqh": wq_h, "wql": wq_l,
            "wk": wk_f, "wv": wv_f, "woh": wo_h,
            "xres": np.ascontiguousarray(xs),
            "e_sel": e_sel,
        }
        if apply_bias:
            m.update(bq=np.asarray(bq, f32), bk=np.asarray(bk, f32),
                     bv=np.asarray(bv, f32), bo=np.asarray(bo, f32))
        if apply_gamma:
            m["gamma"] = np.asarray(gamma, f32)
        if apply_beta:
            m["beta"] = np.asarray(beta, f32)
        in_maps.append(m)

    import os
    try:
        LAST_RESULT = run_bass_kernel_spmd(nc, in_maps, core_ids=list(range(NCORES)))
    except ModuleNotFoundError:
        # no antenv.axon_hooks in this container -> NTFF tracing unavailable
        os.environ["BASS_NEVER_TRACE"] = "1"
        LAST_RESULT = run_bass_kernel_spmd(nc, in_maps, core_ids=list(range(NCORES)))
    out = np.empty((B, N, D), dtype=np.float32)
    for c in range(NCORES):
        b, half = c // 2, c % 2
        out[b, half * TOK:(half + 1) * TOK] = LAST_RESULT.results[c]["out"]
    return out


# revision 63
# speedup vs baseline: 1.0636x; 1.0047x over previous
"""Distributed Trainium2 kernel for the linear-attention transformer block.

Math (per batch element b):
  Q = elu(x @ Wq + bq), K = elu(x @ Wk + bk), V = x @ Wv + bv   (per-head d=64)
  KV_h = K_h^T V_h  [64,64];  Ksum_h = sum_n K_h[n]  [64]
  attn_h = (Q_h @ KV_h) / (Q_h . Ksum_h)
  out = LayerNorm(x + attn @ Wo + bo) * gamma + beta

Sharding: 16384 tokens over 8 cores (2048 each; core c owns batch c//2,
half c%2). Each core computes Q/K/V only for its tokens, partial KV/Ksum,
then a 266KB AllReduce over core pairs {2b, 2b+1} completes the KV stats;
attention + output projection + LayerNorm finish locally.

Precision: the 1/(Q.Ksum) denominators pass near zero (min |den| on the
nominal instance is ~0.35) and amplify projection noise into sign flips,
so the Q projection uses a 3-term bf16 split (x_hi@W_hi + x_hi@W_lo +
x_lo@W_hi; per-element err ~5e-6). The K/V projections run as single
float32r matmuls (full PE rate at N=512; measured HW err ~1.5e-4), which
keeps the Ksum-side denominator error ~0.07 rms - flip-safe with >5 sigma
margin. Ksum and the denominator run in f32. ~1.7e-3 global rel err.
"""

import sys

sys.path.insert(0, "/opt/trn_rl_repo")

import numpy as np
import ml_dtypes

import concourse.bass as bass
import concourse.bass_isa as bass_isa
import concourse.mybir as mybir
import concourse.tile as tile
from concourse import bacc
from concourse.bass_utils import run_bass_kernel_spmd

AF = mybir.ActivationFunctionType
OP = mybir.AluOpType
F32 = mybir.dt.float32
F32R = mybir.dt.float32r
BF16 = mybir.dt.bfloat16

B, N, D = 4, 4096, 1024
H, HD = 16, 64
TOK = 2048            # tokens per core
NCORES = 8
LN_EPS = 1e-3
P = 128
KC = D // P           # 8 contraction chunks
TC = TOK // P         # 16 token chunks of 128
TQ = TOK // 512       # 4 token chunks of 512
WARM_MM = 10          # PE clock-gate warmup matmuls

LAST_RESULT = None    # BassKernelResults of the most recent run (for test.py)
DEBUG_TAPS = False    # set True (e.g. from debug.py) to add intermediate outputs


def _build(apply_bias, apply_gamma, apply_beta):
    nc = bacc.Bacc("TRN2", target_bir_lowering=False, debug=False, num_devices=NCORES)

    def din(name, shape, dtype=BF16):
        return nc.dram_tensor(name, shape, dtype, kind="ExternalInput")

    xtf = din("xtf", [D, TOK], F32R)
    xthi = din("xthi", [D, TOK])
    xtlo = din("xtlo", [D, TOK])
    wqh = din("wqh", [D, D])
    wql = din("wql", [D, D])
    wk = din("wk", [D, D], F32R)
    wv = din("wv", [D, D], F32R)
    woh = din("woh", [D, D])
    xres = din("xres", [TOK, D], F32)
    e_sel = din("e_sel", [2, P])
    if apply_bias:
        bq_d = din("bq", [D], F32)
        bk_d = din("bk", [D], F32)
        bv_d = din("bv", [D], F32)
        bo_d = din("bo", [D], F32)
    if apply_gamma:
        gamma_d = din("gamma", [D], F32)
    if apply_beta:
        beta_d = din("beta", [D], F32)
    out_d = nc.dram_tensor("out", [TOK, D], F32, kind="ExternalOutput")
    if DEBUG_TAPS:
        dbg_k0 = nc.dram_tensor("dbg_k0", [P, 512], F32, kind="ExternalOutput")
        dbg_kv = nc.dram_tensor("dbg_kv", [P, 512], F32, kind="ExternalOutput")
        dbg_ksum = nc.dram_tensor("dbg_ksum", [P, KC], F32, kind="ExternalOutput")
        dbg_ar = nc.dram_tensor("dbg_ar", [P, 520], F32, kind="ExternalOutput")
        dbg_qt0 = nc.dram_tensor("dbg_qt0", [P, TOK], F32, kind="ExternalOutput")
        dbg_at = nc.dram_tensor("dbg_at", [P, KC, TOK], F32, kind="ExternalOutput")

    r8 = lambda t: t.ap().rearrange("(ko p) n -> p ko n", p=P)

    def bcast_row(dram_vec, sb_tile):
        # DMA-broadcast a [D] vector to [P, D] (stride-0 partition dim).
        src = bass.AP(
            tensor=dram_vec.ap().tensor,
            offset=dram_vec.ap().offset,
            ap=[[0, P]] + list(dram_vec.ap().ap),
        )
        nc.sync.dma_start(out=sb_tile, in_=src)

    with tile.TileContext(nc) as tc:
        with (
            tc.tile_pool(name="smalls", bufs=1) as smalls,
            tc.tile_pool(name="dram", bufs=1, space="DRAM") as dram,
        ):
            e_sb = smalls.tile([2, P], BF16)
            nc.sync.dma_start(e_sb[:], e_sel.ap())
            ones_sb = smalls.tile([P, 1], F32)
            nc.vector.memset(ones_sb[:], 1.0)
            eps_sb = smalls.tile([P, 1], F32)
            nc.vector.memset(eps_sb[:], LN_EPS)
            if apply_bias:
                bq_sb = smalls.tile([P, KC], F32)   # per-partition layout for Q^T
                nc.sync.dma_start(bq_sb[:], bq_d.ap().rearrange("(ko p) -> p ko", p=P))
                bk_b = smalls.tile([P, D], F32)
                bv_b = smalls.tile([P, D], F32)
                bo_b = smalls.tile([P, D], F32)
                bcast_row(bk_d, bk_b[:])
                bcast_row(bv_d, bv_b[:])
                bcast_row(bo_d, bo_b[:])
            if apply_gamma:
                gamma_b = smalls.tile([P, D], F32)
                bcast_row(gamma_d, gamma_b[:])
            if apply_beta:
                beta_b = smalls.tile([P, D], F32)
                bcast_row(beta_d, beta_b[:])

            if DEBUG_TAPS:
                kv_loc = smalls.tile([P, 512], F32)
                ksum_loc = smalls.tile([P, KC], F32)

            # PE warmup: matmuls on zeroed tiles release the HAM clock
            # gate (1.2->2.4 GHz) while the first input DMAs are in flight.
            with (
                tc.tile_pool(name="warmsb", bufs=1) as warmsb,
                tc.tile_pool(name="warmps", bufs=2, space="PSUM") as warmps,
            ):
                warm_a = warmsb.tile([P, P], BF16)
                warm_b = warmsb.tile([P, 512], BF16)
                nc.gpsimd.memset(warm_a[:], 0.0)
                nc.gpsimd.memset(warm_b[:], 0.0)
                for w in range(WARM_MM):
                    wp = warmps.tile([P, 512], F32, tag="warm", name=f"warm_{w}")
                    nc.tensor.matmul(wp[:], warm_a[:], warm_b[:], start=True, stop=True)

            # Prefetch the first two Q-weight slices; their DMAs have no deps
            # and fill otherwise-idle DMA time during phase 1.
            wqp_cm = tc.tile_pool(name="wqp", bufs=3)
            wqp = wqp_cm.__enter__()
            wq_tiles = {}

            def load_wq(hp):
                msl = slice(hp * P, (hp + 1) * P)
                wq_t = wqp.tile([P, KC, 2, P], BF16, tag="wq", name=f"wq_{hp}")
                nc.sync.dma_start(wq_t[:, :, 0, :], r8(wqh)[:, :, msl])
                nc.sync.dma_start(wq_t[:, :, 1, :], r8(wql)[:, :, msl])
                wq_tiles[hp] = wq_t

            # ================= Phase 1: K, V, partial KV + Ksum =================
            # K = x @ Wk and V = x @ Wv as single f32r matmuls (full PE rate
            # at N=512). x^T is resident in f32; the four psum accumulators
            # (K/V x dh halves) share each stationary x^T chunk.
            with (
                tc.tile_pool(name="ph1x", bufs=1) as ph1x,
                tc.tile_pool(name="wkv", bufs=1) as wkv,
                tc.tile_pool(name="kvps_pool", bufs=1, space="PSUM") as kvps_pool,
                tc.tile_pool(name="ph1ps", bufs=7, space="PSUM") as ph1ps,
                tc.tile_pool(name="ph1sb", bufs=4) as ph1sb,
            ):
                xtf_sb = ph1x.tile([P, KC, TOK], F32R)
                wk_sb = wkv.tile([P, KC, D], F32R)
                wv_sb = wkv.tile([P, KC, D], F32R)
                # DMA order: first t=0..1 x chunks + Wk (needed by t=0 K
                # matmuls), then Wv, then remaining x token-sliced t-major.
                for k in range(KC):
                    nc.sync.dma_start(wk_sb[:, k, :], r8(wk)[:, k, :])
                    nc.sync.dma_start(xtf_sb[:, k, 0:2 * P], r8(xtf)[:, k, 0:2 * P])
                for k in range(KC):
                    nc.sync.dma_start(wv_sb[:, k, :], r8(wv)[:, k, :])
                for t in range(2, TC):
                    ts = slice(t * P, (t + 1) * P)
                    for k in range(KC):
                        nc.sync.dma_start(xtf_sb[:, k, ts], r8(xtf)[:, k, ts])
                load_wq(0)
                load_wq(1)

                # SBUF accumulator (DVE-add per token chunk; interleaved
                # multi-chunk PSUM accumulation groups proved unreliable).
                # Layout per dh half: [dh*260, dh*260+256) = KV, [+256, +260) = Ksum.
                acc = smalls.tile([P, 520], F32)
                nc.vector.memset(acc[:], 0.0)

                for t in range(TC):
                    ts = slice(t * P, (t + 1) * P)
                    kps = {}
                    vps = {}
                    for dh in range(2):
                        kps[dh] = ph1ps.tile([P, 512], F32, tag="proj",
                                             name=f"kps_{t}_{dh}")
                        vps[dh] = ph1ps.tile([P, 512], F32, tag="proj",
                                             name=f"vps_{t}_{dh}")
                    for k in range(KC):
                        st, sp = (k == 0), (k == KC - 1)
                        for dh in range(2):
                            dsl = slice(dh * 512, (dh + 1) * 512)
                            nc.tensor.matmul(kps[dh][:], xtf_sb[:, k, ts],
                                             wk_sb[:, k, dsl], start=st, stop=sp)
                            nc.tensor.matmul(vps[dh][:], xtf_sb[:, k, ts],
                                             wv_sb[:, k, dsl], start=st, stop=sp)
                    kb_chunks = []
                    kvs_tiles = {}
                    for dh in range(2):
                        dsl = slice(dh * 512, (dh + 1) * 512)
                        if apply_bias:
                            kraw = ph1sb.tile([P, 512], F32, tag="kraw", name=f"kraw_{t}_{dh}")
                            nc.vector.tensor_tensor(kraw[:], kps[dh][:], bk_b[:, dsl], OP.add)
                            ksrc = kraw
                        else:
                            ksrc = kps[dh]
                        kmin = ph1sb.tile([P, 512], F32, tag="kmin", name=f"kmin_{t}_{dh}")
                        nc.vector.tensor_scalar(kmin[:], ksrc[:], 0.0, None, OP.min)
                        kexp = ph1sb.tile([P, 512], F32, tag="kexp", name=f"kexp_{t}_{dh}")
                        nc.scalar.activation(kexp[:], kmin[:], AF.Exp)
                        kmax = ph1sb.tile([P, 512], F32, tag="kmax", name=f"kmax_{t}_{dh}")
                        nc.vector.tensor_scalar(kmax[:], ksrc[:], 0.0, -1.0, OP.max, OP.add)
                        kf = ph1sb.tile([P, 512], F32, tag="kf", name=f"kf_{t}_{dh}")
                        nc.vector.tensor_tensor(kf[:], kmax[:], kexp[:], OP.add)
                        kb = ph1sb.tile([P, 512], BF16, tag="kb", name=f"kb_{t}_{dh}")
                        nc.vector.tensor_copy(kb[:], kf[:])
                        if DEBUG_TAPS and t == 0 and dh == 0:
                            nc.sync.dma_start(dbg_k0.ap(), kf[:])
                        kb_chunks.append(kb)
                        # Ksum column blocks (f32 matmul against ones) go into
                        # cols [256, 260) of the shared kvs_t psum tile.
                        kvs_t = kvps_pool.tile([P, 260], F32, tag="kvs_t",
                                               name=f"kvs_t_{t}_{dh}", bufs=1)
                        kvs_tiles[dh] = kvs_t
                        for j in range(4):
                            nc.tensor.matmul(
                                kvs_t[:, 256 + j:257 + j], kf[:, j * P:(j + 1) * P],
                                ones_sb[:], start=True, stop=True, skip_group_check=True)
                    for dh in range(2):
                        dsl = slice(dh * 512, (dh + 1) * 512)
                        vb = ph1sb.tile([P, 512], BF16, tag="vb", name=f"vb_{t}_{dh}")
                        if apply_bias:
                            nc.vector.tensor_tensor(vb[:], vps[dh][:], bv_b[:, dsl], OP.add)
                        else:
                            nc.any.tensor_copy(vb[:], vps[dh][:])
                        kb = kb_chunks[dh]
                        kvs_t = kvs_tiles[dh]
                        for hh in range(8):
                            h = dh * 8 + hh
                            pr = (h % 2) * 64
                            fc = (h // 2) * 64 - dh * 256
                            nc.tensor.matmul(
                                kvs_t[pr:pr + 64, fc:fc + 64],
                                kb[:, hh * 64:(hh + 1) * 64],
                                vb[:, hh * 64:(hh + 1) * 64],
                                start=True, stop=True,
                                tile_position=(0, pr), skip_group_check=True)
                        nc.vector.tensor_tensor(
                            acc[:, dh * 260:(dh + 1) * 260],
                            acc[:, dh * 260:(dh + 1) * 260], kvs_t[:], OP.add)

                if DEBUG_TAPS:
                    nc.vector.tensor_copy(kv_loc[:, :256], acc[:, :256])
                    nc.vector.tensor_copy(kv_loc[:, 256:], acc[:, 260:516])
                    nc.vector.tensor_copy(ksum_loc[:, :4], acc[:, 256:260])
                    nc.vector.tensor_copy(ksum_loc[:, 4:], acc[:, 516:520])
                    nc.sync.dma_start(dbg_kv.ap(), kv_loc[:])
                    nc.sync.dma_start(dbg_ksum.ap(), ksum_loc[:])

            # ========== Phases 2-4: AllReduce; Q^T; attention (pipelined) ==========
            with (
                tc.tile_pool(name="qx", bufs=1) as qx,
                tc.tile_pool(name="late", bufs=1) as late,
            ):
                # bf16 hi/lo x^T for the Q 3-term split; DMA'd into the SBUF
                # space phase 1 just freed, overlapping the AllReduce.
                xthi_sb = qx.tile([P, KC, TOK], BF16)
                xtlo_sb = qx.tile([P, KC, TOK], BF16)
                for k in range(KC):
                    nc.sync.dma_start(xthi_sb[:, k, :], r8(xthi)[:, k, :])
                    nc.sync.dma_start(xtlo_sb[:, k, :], r8(xtlo)[:, k, :])

                at_sb = late.tile([P, KC, TOK], BF16)
                woh_sb = late.tile([P, KC, D], BF16)

                with (
                    tc.tile_pool(name="qtp", bufs=4) as qtp,
                    tc.tile_pool(name="ph3ps", bufs=4, space="PSUM") as ph3ps,
                    tc.tile_pool(name="ph3sb", bufs=2) as ph3sb,
                    tc.tile_pool(name="ph4ps_a", bufs=2, space="PSUM") as ph4ps_a,
                    tc.tile_pool(name="ph4sb", bufs=3) as ph4sb,
                ):
                    qt_tiles = {}

                    # -- AllReduce of the packed KV/Ksum accumulator --
                    cc_in = dram.tile([P, 520], F32)
                    cc_out = dram.tile([P, 520], F32)
                    nc.sync.dma_start(cc_in[:], acc[:])
                    nc.gpsimd.collective_compute(
                        "AllReduce", OP.add,
                        replica_groups=[[0, 1], [2, 3], [4, 5], [6, 7]],
                        ins=[cc_in[:].opt()], outs=[cc_out[:].opt()])
                    ar_sb = smalls.tile([P, 520], F32)
                    nc.sync.dma_start(ar_sb[:], cc_out[:])
                    if DEBUG_TAPS:
                        nc.sync.dma_start(dbg_ar.ap(), ar_sb[:])
                    kv_bf = smalls.tile([P, 512], BF16)
                    nc.any.tensor_copy(kv_bf[:, :256], ar_sb[:, :256])
                    nc.any.tensor_copy(kv_bf[:, 256:], ar_sb[:, 260:516])
                    # kdp col hp: Ksum_{2hp}/2 on partitions 0-63 and
                    # Ksum_{2hp+1}/2 on 64-127 -- aligned with qt's partition
                    # layout. den halves come from two full-128 partition
                    # reduces (the gpsimd ucode only supports base partition
                    # 0): A = sum(qt*kdp) = (den_e+den_o)/2 over all 128,
                    # B = sum(qt*kdps) = (den_e-den_o)/2 with kdps = +-kdp,
                    # so den_e = A+B (rows 0-63) and den_o = A-B (rows 64+).
                    kdp = smalls.tile([P, KC], F32)
                    for h in range(H):
                        pr = (h % 2) * 64
                        c = h // 2
                        sc = 256 + c if c < 4 else 516 + (c - 4)
                        nc.vector.tensor_scalar(
                            kdp[pr:pr + 64, h // 2:h // 2 + 1],
                            ar_sb[pr:pr + 64, sc:sc + 1], 0.5, None, OP.mult)
                    sgn = smalls.tile([P, 1], F32)
                    nc.vector.memset(sgn[0:64, :], 1.0)
                    nc.vector.memset(sgn[64:128, :], -1.0)
                    kdps = smalls.tile([P, KC], F32)
                    nc.vector.tensor_scalar(kdps[:], kdp[:], sgn[:, 0:1], None, OP.mult)
                    # Block-diagonal KV stationary per head pair: one 128-wide
                    # matmul computes both heads' attention numerators.
                    # kd2 col h: Ksum_h (unhalved) at rows (h%2)*64, for the
                    # PE-matmul den path used by the last two head pairs.
                    kd2 = smalls.tile([P, H], F32)
                    nc.any.memset(kd2[:], 0.0)
                    for h in range(H):
                        pr = (h % 2) * 64
                        c = h // 2
                        sc = 256 + c if c < 4 else 516 + (c - 4)
                        nc.any.tensor_copy(kd2[pr:pr + 64, h:h + 1],
                                           ar_sb[pr:pr + 64, sc:sc + 1])
                    kvq = smalls.tile([P, KC, P], BF16)
                    nc.any.memset(kvq[:], 0.0)
                    for hp in range(KC):
                        nc.any.tensor_copy(kvq[0:64, hp, 0:64],
                                           kv_bf[0:64, hp * 64:(hp + 1) * 64])
                        nc.any.tensor_copy(kvq[64:128, hp, 64:128],
                                           kv_bf[64:128, hp * 64:(hp + 1) * 64])

                    for k in range(KC):
                        nc.sync.dma_start(woh_sb[:, k, :], r8(woh)[:, k, :])

                    def q_proj(hp):
                        wq_t = wq_tiles.pop(hp)
                        qt = qtp.tile([P, TOK], F32, tag="qt", name=f"qt_{hp}")
                        qt_tiles[hp] = qt
                        # k-outer over all four tq psum tiles: each arriving
                        # x^T chunk immediately feeds 12 matmuls, so the
                        # hp=0 wave overlaps the xthi/xtlo DMA chunk-by-chunk.
                        qps_t = [ph3ps.tile([P, 512], F32, tag="qps",
                                            name=f"qps_{hp}_{tq}") for tq in range(TQ)]
                        for k in range(KC):
                            st, sp = (k == 0), (k == KC - 1)
                            for tq in range(TQ):
                                tsl = slice(tq * 512, (tq + 1) * 512)
                                nc.tensor.matmul(qps_t[tq][:], wq_t[:, k, 0, :],
                                                 xthi_sb[:, k, tsl], start=st, stop=False)
                                nc.tensor.matmul(qps_t[tq][:], wq_t[:, k, 1, :],
                                                 xthi_sb[:, k, tsl], start=False, stop=False)
                                nc.tensor.matmul(qps_t[tq][:], wq_t[:, k, 0, :],
                                                 xtlo_sb[:, k, tsl], start=False, stop=sp)
                        for tq in range(TQ):
                            tsl = slice(tq * 512, (tq + 1) * 512)
                            qps = qps_t[tq]
                            if apply_bias:
                                qraw = ph3sb.tile([P, 512], F32, tag="qraw",
                                                  name=f"qraw_{hp}_{tq}")
                                nc.vector.tensor_scalar(qraw[:], qps[:], bq_sb[:, hp:hp + 1],
                                                        None, OP.add)
                                qsrc = qraw
                            else:
                                qsrc = qps
                            qmin = ph3sb.tile([P, 512], F32, tag="qmin", name=f"qmin_{hp}_{tq}")
                            nc.vector.tensor_scalar(qmin[:], qsrc[:], 0.0, None, OP.min)
                            qexp = ph3sb.tile([P, 512], F32, tag="qexp", name=f"qexp_{hp}_{tq}")
                            nc.scalar.activation(qexp[:], qmin[:], AF.Exp)
                            qmax = ph3sb.tile([P, 512], F32, tag="qmax", name=f"qmax_{hp}_{tq}")
                            nc.vector.tensor_scalar(qmax[:], qsrc[:], 0.0, -1.0, OP.max, OP.add)
                            nc.vector.tensor_tensor(qt[:, tsl], qmax[:], qexp[:], OP.add)

                    def attention(hp, pe_den=False):
                        qt = qt_tiles.pop(hp)
                        if DEBUG_TAPS and hp == 0:
                            nc.sync.dma_start(dbg_qt0.ap(), qt[:])
                        for tq in range(TQ):
                            tsl = slice(tq * 512, (tq + 1) * 512)
                            if pe_den:
                                # Tail head-pairs: the PE is idling here and
                                # the den gates phase 5, so compute it with
                                # the short f32-matmul + broadcast chain.
                                dps = ph3ps.tile([2, 512], F32, tag="dps",
                                                 name=f"dps_{hp}_{tq}", bufs=1)
                                nc.tensor.matmul(dps[:], kd2[:, 2 * hp:2 * hp + 2],
                                                 qt[:, tsl], start=True, stop=True)
                                zrf = ph4sb.tile([2, 512], F32, tag="sprd",
                                                 name=f"zrf_{hp}_{tq}")
                                nc.vector.reciprocal(zrf[:], dps[:])
                                zr = ph4sb.tile([2, 512], BF16, tag="qbf",
                                                name=f"zr_{hp}_{tq}")
                                nc.vector.tensor_copy(zr[:], zrf[:])
                                zps = ph3ps.tile([P, 512], F32, tag="zps",
                                                 name=f"zps_{hp}_{tq}", bufs=1)
                                nc.tensor.matmul(zps[:], e_sb[:], zr[:],
                                                 start=True, stop=True)
                                denA = ph4sb.tile([P, 512], F32, tag="denA",
                                                  name=f"denA_{hp}_{tq}")
                                nc.vector.tensor_copy(denA[:], zps[:])
                                qbf = ph4sb.tile([P, 512], BF16, tag="qbf",
                                                 name=f"qbf_{hp}_{tq}")
                                nc.vector.tensor_copy(qbf[:], qt[:, tsl])
                                aps = ph4ps_a.tile([P, 512], F32, tag="aps",
                                                   name=f"aps_{hp}_{tq}")
                                nc.tensor.matmul(aps[:], kvq[:, hp, :], qbf[:],
                                                 start=True, stop=True)
                                nc.vector.tensor_tensor(at_sb[:, hp, tsl], aps[:],
                                                        denA[:], OP.mult)
                                continue
                            # den on gpsimd+DVE (keeps PE free); see kdp note.
                            prod = ph4sb.tile([P, 512], F32, tag="prod",
                                              name=f"prod_{hp}_{tq}")
                            nc.vector.tensor_scalar(prod[:], qt[:, tsl],
                                                    kdp[:, hp:hp + 1], None, OP.mult)
                            sprd = ph4sb.tile([P, 512], F32, tag="sprd",
                                              name=f"sprd_{hp}_{tq}")
                            nc.vector.tensor_scalar(sprd[:], qt[:, tsl],
                                                    kdps[:, hp:hp + 1], None, OP.mult)
                            denA = ph4sb.tile([P, 512], F32, tag="denA",
                                              name=f"denA_{hp}_{tq}")
                            denB = ph4sb.tile([P, 512], F32, tag="denB",
                                              name=f"denB_{hp}_{tq}")
                            nc.gpsimd.partition_all_reduce(
                                denA[:], prod[:], channels=128,
                                reduce_op=bass_isa.ReduceOp.add)
                            nc.gpsimd.partition_all_reduce(
                                denB[:], sprd[:], channels=128,
                                reduce_op=bass_isa.ReduceOp.add)
                            nc.vector.tensor_tensor(denA[0:64, :], denA[0:64, :],
                                                    denB[0:64, :], OP.add)
                            nc.vector.tensor_tensor(denA[64:128, :], denA[64:128, :],
                                                    denB[64:128, :], OP.subtract)
                            nc.vector.reciprocal(denA[:], denA[:])
                            qbf = ph4sb.tile([P, 512], BF16, tag="qbf", name=f"qbf_{hp}_{tq}")
                            nc.vector.tensor_copy(qbf[:], qt[:, tsl])
                            aps = ph4ps_a.tile([P, 512], F32, tag="aps", name=f"aps_{hp}_{tq}")
                            nc.tensor.matmul(aps[:], kvq[:, hp, :], qbf[:],
                                             start=True, stop=True)
                            nc.vector.tensor_tensor(at_sb[:, hp, tsl], aps[:], denA[:], OP.mult)

                    # depth-2 software pipeline: attention(hp) runs two Q chunks
                    # behind, so the AllReduce hides under ~3 Q projections.
                    # Software pipeline: depth 2 while the AllReduce is in
                    # flight, catching up to depth 1 at hp=4 so only
                    # attention(7)'s den chain trails into phase 5.
                    att_sched = {2: [0, 1], 3: [2], 4: [3], 5: [4], 6: [5], 7: [6]}
                    q_proj(0)
                    for hp in range(1, KC):
                        if hp + 1 < KC:
                            load_wq(hp + 1)
                        q_proj(hp)
                        for a in att_sched.get(hp, []):
                            attention(a, pe_den=(a <= 1 or a >= KC - 2))
                    attention(KC - 1, pe_den=True)

                if DEBUG_TAPS:
                    with tc.tile_pool(name="dbgat", bufs=2) as dbgat:
                        for c in range(KC):
                            atf = dbgat.tile([P, TOK], F32, tag="atf", name=f"atf_{c}")
                            nc.vector.tensor_copy(atf[:], at_sb[:, c, :])
                            nc.sync.dma_start(dbg_at.ap()[:, c, :], atf[:])

                # ===== Phase 5: output projection + residual + LayerNorm =====
                with (
                    tc.tile_pool(name="ph5ps", bufs=4, space="PSUM") as ph5ps,
                    tc.tile_pool(name="ph5sb", bufs=4) as ph5sb,
                ):
                    for t in range(TC):
                        ts = slice(t * P, (t + 1) * P)
                        y = ph5sb.tile([P, D], F32, tag="y", name=f"y_{t}")
                        xr = ph5sb.tile([P, D], F32, tag="xr", name=f"xr_{t}")
                        nc.sync.dma_start(xr[:], xres.ap()[ts, :])
                        ops = ph5ps.tile([P, D], F32, tag="ops", name=f"ops_{t}")
                        for dh in range(2):
                            dsl = slice(dh * 512, (dh + 1) * 512)
                            for c in range(KC):
                                nc.tensor.matmul(ops[:, dsl], at_sb[:, c, ts], woh_sb[:, c, dsl],
                                                 start=(c == 0), stop=(c == KC - 1))
                        nc.vector.tensor_tensor(y[:], ops[:], xr[:], OP.add)
                        if apply_bias:
                            nc.vector.tensor_tensor(y[:], y[:], bo_b[:], OP.add)
                        stats = ph5sb.tile([P, 2, 6], F32, tag="stats", name=f"stats_{t}")
                        nc.vector.bn_stats(out=stats[:, 0, :], in_=y[:, :512])
                        nc.vector.bn_stats(out=stats[:, 1, :], in_=y[:, 512:])
                        mv = ph5sb.tile([P, 2], F32, tag="mv", name=f"mv_{t}")
                        nc.vector.bn_aggr(out=mv[:], in_=stats[:])
                        nc.scalar.activation(out=mv[:, 1:2], in_=mv[:, 1:2], func=AF.Sqrt,
                                             bias=eps_sb[:], scale=1.0)
                        nc.vector.reciprocal(mv[:, 1:2], mv[:, 1:2])
                        yo = ph5sb.tile([P, D], F32, tag="yo", name=f"yo_{t}")
                        nc.gpsimd.tensor_scalar(yo[:], y[:], mv[:, 0:1], mv[:, 1:2],
                                                OP.subtract, OP.mult)
                        if apply_gamma:
                            nc.vector.tensor_tensor(yo[:], yo[:], gamma_b[:], OP.mult)
                        if apply_beta:
                            nc.vector.tensor_tensor(yo[:], yo[:], beta_b[:], OP.add)
                        nc.sync.dma_start(out_d.ap()[ts, :], yo[:])

            wqp_cm.__exit__(None, None, None)

    nc.compile()
    return nc


def kernel(x, Wq, bq, Wk, bk, Wv, bv, Wo, bo, gamma, beta):
    global LAST_RESULT
    x = np.asarray(x, dtype=np.float32)
    f32 = np.float32
    bf16 = ml_dtypes.bfloat16

    apply_bias = any(np.any(np.asarray(b)) for b in (bq, bk, bv, bo))
    apply_gamma = not np.all(np.asarray(gamma) == 1.0)
    apply_beta = bool(np.any(np.asarray(beta)))

    nc = _build(apply_bias, apply_gamma, apply_beta)

    def split(W):
        W = np.asarray(W, dtype=f32)
        hi = W.astype(bf16)
        lo = (W - hi.astype(f32)).astype(bf16)
        return hi, lo

    wq_h, wq_l = split(Wq)
    wk_f = np.ascontiguousarray(np.asarray(Wk, dtype=f32))
    wv_f = np.ascontiguousarray(np.asarray(Wv, dtype=f32))
    wo_h, _ = split(Wo)
    e_sel = np.zeros((2, P), dtype=bf16)
    e_sel[0, :64] = 1
    e_sel[1, 64:] = 1

    in_maps = []
    for c in range(NCORES):
        b, half = c // 2, c % 2
        xs = x[b, half * TOK:(half + 1) * TOK]          # [2048, 1024]
        xhi = xs.astype(bf16)
        xlo = (xs - xhi.astype(f32)).astype(bf16)
        m = {
            "xtf": np.ascontiguousarray(xs.T),
            "xthi": np.ascontiguousarray(xhi.T),
            "xtlo": np.ascontiguousarray(xlo.T),
            "wqh": wq_h, "wql": wq_l,
            "wk": wk_f, "wv": wv_f, "woh": wo_h,
            "xres": np.ascontiguousarray(xs),
            "e_sel": e_sel,
        }
        if apply_bias:
            m.update(bq=np.asarray(bq, f32), bk=np.asarray(bk, f32),
                     bv=np.asarray(bv, f32), bo=np.asarray(bo, f32))
        if apply_gamma:
            m["gamma"] = np.asarray(gamma, f32)
        if apply_beta:
            m["beta"] = np.asarray(beta, f32)
        in_maps.append(m)

    import os
    try:
        LAST_RESULT = run_bass_kernel_spmd(nc, in_maps, core_ids=list(range(NCORES)))
    except ModuleNotFoundError:
        # no antenv.axon_hooks in this container -> NTFF tracing unavailable
        os.environ["BASS_NEVER_TRACE"] = "1"
        LAST_RESULT = run_bass_kernel_spmd(nc, in_maps, core_ids=list(range(NCORES)))
    out = np.empty((B, N, D), dtype=np.float32)
    for c in range(NCORES):
        b, half = c // 2, c % 2
        out[b, half * TOK:(half + 1) * TOK] = LAST_RESULT.results[c]["out"]
    return out
